# revision 4
# baseline (speedup 1.0000x reference)
"""Trainium2 Bass kernel for nn_MultiHeadAttention_72069551227273 (v2).

Reference computation (B=2, S=2048, D_MODEL=D_EMB=1024, H=16, d_k=64):
    q_p = q @ W_in + b_in                    (shared input projection)
    qh  = heads(q_p @ Wq + bq)               (per-head projections)
    s   = qh @ kh^T / sqrt(d_k), causal-masked softmax
    out = (attn @ vh, concat heads) @ Wo + bo

Sharding: 8 cores = 2 (batch) x 4 (head groups of 4 heads / 256 emb cols).
Per core the input and head projections are fused on device:
    Q = q @ (W_in @ Wq_slice) + (b_in @ Wq_slice + bq_slice)
The whole score path (W_in@Wq/Wk weight combine, Q/K projections, QK^T)
runs in fp8e4m3 DoubleRow perf mode with contraction chunks paired into
the two DoubleRow slots: 4x MACs/cycle over bf16 for combine+projections
and 2x for scores (score lhsT slots = (K, K) via a stride-0 broadcast,
rhs slots = (Q, 0)).  fp8 score noise washes out in the softmax; the
value path (V, attn@V, Wo) stays bf16 — fp8 there measured 2.4e-2
relative error, over the 2e-2 gate.  V is projected straight into the
natural [seq, head, d_k] layout (no PE transposes) with its bias folded
in via a rank-1 ones matmul.  Softmax is exp(s/8) without max-subtraction;
the denominator comes free from a ones column appended to V.  Fully-masked
score blocks are skipped at trace time, diagonal blocks get an on-chip
triangular mask.  Projection work for chunk c+1 is interleaved into the
attention steps of chunk c (attention is Act/exp-heavy, projections are
PE-heavy), and attention itself is software-pipelined so PV of step i-1
overlaps the exp of step i.

Output stage is sequence-parallel instead of tensor-parallel: after
attention chunk c, an 8-way AllToAll (bf16, 256 KB) redistributes the
attention outputs so every core holds all 1024 features for 64 q rows of
each batch, then applies the full Wo locally — there is no reduction
collective at all.  The per-chunk collectives overlap with the next
chunk's attention; only the last chunk's exchange is exposed.
"""

import sys

sys.path.append("/opt/trn_rl_repo")

import math
from contextlib import ExitStack

import numpy as np

import concourse.bass as bass
import concourse.bacc as bacc
import concourse.mybir as mybir
import concourse.tile as tile
from concourse import bass_utils
from concourse.bass_interp import get_hw_module

# problem dims
B, S, DM, DE, H, DK = 2, 2048, 1024, 1024, 16, 64
N_CORES = 8
P = 128                      # partitions
QC = 512                     # q chunk (psum bank width in fp32)
KB = 128                     # k block (scores^T partition block)
GW = 2                       # kb blocks per score-psum tile (2 banks)
PE_WARMUP = 7               # dummy matmuls to ramp the PE clock at t=0

F32 = mybir.dt.float32
BF16 = mybir.dt.bfloat16
FP8 = mybir.dt.float8e4

FULL, TRI, GEN, SKIP = 0, 1, 2, 3


def make_plan(mask_np, s=S, qc=QC, kb=KB):
    """Classify scores^T blocks [kb x qc] from the (B, S, S) 0/1 mask.

    Returns (blocks, n_gen_tiles, gen_tiles_per_batch):
      blocks[iqc] = list of (ikb, mode, arg)
    """
    nqc, nkb = s // qc, s // kb
    m = np.asarray(mask_np) != 0          # [B, S(q), S(k)] True = attend
    tril = np.tril(np.ones((s, s), bool))
    causal = all(np.array_equal(m[b], tril) for b in range(m.shape[0]))
    blocks = []
    if causal:
        for iqc in range(nqc):
            row = []
            for ikb in range(nkb):
                if (ikb + 1) * kb <= iqc * qc:
                    row.append((ikb, FULL, 0))
                elif ikb * kb < (iqc + 1) * qc:
                    row.append((ikb, TRI, (ikb * kb - iqc * qc) // kb))
                # else fully masked -> skip
            blocks.append(row)
        return blocks, 0, None

    # general path: per-block classification, unioned across batches
    nb = m.shape[0]
    # every query row must attend to >= 1 key (else softmax semantics differ)
    assert m.any(axis=-1).all(), "fully-masked query rows unsupported"
    gen_tiles = [[] for _ in range(nb)]
    for iqc in range(nqc):
        row = []
        for ikb in range(nkb):
            sub = m[:, iqc * qc:(iqc + 1) * qc, ikb * kb:(ikb + 1) * kb]
            if sub.all():
                row.append((ikb, FULL, 0))
            elif not sub.any():
                continue
            else:
                idx = len(gen_tiles[0])
                for b in range(nb):
                    gen_tiles[b].append(sub[b].T.astype(np.int32))  # [kb, qc]
                row.append((ikb, GEN, idx))
        blocks.append(row)
    n_gen = len(gen_tiles[0])
    gt = [np.stack(g) if n_gen else np.zeros((1, kb, qc), np.int32)
          for g in gen_tiles]
    return blocks, n_gen, gt


def build_mha(blocks, n_gen, *, s=S, dm=DM, de=DE, dh=None, mm="fp8",
              collective=True, chunked_cc=True, cc_reps=1):
    """Trace the per-core MHA program.  dh = per-core emb slice (256)."""
    if dh is None:
        dh = DE // 4
    nqc, nkb, ndm, nde = s // QC, s // KB, dm // P, de // P
    ndh = dh // P            # feature chunks per core (2)
    hloc = dh // DK          # heads per core (4)
    nsub = QC // 64          # a2a sub-blocks per chunk (8)
    out_rows = nqc * P       # output rows per core (4 chunks x 2 x 64)

    qk8 = mm in ("fp8", "fp8qk")
    pv8 = (mm == "fp8")
    qkt = FP8 if qk8 else BF16
    pvt = FP8 if pv8 else BF16   # dtype of probs, V, and mask tiles
    st = BF16

    # can attention chunk c start right after projection chunk c?
    causal_dep = all(
        max([c] + [ikb * KB // QC for (ikb, _, _) in blocks[c]]) <= c
        for c in range(nqc))

    nc = bacc.Bacc("TRN2", target_bir_lowering=False, debug=False,
                   num_devices=N_CORES)

    # ---- kernel I/O (per core) ----
    qT = nc.dram_tensor("qT", [dm, s], qkt, kind="ExternalInput")
    kT = nc.dram_tensor("kT", [dm, s], qkt, kind="ExternalInput")
    vT = nc.dram_tensor("vT", [dm, s], BF16, kind="ExternalInput")
    w_inT = nc.dram_tensor("w_inT", [de, dm], BF16, kind="ExternalInput")
    w_inT8 = nc.dram_tensor("w_inT8", [de, dm], qkt, kind="ExternalInput")
    wq = nc.dram_tensor("wq", [de, dh], BF16, kind="ExternalInput")
    wk = nc.dram_tensor("wk", [de, dh], BF16, kind="ExternalInput")
    wq8 = nc.dram_tensor("wq8", [de, dh], qkt, kind="ExternalInput")
    wk8 = nc.dram_tensor("wk8", [de, dh], qkt, kind="ExternalInput")
    wv = nc.dram_tensor("wv", [de, dh], BF16, kind="ExternalInput")
    wo = nc.dram_tensor("wo", [de, dm], BF16, kind="ExternalInput")
    b_in = nc.dram_tensor("b_in", [de], BF16, kind="ExternalInput")
    bq = nc.dram_tensor("bq", [dh], F32, kind="ExternalInput")
    bk = nc.dram_tensor("bk", [dh], F32, kind="ExternalInput")
    bv = nc.dram_tensor("bv", [dh], F32, kind="ExternalInput")
    bo = nc.dram_tensor("bo", [dm], F32, kind="ExternalInput")
    m_tiles = nc.dram_tensor("m_tiles", [max(n_gen, 1), KB, QC], mybir.dt.int32,
                             kind="ExternalInput")
    y_out = nc.dram_tensor("y_out", [out_rows, dm], F32, kind="ExternalOutput")

    # a2a staging: chunked mode [chunk][8 dest blocks][256 feats][64 q];
    # single mode [8 dest blocks][chunk][256 feats][64 q] (one collective)
    if chunked_cc:
        a2a_in = nc.dram_tensor("a2a_in", [nqc, nsub, dh, 64], BF16)
        a2a_out = nc.dram_tensor("a2a_out", [nqc, nsub, dh, 64], BF16)
    else:
        a2a_in = nc.dram_tensor("a2a_in", [nsub, nqc, dh, 64], BF16)
        a2a_out = nc.dram_tensor("a2a_out", [nsub, nqc, dh, 64], BF16)

    with tile.TileContext(nc) as tc, ExitStack() as ex:
        persist = ex.enter_context(tc.tile_pool(name="persist", bufs=1))
        work = ex.enter_context(tc.tile_pool(name="work", bufs=3))
        ps_w = ex.enter_context(tc.tile_pool(name="ps_w", bufs=2, space="PSUM"))
        ps_s = ex.enter_context(tc.tile_pool(name="ps_s", bufs=2, space="PSUM"))
        ps_o = ex.enter_context(tc.tile_pool(name="ps_o", bufs=2, space="PSUM"))
        qbufs = 2 if causal_dep else nqc
        xpool = ex.enter_context(tc.tile_pool(name="xpool", bufs=3))
        qpool = ex.enter_context(tc.tile_pool(name="qpool", bufs=qbufs))
        ppool = ex.enter_context(tc.tile_pool(name="ppool", bufs=6))
        cpool = ex.enter_context(tc.tile_pool(name="cpool", bufs=2))
        ypool = ex.enter_context(tc.tile_pool(name="ypool", bufs=2))
        wpool = ex.enter_context(tc.tile_pool(name="wpool", bufs=1))

        # ---- constants ----
        # tri[k, q] = 1.0 where k <= q (keep), else 0
        tri = persist.tile([P, P], pvt, tag="tri", name="tri")
        tri_b = persist.tile([P, P], st, tag="tri_b", name="tri_b")
        nc.gpsimd.memset(tri_b[:], 0.0)
        nc.gpsimd.affine_select(out=tri_b[:], in_=tri_b[:],
                                compare_op=mybir.AluOpType.is_gt,
                                fill=1.0, base=0,
                                pattern=[[-1, P]], channel_multiplier=1)
        if pvt == st:
            tri = tri_b
        else:
            nc.vector.tensor_copy(tri[:], tri_b[:])
        ones1 = persist.tile([1, P], st, tag="ones1", name="ones1")
        nc.gpsimd.memset(ones1[:], 1.0)
        # preload the Exp table while DMAs stream in
        actwarm = persist.tile([1, 1], F32, tag="actwarm", name="actwarm")
        nc.scalar.activation(actwarm[:], ones1[0:1, 0:1],
                             mybir.ActivationFunctionType.Exp)

        gen_sb = None
        if n_gen:
            gen_sb = persist.tile([P, n_gen, QC], pvt, tag="gen", name="gen")
            gi = persist.tile([P, n_gen, QC], mybir.dt.int32, tag="gen_i",
                              name="gen_i")
            nc.sync.dma_start(gi[:], m_tiles[:].rearrange("n p q -> p n q"))
            for i in range(n_gen):
                if pvt == st:
                    nc.vector.tensor_copy(gen_sb[:, i, :], gi[:, i, :])
                else:
                    gb = work.tile([P, QC], st, tag="gen_b", name="gen_b")
                    nc.vector.tensor_copy(gb[:], gi[:, i, :])
                    nc.vector.tensor_copy(gen_sb[:, i, :], gb[:])

        DR0 = mybir.MatmulPerfMode.DoubleRow

        # ---- persistent activation storage (memsets run at t=0) ----
        kT_sb = [persist.tile([P, s], qkt, tag=f"kT{t}", name=f"kT{t}")
                 for t in range(ndh)]
        # V in natural layout, heads side by side, with a ones column:
        # v_all[kb_row, ikb, h, 0:DK] = v_h[key, :], v_all[.., DK] = 1
        # fp8 DoubleRow ldweights needs 4-byte-aligned slot strides: pad
        # each head's [d_k | ones] slot to VW columns (tail zeroed)
        VW = DK + 4 if pv8 else DK + 1
        v_all = persist.tile([P, nkb, hloc, VW], pvt, tag="v_all",
                             name="v_all")
        nc.gpsimd.memset(v_all[:, :, :, DK], 1.0)
        if VW > DK + 1:
            nc.gpsimd.memset(v_all[:, :, :, DK + 1:VW], 0.0)

        qf_tiles = {}

        def make_qf(iqc, memset=False):
            qf = [qpool.tile([P, 2, QC], qkt, tag=f"qf{t}", name=f"qf{t}_{iqc}")
                  for t in range(ndh)]
            qf_tiles[iqc] = qf
            if memset and qk8:
                # pool ring: zero slots persist across later buffer reuse
                for t in range(ndh):
                    nc.vector.memset(qf[t][:, 1, :], 0.0)
            return qf

        for c in range(qbufs):
            make_qf(c, memset=True)

        # ---- load weights (bf16 from host; batched DMAs) ----
        # spread DMAs across both HWDGE queues (SP + Activation)
        _dmaq = [0]

        def dmaq():
            _dmaq[0] ^= 1
            return nc.sync if _dmaq[0] else nc.scalar

        # wq first, then w_inT in quarters: the first combine matmuls only
        # need w_in chunk u=0 + wq, so PE can start ~2.7us in
        w_sb = {}
        w8_sb = {}
        if qk8:
            # fp8 copies drive the DoubleRow q/k combine (4x MACs/cycle)
            wb_q8 = wpool.tile([P, nde, dh], qkt, tag="wq8", name="wq8_b")
            nc.sync.dma_start(out=wb_q8[:],
                              in_=wq8[:].rearrange("(u p) d -> p u d", p=P))
            w8_sb["q"] = wb_q8
            w_in8_b = wpool.tile([P, nde, dm], qkt, tag="w_in8", name="w_in8_b")
            hd8 = nde // 2
            for i in range(2):
                dmaq().dma_start(
                    out=w_in8_b[:, i * hd8:(i + 1) * hd8, :],
                    in_=w_inT8[i * hd8 * P:(i + 1) * hd8 * P, :]
                        .rearrange("(u p) m -> p u m", p=P))
            wb_k8 = wpool.tile([P, nde, dh], qkt, tag="wk8", name="wk8_b")
            dmaq().dma_start(out=wb_k8[:],
                             in_=wk8[:].rearrange("(u p) d -> p u d", p=P))
            w8_sb["k"] = wb_k8
        w_inT_b = wpool.tile([P, nde, dm], st, tag="w_inT", name="w_inT_b")
        w_inT_sb = [w_inT_b[:, u, :] for u in range(nde)]
        b_inT = wpool.tile([P, nde], st, tag="b_inT", name="b_inT")
        bo_bcast = persist.tile([P, dm], F32, tag="bo_b", name="bo_bcast")
        for name in ("q", "k", "v"):
            wb = wpool.tile([P, nde, dh], st, tag=f"w{name}", name=f"w{name}_b")
            w_sb[name] = [wb[:, u, :] for u in range(nde)]
            w_sb[name + "_t"] = wb

        def load_w_small(name):
            # bf16 head-projection weights (bias combine) + b_in
            def f():
                dmaq().dma_start(
                    out=w_sb[name + "_t"][:],
                    in_={"q": wq, "k": wk, "v": wv}[name]
                        .rearrange("(u p) d -> p u d", p=P))
                if name == "q":
                    nc.scalar.dma_start(
                        out=b_inT[:], in_=b_in[:].rearrange("(t p) -> p t", p=P))
            return f

        def load_w_inT_bf16():
            hd4 = nde // 4
            for i in range(4):
                dmaq().dma_start(
                    out=w_inT_b[:, i * hd4:(i + 1) * hd4, :],
                    in_=w_inT[i * hd4 * P:(i + 1) * hd4 * P, :]
                        .rearrange("(u p) m -> p u m", p=P))
            nc.scalar.dma_start(out=bo_bcast[:],
                                in_=bo[:].unsqueeze(0).broadcast_to([P, dm]))
        wo_sb = persist.tile([P, nde, dm], st, tag="wo", name="wo_b")

        def load_wo():
            # deferred: wo is not needed until the first output projection
            hdo = nde // 2
            nc.sync.dma_start(out=wo_sb[:, 0:hdo, :],
                              in_=wo[0:hdo * P, :].rearrange("(u p) m -> p u m", p=P))
            nc.scalar.dma_start(out=wo_sb[:, hdo:nde, :],
                                in_=wo[hdo * P:, :].rearrange("(u p) m -> p u m", p=P))

        # ---- combine weights: Wc_x = W_in @ Wx (+ bias fold) ----
        # q/k: fp8 DoubleRow over paired de-chunks -> paired-layout wc8
        # (wc8[name][t//2][:, t%2, :] = Wc rows of dm-chunk t); v: bf16
        wc = {}
        wc8 = {}
        bc = {}

        def combine_qk8(name):
            wc8[name] = [persist.tile([P, 2, dh], qkt, tag=f"wc8{name}{t}",
                                      name=f"wc8{name}{t}")
                         for t in range(ndm // 2)]
            for tp in range(ndm // 2):
                ps = ps_w.tile([P, 2 * dh], F32, tag="ps_w", name="ps_w")
                for half in range(2):
                    t = 2 * tp + half
                    for i in range(nde // 2):
                        nc.tensor.matmul(
                            ps[:, half * dh:(half + 1) * dh],
                            w_in8_b[:, 2 * i:2 * i + 2, t * P:(t + 1) * P],
                            w8_sb[name][:, 2 * i:2 * i + 2, :],
                            perf_mode=DR0,
                            start=(i == 0), stop=(i == nde // 2 - 1))
                nc.vector.tensor_copy(
                    wc8[name][tp][:],
                    ps[:].rearrange("p (two d) -> p two d", two=2))

        def combine_bf16_closures(name):
            wc[name] = [persist.tile([P, dh], st, tag=f"wc{name}{t}",
                                     name=f"wc{name}{t}") for t in range(ndm)]

            def piece(ts_):
                def f():
                    for t in ts_:
                        ps = ps_w.tile([P, dh], F32, tag="ps_w", name="ps_w")
                        for u in range(nde):
                            nc.tensor.matmul(
                                ps[:], w_inT_sb[u][:, t * P:(t + 1) * P],
                                w_sb[name][u][:],
                                start=(u == 0), stop=(u == nde - 1))
                        nc.vector.tensor_copy(wc[name][t][:], ps[:])
                return f
            return [piece(ts_) for ts_ in
                    ([0, 1], [2, 3], [4, 5], [6, 7])]

        def bias_qk(name):
            bvec = {"q": bq, "k": bk}[name]
            bxT = wpool.tile([P, ndh], F32, tag=f"bxT{name}", name=f"bxT{name}")
            nc.sync.dma_start(out=bxT[:], in_=bvec[:].rearrange("(t p) -> p t", p=P))
            bc[name] = persist.tile([P, ndh], F32, tag=f"bc{name}", name=f"bc{name}")
            for t in range(ndh):
                ps = ps_w.tile([P, 1], F32, tag="ps_w", name="ps_w")
                for u in range(nde):
                    nc.tensor.matmul(
                        ps[:], w_sb[name][u][:, t * P:(t + 1) * P],
                        b_inT[:, u:u + 1],
                        start=(u == 0), stop=(u == nde - 1))
                nc.vector.tensor_add(bc[name][:, t:t + 1], ps[:], bxT[:, t:t + 1])

        bcv_row = persist.tile([1, dh], st, tag="bcv", name="bcv_row")

        def bias_v():
            bv_row = wpool.tile([1, dh], F32, tag="bv_row", name="bv_row")
            nc.sync.dma_start(out=bv_row[:], in_=bv[:].unsqueeze(0))
            ps = ps_w.tile([1, dh], F32, tag="ps_w", name="ps_w")
            for u in range(nde):
                nc.tensor.matmul(ps[:], b_inT[:, u:u + 1], w_sb["v"][u][:],
                                 start=(u == 0), stop=(u == nde - 1))
            nc.vector.tensor_add(bcv_row[:], ps[:], bv_row[:])

        def proj_closures(iqc):
            """Per-chunk projection emission, split into PE-sized closures."""
            clos = []
            qf = qf_tiles.get(iqc) or make_qf(iqc, memset=iqc < qbufs)
            xbs = {}

            def load(name, xdram):
                def f():
                    dt_ = qkt if (qk8 and name in ("q", "k")) else st
                    tag = "xb8" if (qk8 and name in ("q", "k")) else "xb"
                    xb = xpool.tile([P, ndm, QC], dt_, tag=tag,
                                    name=f"xb_{name}{iqc}")
                    xbs[name] = xb
                    dmaq().dma_start(
                        out=xb[:],
                        in_=xdram[:, iqc * QC:(iqc + 1) * QC]
                            .rearrange("(u p) s -> p u s", p=P))
                return f

            def qk_part(name, t):
                def f():
                    xb = xbs[name]
                    ps = ps_w.tile([P, QC], F32, tag="ps_w", name="ps_w")
                    if qk8:
                        for i in range(ndm // 2):
                            nc.tensor.matmul(
                                ps[:],
                                wc8[name][i][:, :, t * P:(t + 1) * P],
                                xb[:, 2 * i:2 * i + 2, :],
                                perf_mode=DR0,
                                start=(i == 0), stop=(i == ndm // 2 - 1))
                    else:
                        for u in range(ndm):
                            nc.tensor.matmul(
                                ps[:], wc[name][u][:, t * P:(t + 1) * P],
                                xb[:, u, :], start=(u == 0),
                                stop=(u == ndm - 1))
                    if name == "k":
                        nc.vector.tensor_scalar_add(
                            kT_sb[t][:, iqc * QC:(iqc + 1) * QC], ps[:],
                            bc["k"][:, t:t + 1])
                    else:
                        nc.vector.tensor_scalar_add(
                            qf[t][:, 0, :], ps[:], bc["q"][:, t:t + 1])
                return f

            def v_part(j):
                def f():
                    xb = xbs["v"]
                    ikb = iqc * (QC // P) + j
                    ps = ps_w.tile([P, dh], F32, tag="ps_w", name="ps_w")
                    for u in range(ndm):
                        nc.tensor.matmul(
                            ps[:], xb[:, u, j * P:(j + 1) * P], wc["v"][u][:],
                            start=(u == 0), stop=False)
                    nc.tensor.matmul(ps[:], ones1[:], bcv_row[:],
                                     start=False, stop=True)
                    nc.vector.tensor_copy(
                        v_all[:, ikb, :, 0:DK],
                        ps[:].rearrange("p (h d) -> p h d", h=hloc))
                return f

            clos.append(load("q", qT))
            for t in range(ndh):
                clos.append(qk_part("q", t))
            clos.append(load("k", kT))
            for t in range(ndh):
                clos.append(qk_part("k", t))
            clos.append(load("v", vT))
            for j in range(QC // P):
                clos.append(v_part(j))
            return clos

        # ---- attention ----
        inv_sqrt = 1.0 / math.sqrt(DK)
        DR = mybir.MatmulPerfMode.DoubleRow
        cT_tiles = {}

        def attention_chunk(iqc, fillers=(), mid=None):
            """QK+exp of step i overlaps PV of step i-1; `fillers` (next
            chunk's projection closures) are spread over the early steps;
            `mid` (the previous chunk's yin load) fires ~70% through."""
            qf = qf_tiles[iqc]
            cT = cpool.tile([P, ndh, QC], st, tag="cT", name=f"cT{iqc}")
            cT_tiles[iqc] = cT
            blist = blocks[iqc]
            steps = []
            for h in range(hloc):
                grps = [blist[g0:g0 + GW] for g0 in range(0, len(blist), GW)]
                for g in range(len(grps)):
                    steps.append((h, grps[g], g == 0, g == len(grps) - 1))
            po = {}
            pending = []
            fillers = list(fillers)
            n_steps = len(steps)
            fill_at = {}
            if fillers:
                # spread fillers uniformly across the steps
                for fi in range(len(fillers)):
                    at = (fi * n_steps) // len(fillers)
                    fill_at.setdefault(min(at, n_steps - 1), []).append(
                        fillers[fi])
            mid_at = (7 * n_steps) // 10

            def emit_qk_exp(h, grp):
                t, off = h // 2, (h % 2) * DK
                pss = ps_s.tile([P, GW * QC], F32, tag="ps_scores",
                                name="ps_scores")
                for j, (ikb, mode, arg) in enumerate(grp):
                    kv = kT_sb[t][off:off + DK, ikb * KB:(ikb + 1) * KB]
                    if qk8:
                        nc.tensor.matmul(
                            pss[:, j * QC:(j + 1) * QC],
                            kv.unsqueeze(1).broadcast_to([DK, 2, KB]),
                            qf[t][off:off + DK, :, :],
                            perf_mode=DR, start=True, stop=True)
                    else:
                        nc.tensor.matmul(pss[:, j * QC:(j + 1) * QC],
                                         kv, qf[t][off:off + DK, 0, :])
                pt = ppool.tile([P, GW * QC], pvt, tag="p", name="p")
                nw = len(grp) * QC
                nc.scalar.activation(pt[:, 0:nw], pss[:, 0:nw],
                                     mybir.ActivationFunctionType.Exp,
                                     scale=inv_sqrt)
                for j, (ikb, mode, arg) in enumerate(grp):
                    pj = pt[:, j * QC:(j + 1) * QC]
                    if mode == TRI:
                        r = arg
                        if r > 0:
                            nc.gpsimd.memset(pj[:, 0:r * P], 0.0)
                        nc.vector.tensor_mul(
                            pj[:, r * P:(r + 1) * P],
                            pj[:, r * P:(r + 1) * P], tri[:])
                    elif mode == GEN:
                        nc.vector.tensor_mul(pj[:], pj[:], gen_sb[:, arg, :])
                return pt

            def emit_pv(h, grp, pt, first, last):
                if first:
                    po[h] = ps_o.tile([VW, QC], F32, tag="ps_av",
                                      name="ps_av")
                ikbs = [ikb for (ikb, _, _) in grp]
                if pv8 and len(grp) == 2 and ikbs[1] == ikbs[0] + 1:
                    nc.tensor.matmul(
                        po[h][:], v_all[:, ikbs[0]:ikbs[0] + 2, h, :],
                        pt[:].rearrange("p (two q) -> p two q", two=2),
                        perf_mode=DR,
                        start=first, stop=last)
                else:
                    for j, (ikb, mode, arg) in enumerate(grp):
                        nc.tensor.matmul(
                            po[h][:, 0:QC], v_all[:, ikb, h, :],
                            pt[:, j * QC:(j + 1) * QC],
                            start=(first and j == 0),
                            stop=(last and j == len(grp) - 1))
                if last:
                    rec1 = work.tile([1, QC], F32, tag="rec1", name="rec1")
                    nc.vector.reciprocal(rec1[:], po[h][DK:DK + 1, :])
                    recb = work.tile([DK, QC], F32, tag="recb", name="recb")
                    nc.gpsimd.partition_broadcast(recb[:], rec1[:])
                    nc.vector.tensor_mul(
                        cT[(h % 2) * DK:(h % 2) * DK + DK, h // 2, :],
                        po[h][0:DK, :], recb[:])
                    del po[h]

            for i, (h, grp, first, last) in enumerate(steps):
                pt = emit_qk_exp(h, grp)
                pending.append((h, grp, pt, first, last))
                if len(pending) > 3:
                    emit_pv(*pending.pop(0))
                for f in fill_at.get(i, ()):
                    f()
                if mid is not None and i == mid_at:
                    mid()
                    mid = None
            for p_ in pending:
                emit_pv(*p_)
            if mid is not None:
                mid()

        # ---- a2a + output projection ----
        def a2a_chunk(c):
            # cT [128, ndh, QC] -> a2a_in[c or :, c] [8, 256, 64]
            cT = cT_tiles[c]
            dst = a2a_in[c] if chunked_cc else a2a_in[:, c]
            for fh in range(ndh):
                nc.sync.dma_start(
                    out=dst[:, fh * P:(fh + 1) * P, :]
                        .rearrange("r p j -> p r j"),
                    in_=cT[:, fh, :].rearrange("p (r j) -> p r j", r=nsub))
            if not chunked_cc:
                if c == nqc - 1:
                    if collective:
                        for _ in range(cc_reps):
                            nc.gpsimd.collective_compute(
                                "AllToAll", mybir.AluOpType.bypass,
                                replica_groups=[list(range(N_CORES))],
                                ins=[a2a_in[:].opt()], outs=[a2a_out[:].opt()])
                    else:
                        nc.sync.dma_start(out=a2a_out[:], in_=a2a_in[:])
                return
            if collective:
                for _ in range(cc_reps):
                    nc.gpsimd.collective_compute(
                        "AllToAll", mybir.AluOpType.bypass,
                        replica_groups=[list(range(N_CORES))],
                        ins=[a2a_in[c].opt()], outs=[a2a_out[c].opt()])
            else:
                nc.sync.dma_start(out=a2a_out[c], in_=a2a_in[c])

        yin_tiles = {}

        def yin_load(c):
            yin = ypool.tile([P, nde, P], st, tag="yin", name=f"yin{c}")
            yin_tiles[c] = yin
            half = nsub // 2
            if chunked_cc:
                src = a2a_out[c]
                nc.sync.dma_start(
                    out=yin[:, :, 0:64],
                    in_=src[0:half].rearrange("s (fh p) j -> p (s fh) j", p=P))
                nc.sync.dma_start(
                    out=yin[:, :, 64:128],
                    in_=src[half:nsub].rearrange("s (fh p) j -> p (s fh) j",
                                                 p=P))
            else:
                yv = yin[:].rearrange("p (s fh) j -> p s fh j", fh=ndh)
                for b0, sl in ((0, slice(0, 64)), (half, slice(64, 128))):
                    for fh in range(ndh):
                        nc.sync.dma_start(
                            out=yv[:, :, fh, sl],
                            in_=a2a_out[b0:b0 + half, c, fh * P:(fh + 1) * P, :]
                                .rearrange("s p j -> p s j"))

        def yproj_mm(c):
            yin = yin_tiles[c]
            ys = ypool.tile([P, dm], F32, tag="ys", name=f"ys{c}")
            for mb in range(dm // QC):
                ps = ps_w.tile([P, QC], F32, tag="ps_w", name="ps_w")
                for u in range(nde):
                    nc.tensor.matmul(
                        ps[:], yin[:, u, :],
                        wo_sb[:, u, mb * QC:(mb + 1) * QC],
                        start=(u == 0), stop=(u == nde - 1))
                nc.vector.tensor_add(ys[:, mb * QC:(mb + 1) * QC], ps[:],
                                     bo_bcast[:, mb * QC:(mb + 1) * QC])
            nc.sync.dma_start(out=y_out[c * P:(c + 1) * P, :], in_=ys[:])

        # ---- schedule ----
        # head: score-path (fp8) combine + q/k projections first so the
        # first attention exp fires ~7us in; the bf16 v-combine and chunk-0
        # V projection become PE filler inside attention chunk 0.
        # yproj(c) is deferred into later chunks' attention as PE filler:
        # the final chunks have the most exp work and no projections left.
        if causal_dep:
            pc0 = proj_closures(0)
            pc0[0]()                       # load q chunk 0
            pc0[3]()                       # load k chunk 0
            if qk8:
                load_w_small("q")()
                load_w_small("k")()
                combine_qk8("q")
                bias_qk("q")
                pc0[1](); pc0[2]()         # project q chunk 0
                combine_qk8("k")
                bias_qk("k")
                pc0[4](); pc0[5]()         # project k chunk 0
            else:
                load_w_small("q")()
                load_w_small("k")()
                load_w_inT_bf16()
                for f in combine_bf16_closures("q"):
                    f()
                bias_qk("q")
                pc0[1](); pc0[2]()
                for f in combine_bf16_closures("k"):
                    f()
                bias_qk("k")
                pc0[4](); pc0[5]()
            pc0[6]()                       # load v chunk 0
            load_w_small("v")()
            if qk8:
                load_w_inT_bf16()
            head_fill = combine_bf16_closures("v") + [bias_v]
            load_wo()
            for c in range(nqc):
                fillers = list(head_fill)
                head_fill = []
                if c == 0:
                    fillers += pc0[7:]     # chunk-0 V projection
                if c + 1 < nqc:
                    fillers += list(proj_closures(c + 1))
                if chunked_cc and c == nqc - 1:
                    for cc in range(nqc - 2):
                        fillers.append((lambda c2: lambda: yproj_mm(c2))(cc))
                mid = ((lambda cc: lambda: yin_load(cc))(c - 1)
                       if c > 0 and chunked_cc else None)
                attention_chunk(c, fillers=fillers, mid=mid)
                a2a_chunk(c)
            if chunked_cc:
                yproj_mm(nqc - 2)
            else:
                for c in range(nqc - 1):
                    yin_load(c)
                    yproj_mm(c)
        else:
            # general masks: all projections first, then attention
            load_w_small("q")()
            load_w_small("k")()
            load_w_small("v")()
            load_w_inT_bf16()
            if qk8:
                combine_qk8("q")
                combine_qk8("k")
            else:
                for f in combine_bf16_closures("q") + combine_bf16_closures("k"):
                    f()
            bias_qk("q")
            bias_qk("k")
            for f in combine_bf16_closures("v"):
                f()
            bias_v()
            for c in range(nqc):
                for f in proj_closures(c):
                    f()
            load_wo()
            for c in range(nqc):
                mid = (lambda cc: lambda: yin_load(cc))(c - 1) if c > 0 else None
                attention_chunk(c, mid=mid)
                a2a_chunk(c)
                if c > 0:
                    yproj_mm(c - 1)
        yin_load(nqc - 1)
        yproj_mm(nqc - 1)

    nc.compile()
    return nc


# ------------------------------------------------------------------
_CACHE = {}


def _get_compiled(plan_key, blocks, n_gen, mm):
    if plan_key not in _CACHE:
        nc = build_mha(blocks, n_gen, mm=mm)
        nc.m = get_hw_module(nc.m)
        _CACHE[plan_key] = nc
    return _CACHE[plan_key]


def make_in_maps(q, k, v, mask, W_in, b_in, Wq, bq, Wk, bk, Wv, bv, Wo, bo,
                 blocks=None, n_gen=None, gen_tiles=None):
    if blocks is None:
        blocks, n_gen, gen_tiles = make_plan(mask)
    bf16 = mybir.dt.np(BF16)
    fp8 = mybir.dt.np(FP8)
    dh = DE // 4
    tb = lambda a: np.ascontiguousarray(np.asarray(a).T).astype(bf16)
    cb = lambda a: np.ascontiguousarray(np.asarray(a)).astype(bf16)
    t8 = lambda a: np.ascontiguousarray(np.asarray(a).T).astype(fp8)
    c8 = lambda a: np.ascontiguousarray(np.asarray(a)).astype(fp8)
    in_maps = []
    for c in range(N_CORES):
        b, g = c // 4, c % 4
        sl = slice(g * dh, (g + 1) * dh)
        mt = (gen_tiles[b] if n_gen else
              np.zeros((1, KB, QC), np.int32))
        qk8 = MM_MODE in ("fp8", "fp8qk")
        tq = t8 if qk8 else tb
        in_maps.append({
            "qT": tq(q[b]), "kT": tq(k[b]), "vT": tb(v[b]),
            "w_inT": tb(W_in), "w_inT8": t8(W_in),
            "wq": cb(Wq[:, sl]),
            "wk": cb(Wk[:, sl]),
            "wq8": c8(Wq[:, sl]),
            "wk8": c8(Wk[:, sl]),
            "wv": cb(Wv[:, sl]),
            "wo": cb(Wo),
            "b_in": np.asarray(b_in).astype(bf16),
            "bq": np.ascontiguousarray(np.asarray(bq)[sl]),
            "bk": np.ascontiguousarray(np.asarray(bk)[sl]),
            "bv": np.ascontiguousarray(np.asarray(bv)[sl]),
            "bo": np.asarray(bo),
            "m_tiles": mt,
        })
    return in_maps, blocks, n_gen


def assemble(results):
    out = np.empty((B, S, DM), np.float32)
    for core in range(N_CORES):
        y = results[core]["y_out"]            # [nqc*128, DM]
        for c in range(S // QC):
            for b in range(B):
                out[b, c * QC + core * 64:c * QC + (core + 1) * 64, :] = \
                    y[c * P + b * 64:c * P + (b + 1) * 64, :]
    return out


MM_MODE = "fp8qk"


def kernel(**inputs):
    mask = inputs["mask"]
    blocks, n_gen, gen_tiles = make_plan(np.asarray(mask))
    plan_key = (str(blocks), n_gen, MM_MODE)
    nc = _get_compiled(plan_key, blocks, n_gen, MM_MODE)
    in_maps, _, _ = make_in_maps(
        inputs["q"], inputs["k"], inputs["v"], mask,
        inputs["W_in"], inputs["b_in"], inputs["Wq"], inputs["bq"],
        inputs["Wk"], inputs["bk"], inputs["Wv"], inputs["bv"],
        inputs["Wo"], inputs["bo"],
        blocks=blocks, n_gen=n_gen, gen_tiles=gen_tiles)
    res = bass_utils.run_bass_kernel_spmd(nc, in_maps,
                                          core_ids=list(range(N_CORES)))
    return assemble(res.results)


# revision 5
# speedup vs baseline: 1.0281x; 1.0281x over previous
"""Trainium2 Bass kernel for nn_MultiHeadAttention_72069551227273 (v2).

Reference computation (B=2, S=2048, D_MODEL=D_EMB=1024, H=16, d_k=64):
    q_p = q @ W_in + b_in                    (shared input projection)
    qh  = heads(q_p @ Wq + bq)               (per-head projections)
    s   = qh @ kh^T / sqrt(d_k), causal-masked softmax
    out = (attn @ vh, concat heads) @ Wo + bo

Sharding: 8 cores = 2 (batch) x 4 (head groups of 4 heads / 256 emb cols).
Per core the input and head projections are fused on device:
    Q = q @ (W_in @ Wq_slice) + (b_in @ Wq_slice + bq_slice)
The whole score path (W_in@Wq/Wk weight combine, Q/K projections, QK^T)
runs in fp8e4m3 DoubleRow perf mode with contraction chunks paired into
the two DoubleRow slots: 4x MACs/cycle over bf16 for combine+projections
and 2x for scores (score lhsT slots = (K, K) via a stride-0 broadcast,
rhs slots = (Q, 0)).  fp8 score noise washes out in the softmax; the
value path (V, attn@V, Wo) stays bf16 — fp8 there measured 2.4e-2
relative error, over the 2e-2 gate.  V is projected straight into the
natural [seq, head, d_k] layout (no PE transposes) with its bias folded
in via a rank-1 ones matmul.  Softmax is exp(s/8) without max-subtraction;
the denominator comes free from a ones column appended to V.  Fully-masked
score blocks are skipped at trace time, diagonal blocks get an on-chip
triangular mask.  Projection work for chunk c+1 is interleaved into the
attention steps of chunk c (attention is Act/exp-heavy, projections are
PE-heavy), and attention itself is software-pipelined so PV of step i-1
overlaps the exp of step i.

Output stage is sequence-parallel instead of tensor-parallel: after
attention chunk c, an 8-way AllToAll (bf16, 256 KB) redistributes the
attention outputs so every core holds all 1024 features for 64 q rows of
each batch, then applies the full Wo locally — there is no reduction
collective at all.  The per-chunk collectives overlap with the next
chunk's attention; only the last chunk's exchange is exposed.
"""

import sys

sys.path.append("/opt/trn_rl_repo")

import math
from contextlib import ExitStack

import numpy as np

import concourse.bass as bass
import concourse.bacc as bacc
import concourse.mybir as mybir
import concourse.tile as tile
from concourse import bass_utils
from concourse.bass_interp import get_hw_module

# problem dims
B, S, DM, DE, H, DK = 2, 2048, 1024, 1024, 16, 64
N_CORES = 8
P = 128                      # partitions
QC = 512                     # q chunk (psum bank width in fp32)
KB = 128                     # k block (scores^T partition block)
GW = 2                       # kb blocks per score-psum tile (2 banks)
TAIL_WARM = 54               # PE keep-warm matmuls bridging the last a2a wait

F32 = mybir.dt.float32
BF16 = mybir.dt.bfloat16
FP8 = mybir.dt.float8e4

FULL, TRI, GEN, SKIP = 0, 1, 2, 3


def make_plan(mask_np, s=S, qc=QC, kb=KB):
    """Classify scores^T blocks [kb x qc] from the (B, S, S) 0/1 mask.

    Returns (blocks, n_gen_tiles, gen_tiles_per_batch):
      blocks[iqc] = list of (ikb, mode, arg)
    """
    nqc, nkb = s // qc, s // kb
    m = np.asarray(mask_np) != 0          # [B, S(q), S(k)] True = attend
    tril = np.tril(np.ones((s, s), bool))
    causal = all(np.array_equal(m[b], tril) for b in range(m.shape[0]))
    blocks = []
    if causal:
        for iqc in range(nqc):
            row = []
            for ikb in range(nkb):
                if (ikb + 1) * kb <= iqc * qc:
                    row.append((ikb, FULL, 0))
                elif ikb * kb < (iqc + 1) * qc:
                    row.append((ikb, TRI, (ikb * kb - iqc * qc) // kb))
                # else fully masked -> skip
            blocks.append(row)
        return blocks, 0, None

    # general path: per-block classification, unioned across batches
    nb = m.shape[0]
    # every query row must attend to >= 1 key (else softmax semantics differ)
    assert m.any(axis=-1).all(), "fully-masked query rows unsupported"
    gen_tiles = [[] for _ in range(nb)]
    for iqc in range(nqc):
        row = []
        for ikb in range(nkb):
            sub = m[:, iqc * qc:(iqc + 1) * qc, ikb * kb:(ikb + 1) * kb]
            if sub.all():
                row.append((ikb, FULL, 0))
            elif not sub.any():
                continue
            else:
                idx = len(gen_tiles[0])
                for b in range(nb):
                    gen_tiles[b].append(sub[b].T.astype(np.int32))  # [kb, qc]
                row.append((ikb, GEN, idx))
        blocks.append(row)
    n_gen = len(gen_tiles[0])
    gt = [np.stack(g) if n_gen else np.zeros((1, kb, qc), np.int32)
          for g in gen_tiles]
    return blocks, n_gen, gt


def build_mha(blocks, n_gen, *, s=S, dm=DM, de=DE, dh=None, mm="fp8",
              collective=True, chunked_cc=True, cc_reps=1):
    """Trace the per-core MHA program.  dh = per-core emb slice (256)."""
    if dh is None:
        dh = DE // 4
    nqc, nkb, ndm, nde = s // QC, s // KB, dm // P, de // P
    ndh = dh // P            # feature chunks per core (2)
    hloc = dh // DK          # heads per core (4)
    nsub = QC // 64          # a2a sub-blocks per chunk (8)
    out_rows = nqc * P       # output rows per core (4 chunks x 2 x 64)

    qk8 = mm in ("fp8", "fp8qk")
    pv8 = (mm == "fp8")
    qkt = FP8 if qk8 else BF16
    pvt = FP8 if pv8 else BF16   # dtype of probs, V, and mask tiles
    st = BF16

    # can attention chunk c start right after projection chunk c?
    causal_dep = all(
        max([c] + [ikb * KB // QC for (ikb, _, _) in blocks[c]]) <= c
        for c in range(nqc))

    nc = bacc.Bacc("TRN2", target_bir_lowering=False, debug=False,
                   num_devices=N_CORES)

    # ---- kernel I/O (per core) ----
    qT = nc.dram_tensor("qT", [dm, s], qkt, kind="ExternalInput")
    kT = nc.dram_tensor("kT", [dm, s], qkt, kind="ExternalInput")
    vT = nc.dram_tensor("vT", [dm, s], BF16, kind="ExternalInput")
    w_inT = nc.dram_tensor("w_inT", [de, dm], BF16, kind="ExternalInput")
    w_inT8 = nc.dram_tensor("w_inT8", [de, dm], qkt, kind="ExternalInput")
    wq = nc.dram_tensor("wq", [de, dh], BF16, kind="ExternalInput")
    wk = nc.dram_tensor("wk", [de, dh], BF16, kind="ExternalInput")
    wq8 = nc.dram_tensor("wq8", [de, dh], qkt, kind="ExternalInput")
    wk8 = nc.dram_tensor("wk8", [de, dh], qkt, kind="ExternalInput")
    wv = nc.dram_tensor("wv", [de, dh], BF16, kind="ExternalInput")
    wo = nc.dram_tensor("wo", [de, dm], BF16, kind="ExternalInput")
    b_in = nc.dram_tensor("b_in", [de], BF16, kind="ExternalInput")
    bq = nc.dram_tensor("bq", [dh], F32, kind="ExternalInput")
    bk = nc.dram_tensor("bk", [dh], F32, kind="ExternalInput")
    bv = nc.dram_tensor("bv", [dh], F32, kind="ExternalInput")
    bo = nc.dram_tensor("bo", [dm], F32, kind="ExternalInput")
    m_tiles = nc.dram_tensor("m_tiles", [max(n_gen, 1), KB, QC], mybir.dt.int32,
                             kind="ExternalInput")
    y_out = nc.dram_tensor("y_out", [out_rows, dm], F32, kind="ExternalOutput")

    # a2a staging: chunked mode [chunk][8 dest blocks][256 feats][64 q];
    # single mode [8 dest blocks][chunk][256 feats][64 q] (one collective)
    if chunked_cc:
        a2a_in = nc.dram_tensor("a2a_in", [nqc, nsub, dh, 64], BF16)
        a2a_out = nc.dram_tensor("a2a_out", [nqc, nsub, dh, 64], BF16)
    else:
        a2a_in = nc.dram_tensor("a2a_in", [nsub, nqc, dh, 64], BF16)
        a2a_out = nc.dram_tensor("a2a_out", [nsub, nqc, dh, 64], BF16)

    with tile.TileContext(nc) as tc, ExitStack() as ex:
        persist = ex.enter_context(tc.tile_pool(name="persist", bufs=1))
        work = ex.enter_context(tc.tile_pool(name="work", bufs=3))
        ps_w = ex.enter_context(tc.tile_pool(name="ps_w", bufs=2, space="PSUM"))
        ps_s = ex.enter_context(tc.tile_pool(name="ps_s", bufs=2, space="PSUM"))
        ps_o = ex.enter_context(tc.tile_pool(name="ps_o", bufs=2, space="PSUM"))
        qbufs = 2 if causal_dep else nqc
        xpool = ex.enter_context(tc.tile_pool(name="xpool", bufs=3))
        qpool = ex.enter_context(tc.tile_pool(name="qpool", bufs=qbufs))
        ppool = ex.enter_context(tc.tile_pool(name="ppool", bufs=6))
        cpool = ex.enter_context(tc.tile_pool(name="cpool", bufs=2))
        ypool = ex.enter_context(tc.tile_pool(name="ypool", bufs=2))
        wpool = ex.enter_context(tc.tile_pool(name="wpool", bufs=1))

        # ---- constants ----
        # tri[k, q] = 1.0 where k <= q (keep), else 0
        tri = persist.tile([P, P], pvt, tag="tri", name="tri")
        tri_b = persist.tile([P, P], st, tag="tri_b", name="tri_b")
        nc.gpsimd.memset(tri_b[:], 0.0)
        nc.gpsimd.affine_select(out=tri_b[:], in_=tri_b[:],
                                compare_op=mybir.AluOpType.is_gt,
                                fill=1.0, base=0,
                                pattern=[[-1, P]], channel_multiplier=1)
        if pvt == st:
            tri = tri_b
        else:
            nc.vector.tensor_copy(tri[:], tri_b[:])
        ones1 = persist.tile([1, P], st, tag="ones1", name="ones1")
        nc.gpsimd.memset(ones1[:], 1.0)
        # preload the Exp table while DMAs stream in
        actwarm = persist.tile([1, 1], F32, tag="actwarm", name="actwarm")
        nc.scalar.activation(actwarm[:], ones1[0:1, 0:1],
                             mybir.ActivationFunctionType.Exp)
        scr = persist.tile([1, QC], st, tag="scr", name="scr")
        nc.vector.memset(scr[:], 1.0)
        HEAD_WARM = 11

        def pe_keepwarm(n):
            # dummy matmuls bridge a PE idle window so the clock does not
            # drop out of max p-state before the next real matmul burst
            pwu = ps_o.tile([1, QC], F32, tag="ps_av", name="pwu")
            for i in range(n):
                nc.tensor.matmul(pwu[:], scr[0:1, 0:1], scr[:],
                                 start=(i == 0), stop=(i == n - 1))

        gen_sb = None
        if n_gen:
            gen_sb = persist.tile([P, n_gen, QC], pvt, tag="gen", name="gen")
            gi = persist.tile([P, n_gen, QC], mybir.dt.int32, tag="gen_i",
                              name="gen_i")
            nc.sync.dma_start(gi[:], m_tiles[:].rearrange("n p q -> p n q"))
            for i in range(n_gen):
                if pvt == st:
                    nc.vector.tensor_copy(gen_sb[:, i, :], gi[:, i, :])
                else:
                    gb = work.tile([P, QC], st, tag="gen_b", name="gen_b")
                    nc.vector.tensor_copy(gb[:], gi[:, i, :])
                    nc.vector.tensor_copy(gen_sb[:, i, :], gb[:])

        DR0 = mybir.MatmulPerfMode.DoubleRow

        # ---- persistent activation storage (memsets run at t=0) ----
        kT_sb = [persist.tile([P, s], qkt, tag=f"kT{t}", name=f"kT{t}")
                 for t in range(ndh)]
        # V in natural layout, heads side by side, with a ones column:
        # v_all[kb_row, ikb, h, 0:DK] = v_h[key, :], v_all[.., DK] = 1
        # fp8 DoubleRow ldweights needs 4-byte-aligned slot strides: pad
        # each head's [d_k | ones] slot to VW columns (tail zeroed)
        VW = DK + 4 if pv8 else DK + 1
        v_all = persist.tile([P, nkb, hloc, VW], pvt, tag="v_all",
                             name="v_all")
        nc.gpsimd.memset(v_all[:, :, :, DK], 1.0)
        if VW > DK + 1:
            nc.gpsimd.memset(v_all[:, :, :, DK + 1:VW], 0.0)

        qf_tiles = {}

        def make_qf(iqc, memset=False):
            qf = [qpool.tile([P, 2, QC], qkt, tag=f"qf{t}", name=f"qf{t}_{iqc}")
                  for t in range(ndh)]
            qf_tiles[iqc] = qf
            if memset and qk8:
                # pool ring: zero slots persist across later buffer reuse
                for t in range(ndh):
                    nc.vector.memset(qf[t][:, 1, :], 0.0)
            return qf

        for c in range(qbufs):
            make_qf(c, memset=True)

        # ---- load weights (bf16 from host; batched DMAs) ----
        # spread DMAs across both HWDGE queues (SP + Activation)
        _dmaq = [0]

        def dmaq():
            _dmaq[0] ^= 1
            return nc.sync if _dmaq[0] else nc.scalar

        # wq first, then w_inT in quarters: the first combine matmuls only
        # need w_in chunk u=0 + wq, so PE can start ~2.7us in
        w_sb = {}
        w8_sb = {}
        if qk8:
            # fp8 copies drive the DoubleRow q/k combine (4x MACs/cycle)
            wb_q8 = wpool.tile([P, nde, dh], qkt, tag="wq8", name="wq8_b")
            nc.sync.dma_start(out=wb_q8[:],
                              in_=wq8[:].rearrange("(u p) d -> p u d", p=P))
            w8_sb["q"] = wb_q8
            w_in8_b = wpool.tile([P, nde, dm], qkt, tag="w_in8", name="w_in8_b")
            hd8 = nde // 2
            for i in range(2):
                dmaq().dma_start(
                    out=w_in8_b[:, i * hd8:(i + 1) * hd8, :],
                    in_=w_inT8[i * hd8 * P:(i + 1) * hd8 * P, :]
                        .rearrange("(u p) m -> p u m", p=P))
            wb_k8 = wpool.tile([P, nde, dh], qkt, tag="wk8", name="wk8_b")
            dmaq().dma_start(out=wb_k8[:],
                             in_=wk8[:].rearrange("(u p) d -> p u d", p=P))
            w8_sb["k"] = wb_k8
        w_inT_b = wpool.tile([P, nde, dm], st, tag="w_inT", name="w_inT_b")
        w_inT_sb = [w_inT_b[:, u, :] for u in range(nde)]
        b_inT = wpool.tile([P, nde], st, tag="b_inT", name="b_inT")
        bo_bcast = persist.tile([P, dm], F32, tag="bo_b", name="bo_bcast")
        for name in ("q", "k", "v"):
            wb = wpool.tile([P, nde, dh], st, tag=f"w{name}", name=f"w{name}_b")
            w_sb[name] = [wb[:, u, :] for u in range(nde)]
            w_sb[name + "_t"] = wb

        def load_w_small(name):
            # bf16 head-projection weights (bias combine) + b_in
            def f():
                dmaq().dma_start(
                    out=w_sb[name + "_t"][:],
                    in_={"q": wq, "k": wk, "v": wv}[name]
                        .rearrange("(u p) d -> p u d", p=P))
                if name == "q":
                    nc.scalar.dma_start(
                        out=b_inT[:], in_=b_in[:].rearrange("(t p) -> p t", p=P))
            return f

        def load_w_inT_bf16():
            hd4 = nde // 4
            for i in range(4):
                dmaq().dma_start(
                    out=w_inT_b[:, i * hd4:(i + 1) * hd4, :],
                    in_=w_inT[i * hd4 * P:(i + 1) * hd4 * P, :]
                        .rearrange("(u p) m -> p u m", p=P))
            nc.scalar.dma_start(out=bo_bcast[:],
                                in_=bo[:].unsqueeze(0).broadcast_to([P, dm]))
        wo_sb = persist.tile([P, nde, dm], st, tag="wo", name="wo_b")

        def load_wo():
            # deferred: wo is not needed until the first output projection
            hdo = nde // 2
            nc.sync.dma_start(out=wo_sb[:, 0:hdo, :],
                              in_=wo[0:hdo * P, :].rearrange("(u p) m -> p u m", p=P))
            nc.scalar.dma_start(out=wo_sb[:, hdo:nde, :],
                                in_=wo[hdo * P:, :].rearrange("(u p) m -> p u m", p=P))

        # ---- combine weights: Wc_x = W_in @ Wx (+ bias fold) ----
        # q/k: fp8 DoubleRow over paired de-chunks -> paired-layout wc8
        # (wc8[name][t//2][:, t%2, :] = Wc rows of dm-chunk t); v: bf16
        wc = {}
        wc8 = {}
        bc = {}

        def combine_qk8(name):
            wc8[name] = [persist.tile([P, 2, dh], qkt, tag=f"wc8{name}{t}",
                                      name=f"wc8{name}{t}")
                         for t in range(ndm // 2)]
            for tp in range(ndm // 2):
                ps = ps_w.tile([P, 2 * dh], F32, tag="ps_w", name="ps_w")
                for half in range(2):
                    t = 2 * tp + half
                    for i in range(nde // 2):
                        nc.tensor.matmul(
                            ps[:, half * dh:(half + 1) * dh],
                            w_in8_b[:, 2 * i:2 * i + 2, t * P:(t + 1) * P],
                            w8_sb[name][:, 2 * i:2 * i + 2, :],
                            perf_mode=DR0,
                            start=(i == 0), stop=(i == nde // 2 - 1))
                nc.vector.tensor_copy(
                    wc8[name][tp][:],
                    ps[:].rearrange("p (two d) -> p two d", two=2))

        def combine_bf16_closures(name):
            wc[name] = [persist.tile([P, dh], st, tag=f"wc{name}{t}",
                                     name=f"wc{name}{t}") for t in range(ndm)]

            def piece(ts_):
                def f():
                    for t in ts_:
                        ps = ps_w.tile([P, dh], F32, tag="ps_w", name="ps_w")
                        for u in range(nde):
                            nc.tensor.matmul(
                                ps[:], w_inT_sb[u][:, t * P:(t + 1) * P],
                                w_sb[name][u][:],
                                start=(u == 0), stop=(u == nde - 1))
                        nc.vector.tensor_copy(wc[name][t][:], ps[:])
                return f
            return [piece(ts_) for ts_ in
                    ([0, 1], [2, 3], [4, 5], [6, 7])]

        def bias_qk(name):
            bvec = {"q": bq, "k": bk}[name]
            bxT = wpool.tile([P, ndh], F32, tag=f"bxT{name}", name=f"bxT{name}")
            nc.sync.dma_start(out=bxT[:], in_=bvec[:].rearrange("(t p) -> p t", p=P))
            bc[name] = persist.tile([P, ndh], F32, tag=f"bc{name}", name=f"bc{name}")
            for t in range(ndh):
                ps = ps_w.tile([P, 1], F32, tag="ps_w", name="ps_w")
                for u in range(nde):
                    nc.tensor.matmul(
                        ps[:], w_sb[name][u][:, t * P:(t + 1) * P],
                        b_inT[:, u:u + 1],
                        start=(u == 0), stop=(u == nde - 1))
                nc.vector.tensor_add(bc[name][:, t:t + 1], ps[:], bxT[:, t:t + 1])

        bcv_row = persist.tile([1, dh], st, tag="bcv", name="bcv_row")

        def bias_v():
            bv_row = wpool.tile([1, dh], F32, tag="bv_row", name="bv_row")
            nc.sync.dma_start(out=bv_row[:], in_=bv[:].unsqueeze(0))
            ps = ps_w.tile([1, dh], F32, tag="ps_w", name="ps_w")
            for u in range(nde):
                nc.tensor.matmul(ps[:], b_inT[:, u:u + 1], w_sb["v"][u][:],
                                 start=(u == 0), stop=(u == nde - 1))
            nc.vector.tensor_add(bcv_row[:], ps[:], bv_row[:])

        def proj_closures(iqc):
            """Per-chunk projection emission, split into PE-sized closures."""
            clos = []
            qf = qf_tiles.get(iqc) or make_qf(iqc, memset=iqc < qbufs)
            xbs = {}

            def load(name, xdram):
                def f():
                    dt_ = qkt if (qk8 and name in ("q", "k")) else st
                    tag = "xb8" if (qk8 and name in ("q", "k")) else "xb"
                    xb = xpool.tile([P, ndm, QC], dt_, tag=tag,
                                    name=f"xb_{name}{iqc}")
                    xbs[name] = xb
                    dmaq().dma_start(
                        out=xb[:],
                        in_=xdram[:, iqc * QC:(iqc + 1) * QC]
                            .rearrange("(u p) s -> p u s", p=P))
                return f

            def qk_part(name, t):
                def f():
                    xb = xbs[name]
                    ps = ps_w.tile([P, QC], F32, tag="ps_w", name="ps_w")
                    if qk8:
                        for i in range(ndm // 2):
                            nc.tensor.matmul(
                                ps[:],
                                wc8[name][i][:, :, t * P:(t + 1) * P],
                                xb[:, 2 * i:2 * i + 2, :],
                                perf_mode=DR0,
                                start=(i == 0), stop=(i == ndm // 2 - 1))
                    else:
                        for u in range(ndm):
                            nc.tensor.matmul(
                                ps[:], wc[name][u][:, t * P:(t + 1) * P],
                                xb[:, u, :], start=(u == 0),
                                stop=(u == ndm - 1))
                    if name == "k":
                        nc.vector.tensor_scalar_add(
                            kT_sb[t][:, iqc * QC:(iqc + 1) * QC], ps[:],
                            bc["k"][:, t:t + 1])
                    else:
                        nc.vector.tensor_scalar_add(
                            qf[t][:, 0, :], ps[:], bc["q"][:, t:t + 1])
                return f

            def v_part(j):
                def f():
                    xb = xbs["v"]
                    ikb = iqc * (QC // P) + j
                    ps = ps_w.tile([P, dh], F32, tag="ps_w", name="ps_w")
                    for u in range(ndm):
                        nc.tensor.matmul(
                            ps[:], xb[:, u, j * P:(j + 1) * P], wc["v"][u][:],
                            start=(u == 0), stop=False)
                    nc.tensor.matmul(ps[:], ones1[:], bcv_row[:],
                                     start=False, stop=True)
                    nc.vector.tensor_copy(
                        v_all[:, ikb, :, 0:DK],
                        ps[:].rearrange("p (h d) -> p h d", h=hloc))
                return f

            clos.append(load("q", qT))
            for t in range(ndh):
                clos.append(qk_part("q", t))
            clos.append(load("k", kT))
            for t in range(ndh):
                clos.append(qk_part("k", t))
            clos.append(load("v", vT))
            for j in range(QC // P):
                clos.append(v_part(j))
            return clos

        # ---- attention ----
        inv_sqrt = 1.0 / math.sqrt(DK)
        DR = mybir.MatmulPerfMode.DoubleRow
        cT_tiles = {}

        def attention_chunk(iqc, fillers=(), mid=None):
            """QK+exp of step i overlaps PV of step i-1; `fillers` (next
            chunk's projection closures) are spread over the early steps;
            `mid` (the previous chunk's yin load) fires ~70% through."""
            qf = qf_tiles[iqc]
            cT = cpool.tile([P, ndh, QC], st, tag="cT", name=f"cT{iqc}")
            cT_tiles[iqc] = cT
            blist = blocks[iqc]
            steps = []
            for h in range(hloc):
                grps = [blist[g0:g0 + GW] for g0 in range(0, len(blist), GW)]
                for g in range(len(grps)):
                    steps.append((h, grps[g], g == 0, g == len(grps) - 1))
            po = {}
            pending = []
            fillers = list(fillers)
            n_steps = len(steps)
            fill_at = {}
            if fillers:
                # spread fillers uniformly across the steps
                for fi in range(len(fillers)):
                    at = (fi * n_steps) // len(fillers)
                    fill_at.setdefault(min(at, n_steps - 1), []).append(
                        fillers[fi])
            mid_at = (7 * n_steps) // 10

            def emit_qk_exp(h, grp):
                t, off = h // 2, (h % 2) * DK
                pss = ps_s.tile([P, GW * QC], F32, tag="ps_scores",
                                name="ps_scores")
                for j, (ikb, mode, arg) in enumerate(grp):
                    kv = kT_sb[t][off:off + DK, ikb * KB:(ikb + 1) * KB]
                    if qk8:
                        nc.tensor.matmul(
                            pss[:, j * QC:(j + 1) * QC],
                            kv.unsqueeze(1).broadcast_to([DK, 2, KB]),
                            qf[t][off:off + DK, :, :],
                            perf_mode=DR, start=True, stop=True)
                    else:
                        nc.tensor.matmul(pss[:, j * QC:(j + 1) * QC],
                                         kv, qf[t][off:off + DK, 0, :])
                pt = ppool.tile([P, GW * QC], pvt, tag="p", name="p")
                nw = len(grp) * QC
                nc.scalar.activation(pt[:, 0:nw], pss[:, 0:nw],
                                     mybir.ActivationFunctionType.Exp,
                                     scale=inv_sqrt)
                for j, (ikb, mode, arg) in enumerate(grp):
                    pj = pt[:, j * QC:(j + 1) * QC]
                    if mode == TRI:
                        r = arg
                        if r > 0:
                            nc.gpsimd.memset(pj[:, 0:r * P], 0.0)
                        nc.vector.tensor_mul(
                            pj[:, r * P:(r + 1) * P],
                            pj[:, r * P:(r + 1) * P], tri[:])
                    elif mode == GEN:
                        nc.vector.tensor_mul(pj[:], pj[:], gen_sb[:, arg, :])
                return pt

            def emit_pv(h, grp, pt, first, last):
                if first:
                    po[h] = ps_o.tile([VW, QC], F32, tag="ps_av",
                                      name="ps_av")
                ikbs = [ikb for (ikb, _, _) in grp]
                if pv8 and len(grp) == 2 and ikbs[1] == ikbs[0] + 1:
                    nc.tensor.matmul(
                        po[h][:], v_all[:, ikbs[0]:ikbs[0] + 2, h, :],
                        pt[:].rearrange("p (two q) -> p two q", two=2),
                        perf_mode=DR,
                        start=first, stop=last)
                else:
                    for j, (ikb, mode, arg) in enumerate(grp):
                        nc.tensor.matmul(
                            po[h][:, 0:QC], v_all[:, ikb, h, :],
                            pt[:, j * QC:(j + 1) * QC],
                            start=(first and j == 0),
                            stop=(last and j == len(grp) - 1))
                if last:
                    rec1 = work.tile([1, QC], F32, tag="rec1", name="rec1")
                    nc.vector.reciprocal(rec1[:], po[h][DK:DK + 1, :])
                    recb = work.tile([DK, QC], F32, tag="recb", name="recb")
                    nc.gpsimd.partition_broadcast(recb[:], rec1[:])
                    nc.vector.tensor_mul(
                        cT[(h % 2) * DK:(h % 2) * DK + DK, h // 2, :],
                        po[h][0:DK, :], recb[:])
                    del po[h]

            for i, (h, grp, first, last) in enumerate(steps):
                pt = emit_qk_exp(h, grp)
                pending.append((h, grp, pt, first, last))
                if len(pending) > 3:
                    emit_pv(*pending.pop(0))
                for f in fill_at.get(i, ()):
                    f()
                if mid is not None and i == mid_at:
                    mid()
                    mid = None
            for p_ in pending:
                emit_pv(*p_)
            if mid is not None:
                mid()

        # ---- a2a + output projection ----
        def a2a_chunk(c):
            # cT [128, ndh, QC] -> a2a_in[c or :, c] [8, 256, 64]
            cT = cT_tiles[c]
            dst = a2a_in[c] if chunked_cc else a2a_in[:, c]
            for fh in range(ndh):
                (nc.sync if fh % 2 == 0 else nc.scalar).dma_start(
                    out=dst[:, fh * P:(fh + 1) * P, :]
                        .rearrange("r p j -> p r j"),
                    in_=cT[:, fh, :].rearrange("p (r j) -> p r j", r=nsub))
            if not chunked_cc:
                if c == nqc - 1:
                    if collective:
                        for _ in range(cc_reps):
                            nc.gpsimd.collective_compute(
                                "AllToAll", mybir.AluOpType.bypass,
                                replica_groups=[list(range(N_CORES))],
                                ins=[a2a_in[:].opt()], outs=[a2a_out[:].opt()])
                    else:
                        nc.sync.dma_start(out=a2a_out[:], in_=a2a_in[:])
                return
            if collective:
                for _ in range(cc_reps):
                    nc.gpsimd.collective_compute(
                        "AllToAll", mybir.AluOpType.bypass,
                        replica_groups=[list(range(N_CORES))],
                        ins=[a2a_in[c].opt()], outs=[a2a_out[c].opt()])
            else:
                nc.sync.dma_start(out=a2a_out[c], in_=a2a_in[c])

        yin_tiles = {}

        def yin_load(c):
            yin = ypool.tile([P, nde, P], st, tag="yin", name=f"yin{c}")
            yin_tiles[c] = yin
            half = nsub // 2
            if chunked_cc:
                src = a2a_out[c]
                nc.sync.dma_start(
                    out=yin[:, :, 0:64],
                    in_=src[0:half].rearrange("s (fh p) j -> p (s fh) j", p=P))
                nc.scalar.dma_start(
                    out=yin[:, :, 64:128],
                    in_=src[half:nsub].rearrange("s (fh p) j -> p (s fh) j",
                                                 p=P))
            else:
                yv = yin[:].rearrange("p (s fh) j -> p s fh j", fh=ndh)
                for b0, sl in ((0, slice(0, 64)), (half, slice(64, 128))):
                    for fh in range(ndh):
                        nc.sync.dma_start(
                            out=yv[:, :, fh, sl],
                            in_=a2a_out[b0:b0 + half, c, fh * P:(fh + 1) * P, :]
                                .rearrange("s p j -> p s j"))

        def yproj_mm(c):
            yin = yin_tiles[c]
            ys = ypool.tile([P, dm], F32, tag="ys", name=f"ys{c}")
            for mb in range(dm // QC):
                ps = ps_w.tile([P, QC], F32, tag="ps_w", name="ps_w")
                for u in range(nde):
                    nc.tensor.matmul(
                        ps[:], yin[:, u, :],
                        wo_sb[:, u, mb * QC:(mb + 1) * QC],
                        start=(u == 0), stop=(u == nde - 1))
                nc.vector.tensor_add(ys[:, mb * QC:(mb + 1) * QC], ps[:],
                                     bo_bcast[:, mb * QC:(mb + 1) * QC])
            nc.sync.dma_start(out=y_out[c * P:(c + 1) * P, :], in_=ys[:])

        # ---- schedule ----
        # head: score-path (fp8) combine + q/k projections first so the
        # first attention exp fires ~7us in; the bf16 v-combine and chunk-0
        # V projection become PE filler inside attention chunk 0.
        # yproj(c) is deferred into later chunks' attention as PE filler:
        # the final chunks have the most exp work and no projections left.
        if causal_dep:
            pc0 = proj_closures(0)
            pc0[0]()                       # load q chunk 0
            pc0[3]()                       # load k chunk 0
            if qk8:
                load_w_small("q")()
                load_w_small("k")()
                combine_qk8("q")
                bias_qk("q")
                pc0[1](); pc0[2]()         # project q chunk 0
                combine_qk8("k")
                bias_qk("k")
                pc0[4](); pc0[5]()         # project k chunk 0
            else:
                load_w_small("q")()
                load_w_small("k")()
                load_w_inT_bf16()
                for f in combine_bf16_closures("q"):
                    f()
                bias_qk("q")
                pc0[1](); pc0[2]()
                for f in combine_bf16_closures("k"):
                    f()
                bias_qk("k")
                pc0[4](); pc0[5]()
            pc0[6]()                       # load v chunk 0
            load_w_small("v")()
            if qk8:
                load_w_inT_bf16()
            head_fill = combine_bf16_closures("v") + [bias_v]
            load_wo()
            for c in range(nqc):
                fillers = list(head_fill)
                head_fill = []
                if c == 0:
                    fillers += pc0[7:]     # chunk-0 V projection
                if c + 1 < nqc:
                    fillers += list(proj_closures(c + 1))
                if chunked_cc and c == nqc - 1:
                    for cc in range(nqc - 2):
                        fillers.append((lambda c2: lambda: yproj_mm(c2))(cc))
                mid = ((lambda cc: lambda: yin_load(cc))(c - 1)
                       if c > 0 and chunked_cc else None)
                attention_chunk(c, fillers=fillers, mid=mid)
                a2a_chunk(c)
            if chunked_cc:
                yproj_mm(nqc - 2)
                pe_keepwarm(TAIL_WARM)
            else:
                for c in range(nqc - 1):
                    yin_load(c)
                    yproj_mm(c)
        else:
            # general masks: all projections first, then attention
            load_w_small("q")()
            load_w_small("k")()
            load_w_small("v")()
            load_w_inT_bf16()
            if qk8:
                combine_qk8("q")
                combine_qk8("k")
            else:
                for f in combine_bf16_closures("q") + combine_bf16_closures("k"):
                    f()
            bias_qk("q")
            bias_qk("k")
            for f in combine_bf16_closures("v"):
                f()
            bias_v()
            for c in range(nqc):
                for f in proj_closures(c):
                    f()
            load_wo()
            for c in range(nqc):
                mid = (lambda cc: lambda: yin_load(cc))(c - 1) if c > 0 else None
                attention_chunk(c, mid=mid)
                a2a_chunk(c)
                if c > 0:
                    yproj_mm(c - 1)
        yin_load(nqc - 1)
        yproj_mm(nqc - 1)

    nc.compile()
    return nc


# ------------------------------------------------------------------
_CACHE = {}


def _get_compiled(plan_key, blocks, n_gen, mm):
    if plan_key not in _CACHE:
        nc = build_mha(blocks, n_gen, mm=mm)
        nc.m = get_hw_module(nc.m)
        _CACHE[plan_key] = nc
    return _CACHE[plan_key]


def make_in_maps(q, k, v, mask, W_in, b_in, Wq, bq, Wk, bk, Wv, bv, Wo, bo,
                 blocks=None, n_gen=None, gen_tiles=None):
    if blocks is None:
        blocks, n_gen, gen_tiles = make_plan(mask)
    bf16 = mybir.dt.np(BF16)
    fp8 = mybir.dt.np(FP8)
    dh = DE // 4
    tb = lambda a: np.ascontiguousarray(np.asarray(a).T).astype(bf16)
    cb = lambda a: np.ascontiguousarray(np.asarray(a)).astype(bf16)
    t8 = lambda a: np.ascontiguousarray(np.asarray(a).T).astype(fp8)
    c8 = lambda a: np.ascontiguousarray(np.asarray(a)).astype(fp8)
    in_maps = []
    for c in range(N_CORES):
        b, g = c // 4, c % 4
        sl = slice(g * dh, (g + 1) * dh)
        mt = (gen_tiles[b] if n_gen else
              np.zeros((1, KB, QC), np.int32))
        qk8 = MM_MODE in ("fp8", "fp8qk")
        tq = t8 if qk8 else tb
        in_maps.append({
            "qT": tq(q[b]), "kT": tq(k[b]), "vT": tb(v[b]),
            "w_inT": tb(W_in), "w_inT8": t8(W_in),
            "wq": cb(Wq[:, sl]),
            "wk": cb(Wk[:, sl]),
            "wq8": c8(Wq[:, sl]),
            "wk8": c8(Wk[:, sl]),
            "wv": cb(Wv[:, sl]),
            "wo": cb(Wo),
            "b_in": np.asarray(b_in).astype(bf16),
            "bq": np.ascontiguousarray(np.asarray(bq)[sl]),
            "bk": np.ascontiguousarray(np.asarray(bk)[sl]),
            "bv": np.ascontiguousarray(np.asarray(bv)[sl]),
            "bo": np.asarray(bo),
            "m_tiles": mt,
        })
    return in_maps, blocks, n_gen


def assemble(results):
    out = np.empty((B, S, DM), np.float32)
    for core in range(N_CORES):
        y = results[core]["y_out"]            # [nqc*128, DM]
        for c in range(S // QC):
            for b in range(B):
                out[b, c * QC + core * 64:c * QC + (core + 1) * 64, :] = \
                    y[c * P + b * 64:c * P + (b + 1) * 64, :]
    return out


MM_MODE = "fp8qk"


def kernel(**inputs):
    mask = inputs["mask"]
    blocks, n_gen, gen_tiles = make_plan(np.asarray(mask))
    plan_key = (str(blocks), n_gen, MM_MODE)
    nc = _get_compiled(plan_key, blocks, n_gen, MM_MODE)
    in_maps, _, _ = make_in_maps(
        inputs["q"], inputs["k"], inputs["v"], mask,
        inputs["W_in"], inputs["b_in"], inputs["Wq"], inputs["bq"],
        inputs["Wk"], inputs["bk"], inputs["Wv"], inputs["bv"],
        inputs["Wo"], inputs["bo"],
        blocks=blocks, n_gen=n_gen, gen_tiles=gen_tiles)
    res = bass_utils.run_bass_kernel_spmd(nc, in_maps,
                                          core_ids=list(range(N_CORES)))
    return assemble(res.results)


# revision 6
# speedup vs baseline: 1.0348x; 1.0066x over previous
"""Trainium2 Bass kernel for nn_MultiHeadAttention_72069551227273 (v2).

Reference computation (B=2, S=2048, D_MODEL=D_EMB=1024, H=16, d_k=64):
    q_p = q @ W_in + b_in                    (shared input projection)
    qh  = heads(q_p @ Wq + bq)               (per-head projections)
    s   = qh @ kh^T / sqrt(d_k), causal-masked softmax
    out = (attn @ vh, concat heads) @ Wo + bo

Sharding: 8 cores = 2 (batch) x 4 (head groups of 4 heads / 256 emb cols).
Per core the input and head projections are fused on device:
    Q = q @ (W_in @ Wq_slice) + (b_in @ Wq_slice + bq_slice)
The whole score path (W_in@Wq/Wk weight combine, Q/K projections, QK^T)
runs in fp8e4m3 DoubleRow perf mode with contraction chunks paired into
the two DoubleRow slots: 4x MACs/cycle over bf16 for combine+projections
and 2x for scores (score lhsT slots = (K, K) via a stride-0 broadcast,
rhs slots = (Q, 0)).  fp8 score noise washes out in the softmax; the
value path (V, attn@V, Wo) stays bf16 — fp8 there measured 2.4e-2
relative error, over the 2e-2 gate.  V is projected straight into the
natural [seq, head, d_k] layout (no PE transposes) with its bias folded
in via a rank-1 ones matmul.  Softmax is exp(s/8) without max-subtraction;
the denominator comes free from a ones column appended to V.  Fully-masked
score blocks are skipped at trace time, diagonal blocks get an on-chip
triangular mask.  Projection work for chunk c+1 is interleaved into the
attention steps of chunk c (attention is Act/exp-heavy, projections are
PE-heavy), and attention itself is software-pipelined so PV of step i-1
overlaps the exp of step i.

Output stage is sequence-parallel instead of tensor-parallel: after
attention chunk c, an 8-way AllToAll (bf16, 256 KB) redistributes the
attention outputs so every core holds all 1024 features for 64 q rows of
each batch, then applies the full Wo locally — there is no reduction
collective at all.  The per-chunk collectives overlap with the next
chunk's attention; only the last chunk's exchange is exposed.
"""

import sys

sys.path.append("/opt/trn_rl_repo")

import math
from contextlib import ExitStack

import numpy as np

import concourse.bass as bass
import concourse.bacc as bacc
import concourse.mybir as mybir
import concourse.tile as tile
from concourse import bass_utils
from concourse.bass_interp import get_hw_module

# problem dims
B, S, DM, DE, H, DK = 2, 2048, 1024, 1024, 16, 64
N_CORES = 8
P = 128                      # partitions
QC = 512                     # q chunk (psum bank width in fp32)
KB = 128                     # k block (scores^T partition block)
GW = 2                       # kb blocks per score-psum tile (2 banks)
TAIL_WARM = 58               # PE keep-warm matmuls bridging the last a2a wait

F32 = mybir.dt.float32
BF16 = mybir.dt.bfloat16
FP8 = mybir.dt.float8e4

FULL, TRI, GEN, SKIP = 0, 1, 2, 3


def make_plan(mask_np, s=S, qc=QC, kb=KB):
    """Classify scores^T blocks [kb x qc] from the (B, S, S) 0/1 mask.

    Returns (blocks, n_gen_tiles, gen_tiles_per_batch):
      blocks[iqc] = list of (ikb, mode, arg)
    """
    nqc, nkb = s // qc, s // kb
    m = np.asarray(mask_np) != 0          # [B, S(q), S(k)] True = attend
    tril = np.tril(np.ones((s, s), bool))
    causal = all(np.array_equal(m[b], tril) for b in range(m.shape[0]))
    blocks = []
    if causal:
        for iqc in range(nqc):
            row = []
            for ikb in range(nkb):
                if (ikb + 1) * kb <= iqc * qc:
                    row.append((ikb, FULL, 0))
                elif ikb * kb < (iqc + 1) * qc:
                    row.append((ikb, TRI, (ikb * kb - iqc * qc) // kb))
                # else fully masked -> skip
            blocks.append(row)
        return blocks, 0, None

    # general path: per-block classification, unioned across batches
    nb = m.shape[0]
    # every query row must attend to >= 1 key (else softmax semantics differ)
    assert m.any(axis=-1).all(), "fully-masked query rows unsupported"
    gen_tiles = [[] for _ in range(nb)]
    for iqc in range(nqc):
        row = []
        for ikb in range(nkb):
            sub = m[:, iqc * qc:(iqc + 1) * qc, ikb * kb:(ikb + 1) * kb]
            if sub.all():
                row.append((ikb, FULL, 0))
            elif not sub.any():
                continue
            else:
                idx = len(gen_tiles[0])
                for b in range(nb):
                    gen_tiles[b].append(sub[b].T.astype(np.int32))  # [kb, qc]
                row.append((ikb, GEN, idx))
        blocks.append(row)
    n_gen = len(gen_tiles[0])
    gt = [np.stack(g) if n_gen else np.zeros((1, kb, qc), np.int32)
          for g in gen_tiles]
    return blocks, n_gen, gt


def build_mha(blocks, n_gen, *, s=S, dm=DM, de=DE, dh=None, mm="fp8",
              collective=True, chunked_cc=True, cc_reps=1):
    """Trace the per-core MHA program.  dh = per-core emb slice (256)."""
    if dh is None:
        dh = DE // 4
    nqc, nkb, ndm, nde = s // QC, s // KB, dm // P, de // P
    ndh = dh // P            # feature chunks per core (2)
    hloc = dh // DK          # heads per core (4)
    nsub = QC // 64          # a2a sub-blocks per chunk (8)
    out_rows = nqc * P       # output rows per core (4 chunks x 2 x 64)

    qk8 = mm in ("fp8", "fp8qk")
    pv8 = (mm == "fp8")
    qkt = FP8 if qk8 else BF16
    pvt = FP8 if pv8 else BF16   # dtype of probs, V, and mask tiles
    st = BF16

    # can attention chunk c start right after projection chunk c?
    causal_dep = all(
        max([c] + [ikb * KB // QC for (ikb, _, _) in blocks[c]]) <= c
        for c in range(nqc))

    nc = bacc.Bacc("TRN2", target_bir_lowering=False, debug=False,
                   num_devices=N_CORES)

    # ---- kernel I/O (per core) ----
    qT = nc.dram_tensor("qT", [dm, s], qkt, kind="ExternalInput")
    kT = nc.dram_tensor("kT", [dm, s], qkt, kind="ExternalInput")
    vT = nc.dram_tensor("vT", [dm, s], BF16, kind="ExternalInput")
    w_inT = nc.dram_tensor("w_inT", [de, dm], BF16, kind="ExternalInput")
    w_inT8 = nc.dram_tensor("w_inT8", [de, dm], qkt, kind="ExternalInput")
    wq = nc.dram_tensor("wq", [de, dh], BF16, kind="ExternalInput")
    wk = nc.dram_tensor("wk", [de, dh], BF16, kind="ExternalInput")
    wq8 = nc.dram_tensor("wq8", [de, dh], qkt, kind="ExternalInput")
    wk8 = nc.dram_tensor("wk8", [de, dh], qkt, kind="ExternalInput")
    wv = nc.dram_tensor("wv", [de, dh], BF16, kind="ExternalInput")
    wo = nc.dram_tensor("wo", [de, dm], BF16, kind="ExternalInput")
    b_in = nc.dram_tensor("b_in", [de], BF16, kind="ExternalInput")
    bq = nc.dram_tensor("bq", [dh], F32, kind="ExternalInput")
    bk = nc.dram_tensor("bk", [dh], F32, kind="ExternalInput")
    bv = nc.dram_tensor("bv", [dh], F32, kind="ExternalInput")
    bo = nc.dram_tensor("bo", [dm], F32, kind="ExternalInput")
    m_tiles = nc.dram_tensor("m_tiles", [max(n_gen, 1), KB, QC], mybir.dt.int32,
                             kind="ExternalInput")
    y_out = nc.dram_tensor("y_out", [out_rows, dm], F32, kind="ExternalOutput")

    # a2a staging: chunked mode [chunk][8 dest blocks][256 feats][64 q];
    # single mode [8 dest blocks][chunk][256 feats][64 q] (one collective)
    if chunked_cc:
        a2a_in = nc.dram_tensor("a2a_in", [nqc, nsub, dh, 64], BF16)
        a2a_out = nc.dram_tensor("a2a_out", [nqc, nsub, dh, 64], BF16)
    else:
        a2a_in = nc.dram_tensor("a2a_in", [nsub, nqc, dh, 64], BF16)
        a2a_out = nc.dram_tensor("a2a_out", [nsub, nqc, dh, 64], BF16)

    with tile.TileContext(nc) as tc, ExitStack() as ex:
        persist = ex.enter_context(tc.tile_pool(name="persist", bufs=1))
        work = ex.enter_context(tc.tile_pool(name="work", bufs=4))
        ps_w = ex.enter_context(tc.tile_pool(name="ps_w", bufs=2, space="PSUM"))
        ps_s = ex.enter_context(tc.tile_pool(name="ps_s", bufs=2, space="PSUM"))
        ps_o = ex.enter_context(tc.tile_pool(name="ps_o", bufs=2, space="PSUM"))
        qbufs = 2 if causal_dep else nqc
        xpool = ex.enter_context(tc.tile_pool(name="xpool", bufs=4))
        qpool = ex.enter_context(tc.tile_pool(name="qpool", bufs=qbufs))
        ppool = ex.enter_context(tc.tile_pool(name="ppool", bufs=8))
        cpool = ex.enter_context(tc.tile_pool(name="cpool", bufs=2))
        ypool = ex.enter_context(tc.tile_pool(name="ypool", bufs=2))
        wpool = ex.enter_context(tc.tile_pool(name="wpool", bufs=1))

        # ---- constants ----
        # tri[k, q] = 1.0 where k <= q (keep), else 0
        tri = persist.tile([P, P], pvt, tag="tri", name="tri")
        tri_b = persist.tile([P, P], st, tag="tri_b", name="tri_b")
        nc.gpsimd.memset(tri_b[:], 0.0)
        nc.gpsimd.affine_select(out=tri_b[:], in_=tri_b[:],
                                compare_op=mybir.AluOpType.is_gt,
                                fill=1.0, base=0,
                                pattern=[[-1, P]], channel_multiplier=1)
        if pvt == st:
            tri = tri_b
        else:
            nc.vector.tensor_copy(tri[:], tri_b[:])
        ones1 = persist.tile([1, P], st, tag="ones1", name="ones1")
        nc.gpsimd.memset(ones1[:], 1.0)
        # preload the Exp table while DMAs stream in
        actwarm = persist.tile([1, 1], F32, tag="actwarm", name="actwarm")
        nc.scalar.activation(actwarm[:], ones1[0:1, 0:1],
                             mybir.ActivationFunctionType.Exp)
        scr = persist.tile([1, QC], st, tag="scr", name="scr")
        nc.vector.memset(scr[:], 1.0)
        HEAD_WARM = 11

        def pe_keepwarm(n):
            # dummy matmuls bridge a PE idle window so the clock does not
            # drop out of max p-state before the next real matmul burst
            pwu = ps_o.tile([1, QC], F32, tag="ps_av", name="pwu")
            for i in range(n):
                nc.tensor.matmul(pwu[:], scr[0:1, 0:1], scr[:],
                                 start=(i == 0), stop=(i == n - 1))

        gen_sb = None
        if n_gen:
            gen_sb = persist.tile([P, n_gen, QC], pvt, tag="gen", name="gen")
            gi = persist.tile([P, n_gen, QC], mybir.dt.int32, tag="gen_i",
                              name="gen_i")
            nc.sync.dma_start(gi[:], m_tiles[:].rearrange("n p q -> p n q"))
            for i in range(n_gen):
                if pvt == st:
                    nc.vector.tensor_copy(gen_sb[:, i, :], gi[:, i, :])
                else:
                    gb = work.tile([P, QC], st, tag="gen_b", name="gen_b")
                    nc.vector.tensor_copy(gb[:], gi[:, i, :])
                    nc.vector.tensor_copy(gen_sb[:, i, :], gb[:])

        DR0 = mybir.MatmulPerfMode.DoubleRow

        # ---- persistent activation storage (memsets run at t=0) ----
        kT_sb = [persist.tile([P, s], qkt, tag=f"kT{t}", name=f"kT{t}")
                 for t in range(ndh)]
        # V in natural layout, heads side by side, with a ones column:
        # v_all[kb_row, ikb, h, 0:DK] = v_h[key, :], v_all[.., DK] = 1
        # fp8 DoubleRow ldweights needs 4-byte-aligned slot strides: pad
        # each head's [d_k | ones] slot to VW columns (tail zeroed)
        VW = DK + 4 if pv8 else DK + 1
        v_all = persist.tile([P, nkb, hloc, VW], pvt, tag="v_all",
                             name="v_all")
        nc.gpsimd.memset(v_all[:, :, :, DK], 1.0)
        if VW > DK + 1:
            nc.gpsimd.memset(v_all[:, :, :, DK + 1:VW], 0.0)

        qf_tiles = {}

        def make_qf(iqc, memset=False):
            qf = [qpool.tile([P, 2, QC], qkt, tag=f"qf{t}", name=f"qf{t}_{iqc}")
                  for t in range(ndh)]
            qf_tiles[iqc] = qf
            if memset and qk8:
                # pool ring: zero slots persist across later buffer reuse
                for t in range(ndh):
                    nc.vector.memset(qf[t][:, 1, :], 0.0)
            return qf

        for c in range(qbufs):
            make_qf(c, memset=True)

        # ---- load weights (bf16 from host; batched DMAs) ----
        # spread DMAs across both HWDGE queues (SP + Activation)
        _dmaq = [0]

        def dmaq():
            _dmaq[0] ^= 1
            return nc.sync if _dmaq[0] else nc.scalar

        # wq first, then w_inT in quarters: the first combine matmuls only
        # need w_in chunk u=0 + wq, so PE can start ~2.7us in
        w_sb = {}
        w8_sb = {}
        if qk8:
            # fp8 copies drive the DoubleRow q/k combine (4x MACs/cycle)
            wb_q8 = wpool.tile([P, nde, dh], qkt, tag="wq8", name="wq8_b")
            nc.sync.dma_start(out=wb_q8[:],
                              in_=wq8[:].rearrange("(u p) d -> p u d", p=P))
            w8_sb["q"] = wb_q8
            w_in8_b = wpool.tile([P, nde, dm], qkt, tag="w_in8", name="w_in8_b")
            hd8 = nde // 2
            for i in range(2):
                dmaq().dma_start(
                    out=w_in8_b[:, i * hd8:(i + 1) * hd8, :],
                    in_=w_inT8[i * hd8 * P:(i + 1) * hd8 * P, :]
                        .rearrange("(u p) m -> p u m", p=P))
            wb_k8 = wpool.tile([P, nde, dh], qkt, tag="wk8", name="wk8_b")
            dmaq().dma_start(out=wb_k8[:],
                             in_=wk8[:].rearrange("(u p) d -> p u d", p=P))
            w8_sb["k"] = wb_k8
        w_inT_b = wpool.tile([P, nde, dm], st, tag="w_inT", name="w_inT_b")
        w_inT_sb = [w_inT_b[:, u, :] for u in range(nde)]
        b_inT = wpool.tile([P, nde], st, tag="b_inT", name="b_inT")
        bo_bcast = persist.tile([P, dm], F32, tag="bo_b", name="bo_bcast")
        for name in ("q", "k", "v"):
            wb = wpool.tile([P, nde, dh], st, tag=f"w{name}", name=f"w{name}_b")
            w_sb[name] = [wb[:, u, :] for u in range(nde)]
            w_sb[name + "_t"] = wb

        def load_w_small(name):
            # bf16 head-projection weights (bias combine) + b_in
            def f():
                dmaq().dma_start(
                    out=w_sb[name + "_t"][:],
                    in_={"q": wq, "k": wk, "v": wv}[name]
                        .rearrange("(u p) d -> p u d", p=P))
                if name == "q":
                    nc.scalar.dma_start(
                        out=b_inT[:], in_=b_in[:].rearrange("(t p) -> p t", p=P))
            return f

        def load_w_inT_bf16():
            hd4 = nde // 4
            for i in range(4):
                dmaq().dma_start(
                    out=w_inT_b[:, i * hd4:(i + 1) * hd4, :],
                    in_=w_inT[i * hd4 * P:(i + 1) * hd4 * P, :]
                        .rearrange("(u p) m -> p u m", p=P))
            nc.scalar.dma_start(out=bo_bcast[:],
                                in_=bo[:].unsqueeze(0).broadcast_to([P, dm]))
        wo_sb = persist.tile([P, nde, dm], st, tag="wo", name="wo_b")

        def load_wo():
            # deferred: wo is not needed until the first output projection
            hdo = nde // 2
            nc.sync.dma_start(out=wo_sb[:, 0:hdo, :],
                              in_=wo[0:hdo * P, :].rearrange("(u p) m -> p u m", p=P))
            nc.scalar.dma_start(out=wo_sb[:, hdo:nde, :],
                                in_=wo[hdo * P:, :].rearrange("(u p) m -> p u m", p=P))

        # ---- combine weights: Wc_x = W_in @ Wx (+ bias fold) ----
        # q/k: fp8 DoubleRow over paired de-chunks -> paired-layout wc8
        # (wc8[name][t//2][:, t%2, :] = Wc rows of dm-chunk t); v: bf16
        wc = {}
        wc8 = {}
        bc = {}

        def combine_qk8(name):
            wc8[name] = [persist.tile([P, 2, dh], qkt, tag=f"wc8{name}{t}",
                                      name=f"wc8{name}{t}")
                         for t in range(ndm // 2)]
            for tp in range(ndm // 2):
                ps = ps_w.tile([P, 2 * dh], F32, tag="ps_w", name="ps_w")
                for half in range(2):
                    t = 2 * tp + half
                    for i in range(nde // 2):
                        nc.tensor.matmul(
                            ps[:, half * dh:(half + 1) * dh],
                            w_in8_b[:, 2 * i:2 * i + 2, t * P:(t + 1) * P],
                            w8_sb[name][:, 2 * i:2 * i + 2, :],
                            perf_mode=DR0,
                            start=(i == 0), stop=(i == nde // 2 - 1))
                nc.vector.tensor_copy(
                    wc8[name][tp][:],
                    ps[:].rearrange("p (two d) -> p two d", two=2))

        def combine_bf16_closures(name):
            wc[name] = [persist.tile([P, dh], st, tag=f"wc{name}{t}",
                                     name=f"wc{name}{t}") for t in range(ndm)]

            def piece(ts_):
                def f():
                    for t in ts_:
                        ps = ps_w.tile([P, dh], F32, tag="ps_w", name="ps_w")
                        for u in range(nde):
                            nc.tensor.matmul(
                                ps[:], w_inT_sb[u][:, t * P:(t + 1) * P],
                                w_sb[name][u][:],
                                start=(u == 0), stop=(u == nde - 1))
                        nc.vector.tensor_copy(wc[name][t][:], ps[:])
                return f
            return [piece(ts_) for ts_ in
                    ([0, 1], [2, 3], [4, 5], [6, 7])]

        def bias_qk(name):
            bvec = {"q": bq, "k": bk}[name]
            bxT = wpool.tile([P, ndh], F32, tag=f"bxT{name}", name=f"bxT{name}")
            nc.sync.dma_start(out=bxT[:], in_=bvec[:].rearrange("(t p) -> p t", p=P))
            bc[name] = persist.tile([P, ndh], F32, tag=f"bc{name}", name=f"bc{name}")
            for t in range(ndh):
                ps = ps_w.tile([P, 1], F32, tag="ps_w", name="ps_w")
                for u in range(nde):
                    nc.tensor.matmul(
                        ps[:], w_sb[name][u][:, t * P:(t + 1) * P],
                        b_inT[:, u:u + 1],
                        start=(u == 0), stop=(u == nde - 1))
                nc.vector.tensor_add(bc[name][:, t:t + 1], ps[:], bxT[:, t:t + 1])

        bcv_row = persist.tile([1, dh], st, tag="bcv", name="bcv_row")

        def bias_v():
            bv_row = wpool.tile([1, dh], F32, tag="bv_row", name="bv_row")
            nc.sync.dma_start(out=bv_row[:], in_=bv[:].unsqueeze(0))
            ps = ps_w.tile([1, dh], F32, tag="ps_w", name="ps_w")
            for u in range(nde):
                nc.tensor.matmul(ps[:], b_inT[:, u:u + 1], w_sb["v"][u][:],
                                 start=(u == 0), stop=(u == nde - 1))
            nc.vector.tensor_add(bcv_row[:], ps[:], bv_row[:])

        def proj_closures(iqc):
            """Per-chunk projection emission, split into PE-sized closures."""
            clos = []
            qf = qf_tiles.get(iqc) or make_qf(iqc, memset=iqc < qbufs)
            xbs = {}

            def load(name, xdram):
                def f():
                    dt_ = qkt if (qk8 and name in ("q", "k")) else st
                    tag = "xb8" if (qk8 and name in ("q", "k")) else "xb"
                    xb = xpool.tile([P, ndm, QC], dt_, tag=tag,
                                    name=f"xb_{name}{iqc}")
                    xbs[name] = xb
                    dmaq().dma_start(
                        out=xb[:],
                        in_=xdram[:, iqc * QC:(iqc + 1) * QC]
                            .rearrange("(u p) s -> p u s", p=P))
                return f

            def qk_part(name, t):
                def f():
                    xb = xbs[name]
                    ps = ps_w.tile([P, QC], F32, tag="ps_w", name="ps_w")
                    if qk8:
                        for i in range(ndm // 2):
                            nc.tensor.matmul(
                                ps[:],
                                wc8[name][i][:, :, t * P:(t + 1) * P],
                                xb[:, 2 * i:2 * i + 2, :],
                                perf_mode=DR0,
                                start=(i == 0), stop=(i == ndm // 2 - 1))
                    else:
                        for u in range(ndm):
                            nc.tensor.matmul(
                                ps[:], wc[name][u][:, t * P:(t + 1) * P],
                                xb[:, u, :], start=(u == 0),
                                stop=(u == ndm - 1))
                    if name == "k":
                        nc.vector.tensor_scalar_add(
                            kT_sb[t][:, iqc * QC:(iqc + 1) * QC], ps[:],
                            bc["k"][:, t:t + 1])
                    else:
                        nc.vector.tensor_scalar_add(
                            qf[t][:, 0, :], ps[:], bc["q"][:, t:t + 1])
                return f

            def v_part(j):
                def f():
                    xb = xbs["v"]
                    ikb = iqc * (QC // P) + j
                    ps = ps_w.tile([P, dh], F32, tag="ps_w", name="ps_w")
                    for u in range(ndm):
                        nc.tensor.matmul(
                            ps[:], xb[:, u, j * P:(j + 1) * P], wc["v"][u][:],
                            start=(u == 0), stop=False)
                    nc.tensor.matmul(ps[:], ones1[:], bcv_row[:],
                                     start=False, stop=True)
                    nc.vector.tensor_copy(
                        v_all[:, ikb, :, 0:DK],
                        ps[:].rearrange("p (h d) -> p h d", h=hloc))
                return f

            clos.append(load("q", qT))
            for t in range(ndh):
                clos.append(qk_part("q", t))
            clos.append(load("k", kT))
            for t in range(ndh):
                clos.append(qk_part("k", t))
            clos.append(load("v", vT))
            for j in range(QC // P):
                clos.append(v_part(j))
            return clos

        # ---- attention ----
        inv_sqrt = 1.0 / math.sqrt(DK)
        DR = mybir.MatmulPerfMode.DoubleRow
        cT_tiles = {}

        def attention_chunk(iqc, fillers=(), mid=None):
            """QK+exp of step i overlaps PV of step i-1; `fillers` (next
            chunk's projection closures) are spread over the early steps;
            `mid` (the previous chunk's yin load) fires ~70% through."""
            qf = qf_tiles[iqc]
            cT = cpool.tile([P, ndh, QC], st, tag="cT", name=f"cT{iqc}")
            cT_tiles[iqc] = cT
            blist = blocks[iqc]
            steps = []
            for h in range(hloc):
                grps = [blist[g0:g0 + GW] for g0 in range(0, len(blist), GW)]
                for g in range(len(grps)):
                    steps.append((h, grps[g], g == 0, g == len(grps) - 1))
            po = {}
            pending = []
            fillers = list(fillers)
            n_steps = len(steps)
            fill_at = {}
            if fillers:
                # spread fillers uniformly across the steps
                for fi in range(len(fillers)):
                    at = (fi * n_steps) // len(fillers)
                    fill_at.setdefault(min(at, n_steps - 1), []).append(
                        fillers[fi])
            mid_at = (6 * n_steps) // 10

            def emit_qk_exp(h, grp):
                t, off = h // 2, (h % 2) * DK
                pss = ps_s.tile([P, GW * QC], F32, tag="ps_scores",
                                name="ps_scores")
                for j, (ikb, mode, arg) in enumerate(grp):
                    kv = kT_sb[t][off:off + DK, ikb * KB:(ikb + 1) * KB]
                    if qk8:
                        nc.tensor.matmul(
                            pss[:, j * QC:(j + 1) * QC],
                            kv.unsqueeze(1).broadcast_to([DK, 2, KB]),
                            qf[t][off:off + DK, :, :],
                            perf_mode=DR, start=True, stop=True)
                    else:
                        nc.tensor.matmul(pss[:, j * QC:(j + 1) * QC],
                                         kv, qf[t][off:off + DK, 0, :])
                pt = ppool.tile([P, GW * QC], pvt, tag="p", name="p")
                nw = len(grp) * QC
                nc.scalar.activation(pt[:, 0:nw], pss[:, 0:nw],
                                     mybir.ActivationFunctionType.Exp,
                                     scale=inv_sqrt)
                for j, (ikb, mode, arg) in enumerate(grp):
                    pj = pt[:, j * QC:(j + 1) * QC]
                    if mode == TRI:
                        r = arg
                        if r > 0:
                            nc.gpsimd.memset(pj[:, 0:r * P], 0.0)
                        nc.vector.tensor_mul(
                            pj[:, r * P:(r + 1) * P],
                            pj[:, r * P:(r + 1) * P], tri[:])
                    elif mode == GEN:
                        nc.vector.tensor_mul(pj[:], pj[:], gen_sb[:, arg, :])
                return pt

            def emit_pv(h, grp, pt, first, last):
                if first:
                    po[h] = ps_o.tile([VW, QC], F32, tag="ps_av",
                                      name="ps_av")
                ikbs = [ikb for (ikb, _, _) in grp]
                if pv8 and len(grp) == 2 and ikbs[1] == ikbs[0] + 1:
                    nc.tensor.matmul(
                        po[h][:], v_all[:, ikbs[0]:ikbs[0] + 2, h, :],
                        pt[:].rearrange("p (two q) -> p two q", two=2),
                        perf_mode=DR,
                        start=first, stop=last)
                else:
                    for j, (ikb, mode, arg) in enumerate(grp):
                        nc.tensor.matmul(
                            po[h][:, 0:QC], v_all[:, ikb, h, :],
                            pt[:, j * QC:(j + 1) * QC],
                            start=(first and j == 0),
                            stop=(last and j == len(grp) - 1))
                if last:
                    rec1 = work.tile([1, QC], F32, tag="rec1", name="rec1")
                    nc.vector.reciprocal(rec1[:], po[h][DK:DK + 1, :])
                    recb = work.tile([DK, QC], F32, tag="recb", name="recb")
                    nc.gpsimd.partition_broadcast(recb[:], rec1[:])
                    nc.vector.tensor_mul(
                        cT[(h % 2) * DK:(h % 2) * DK + DK, h // 2, :],
                        po[h][0:DK, :], recb[:])
                    del po[h]

            for i, (h, grp, first, last) in enumerate(steps):
                pt = emit_qk_exp(h, grp)
                pending.append((h, grp, pt, first, last))
                if len(pending) > 3:
                    emit_pv(*pending.pop(0))
                for f in fill_at.get(i, ()):
                    f()
                if mid is not None and i == mid_at:
                    mid()
                    mid = None
            for p_ in pending:
                emit_pv(*p_)
            if mid is not None:
                mid()

        # ---- a2a + output projection ----
        def a2a_chunk(c):
            # cT [128, ndh, QC] -> a2a_in[c or :, c] [8, 256, 64]
            cT = cT_tiles[c]
            dst = a2a_in[c] if chunked_cc else a2a_in[:, c]
            for fh in range(ndh):
                (nc.sync if fh % 2 == 0 else nc.scalar).dma_start(
                    out=dst[:, fh * P:(fh + 1) * P, :]
                        .rearrange("r p j -> p r j"),
                    in_=cT[:, fh, :].rearrange("p (r j) -> p r j", r=nsub))
            if not chunked_cc:
                if c == nqc - 1:
                    if collective:
                        for _ in range(cc_reps):
                            nc.gpsimd.collective_compute(
                                "AllToAll", mybir.AluOpType.bypass,
                                replica_groups=[list(range(N_CORES))],
                                ins=[a2a_in[:].opt()], outs=[a2a_out[:].opt()])
                    else:
                        nc.sync.dma_start(out=a2a_out[:], in_=a2a_in[:])
                return
            if collective:
                for _ in range(cc_reps):
                    nc.gpsimd.collective_compute(
                        "AllToAll", mybir.AluOpType.bypass,
                        replica_groups=[list(range(N_CORES))],
                        ins=[a2a_in[c].opt()], outs=[a2a_out[c].opt()])
            else:
                nc.sync.dma_start(out=a2a_out[c], in_=a2a_in[c])

        yin_tiles = {}

        def yin_load(c):
            yin = ypool.tile([P, nde, P], st, tag="yin", name=f"yin{c}")
            yin_tiles[c] = yin
            half = nsub // 2
            if chunked_cc:
                src = a2a_out[c]
                nc.sync.dma_start(
                    out=yin[:, :, 0:64],
                    in_=src[0:half].rearrange("s (fh p) j -> p (s fh) j", p=P))
                nc.scalar.dma_start(
                    out=yin[:, :, 64:128],
                    in_=src[half:nsub].rearrange("s (fh p) j -> p (s fh) j",
                                                 p=P))
            else:
                yv = yin[:].rearrange("p (s fh) j -> p s fh j", fh=ndh)
                for b0, sl in ((0, slice(0, 64)), (half, slice(64, 128))):
                    for fh in range(ndh):
                        nc.sync.dma_start(
                            out=yv[:, :, fh, sl],
                            in_=a2a_out[b0:b0 + half, c, fh * P:(fh + 1) * P, :]
                                .rearrange("s p j -> p s j"))

        def yproj_mm(c):
            yin = yin_tiles[c]
            ys = ypool.tile([P, dm], F32, tag="ys", name=f"ys{c}")
            for mb in range(dm // QC):
                ps = ps_w.tile([P, QC], F32, tag="ps_w", name="ps_w")
                for u in range(nde):
                    nc.tensor.matmul(
                        ps[:], yin[:, u, :],
                        wo_sb[:, u, mb * QC:(mb + 1) * QC],
                        start=(u == 0), stop=(u == nde - 1))
                nc.vector.tensor_add(ys[:, mb * QC:(mb + 1) * QC], ps[:],
                                     bo_bcast[:, mb * QC:(mb + 1) * QC])
            nc.sync.dma_start(out=y_out[c * P:(c + 1) * P, :], in_=ys[:])

        # ---- schedule ----
        # head: score-path (fp8) combine + q/k projections first so the
        # first attention exp fires ~7us in; the bf16 v-combine and chunk-0
        # V projection become PE filler inside attention chunk 0.
        # yproj(c) is deferred into later chunks' attention as PE filler:
        # the final chunks have the most exp work and no projections left.
        if causal_dep:
            pc0 = proj_closures(0)
            pc0[0]()                       # load q chunk 0
            pc0[3]()                       # load k chunk 0
            if qk8:
                load_w_small("q")()
                load_w_small("k")()
                combine_qk8("q")
                bias_qk("q")
                pc0[1](); pc0[2]()         # project q chunk 0
                combine_qk8("k")
                bias_qk("k")
                pc0[4](); pc0[5]()         # project k chunk 0
            else:
                load_w_small("q")()
                load_w_small("k")()
                load_w_inT_bf16()
                for f in combine_bf16_closures("q"):
                    f()
                bias_qk("q")
                pc0[1](); pc0[2]()
                for f in combine_bf16_closures("k"):
                    f()
                bias_qk("k")
                pc0[4](); pc0[5]()
            pc0[6]()                       # load v chunk 0
            load_w_small("v")()
            if qk8:
                load_w_inT_bf16()
            head_fill = combine_bf16_closures("v") + [bias_v]
            load_wo()
            for c in range(nqc):
                fillers = list(head_fill)
                head_fill = []
                if c == 0:
                    fillers += pc0[7:]     # chunk-0 V projection
                if c + 1 < nqc:
                    fillers += list(proj_closures(c + 1))
                if chunked_cc and c == nqc - 1:
                    for cc in range(nqc - 2):
                        fillers.append((lambda c2: lambda: yproj_mm(c2))(cc))
                mid = ((lambda cc: lambda: yin_load(cc))(c - 1)
                       if c > 0 and chunked_cc else None)
                attention_chunk(c, fillers=fillers, mid=mid)
                a2a_chunk(c)
            if chunked_cc:
                yproj_mm(nqc - 2)
                pe_keepwarm(TAIL_WARM)
            else:
                for c in range(nqc - 1):
                    yin_load(c)
                    yproj_mm(c)
        else:
            # general masks: all projections first, then attention
            load_w_small("q")()
            load_w_small("k")()
            load_w_small("v")()
            load_w_inT_bf16()
            if qk8:
                combine_qk8("q")
                combine_qk8("k")
            else:
                for f in combine_bf16_closures("q") + combine_bf16_closures("k"):
                    f()
            bias_qk("q")
            bias_qk("k")
            for f in combine_bf16_closures("v"):
                f()
            bias_v()
            for c in range(nqc):
                for f in proj_closures(c):
                    f()
            load_wo()
            for c in range(nqc):
                mid = (lambda cc: lambda: yin_load(cc))(c - 1) if c > 0 else None
                attention_chunk(c, mid=mid)
                a2a_chunk(c)
                if c > 0:
                    yproj_mm(c - 1)
        yin_load(nqc - 1)
        yproj_mm(nqc - 1)

    nc.compile()
    return nc


# ------------------------------------------------------------------
_CACHE = {}


def _get_compiled(plan_key, blocks, n_gen, mm):
    if plan_key not in _CACHE:
        nc = build_mha(blocks, n_gen, mm=mm)
        nc.m = get_hw_module(nc.m)
        _CACHE[plan_key] = nc
    return _CACHE[plan_key]


def make_in_maps(q, k, v, mask, W_in, b_in, Wq, bq, Wk, bk, Wv, bv, Wo, bo,
                 blocks=None, n_gen=None, gen_tiles=None):
    if blocks is None:
        blocks, n_gen, gen_tiles = make_plan(mask)
    bf16 = mybir.dt.np(BF16)
    fp8 = mybir.dt.np(FP8)
    dh = DE // 4
    tb = lambda a: np.ascontiguousarray(np.asarray(a).T).astype(bf16)
    cb = lambda a: np.ascontiguousarray(np.asarray(a)).astype(bf16)
    t8 = lambda a: np.ascontiguousarray(np.asarray(a).T).astype(fp8)
    c8 = lambda a: np.ascontiguousarray(np.asarray(a)).astype(fp8)
    in_maps = []
    for c in range(N_CORES):
        b, g = c // 4, c % 4
        sl = slice(g * dh, (g + 1) * dh)
        mt = (gen_tiles[b] if n_gen else
              np.zeros((1, KB, QC), np.int32))
        qk8 = MM_MODE in ("fp8", "fp8qk")
        tq = t8 if qk8 else tb
        in_maps.append({
            "qT": tq(q[b]), "kT": tq(k[b]), "vT": tb(v[b]),
            "w_inT": tb(W_in), "w_inT8": t8(W_in),
            "wq": cb(Wq[:, sl]),
            "wk": cb(Wk[:, sl]),
            "wq8": c8(Wq[:, sl]),
            "wk8": c8(Wk[:, sl]),
            "wv": cb(Wv[:, sl]),
            "wo": cb(Wo),
            "b_in": np.asarray(b_in).astype(bf16),
            "bq": np.ascontiguousarray(np.asarray(bq)[sl]),
            "bk": np.ascontiguousarray(np.asarray(bk)[sl]),
            "bv": np.ascontiguousarray(np.asarray(bv)[sl]),
            "bo": np.asarray(bo),
            "m_tiles": mt,
        })
    return in_maps, blocks, n_gen


def assemble(results):
    out = np.empty((B, S, DM), np.float32)
    for core in range(N_CORES):
        y = results[core]["y_out"]            # [nqc*128, DM]
        for c in range(S // QC):
            for b in range(B):
                out[b, c * QC + core * 64:c * QC + (core + 1) * 64, :] = \
                    y[c * P + b * 64:c * P + (b + 1) * 64, :]
    return out


MM_MODE = "fp8qk"


def kernel(**inputs):
    mask = inputs["mask"]
    blocks, n_gen, gen_tiles = make_plan(np.asarray(mask))
    plan_key = (str(blocks), n_gen, MM_MODE)
    nc = _get_compiled(plan_key, blocks, n_gen, MM_MODE)
    in_maps, _, _ = make_in_maps(
        inputs["q"], inputs["k"], inputs["v"], mask,
        inputs["W_in"], inputs["b_in"], inputs["Wq"], inputs["bq"],
        inputs["Wk"], inputs["bk"], inputs["Wv"], inputs["bv"],
        inputs["Wo"], inputs["bo"],
        blocks=blocks, n_gen=n_gen, gen_tiles=gen_tiles)
    res = bass_utils.run_bass_kernel_spmd(nc, in_maps,
                                          core_ids=list(range(N_CORES)))
    return assemble(res.results)


# revision 7
# speedup vs baseline: 1.0385x; 1.0036x over previous
"""Trainium2 Bass kernel for nn_MultiHeadAttention_72069551227273 (v2).

Reference computation (B=2, S=2048, D_MODEL=D_EMB=1024, H=16, d_k=64):
    q_p = q @ W_in + b_in                    (shared input projection)
    qh  = heads(q_p @ Wq + bq)               (per-head projections)
    s   = qh @ kh^T / sqrt(d_k), causal-masked softmax
    out = (attn @ vh, concat heads) @ Wo + bo

Sharding: 8 cores = 2 (batch) x 4 (head groups of 4 heads / 256 emb cols).
Per core the input and head projections are fused on device:
    Q = q @ (W_in @ Wq_slice) + (b_in @ Wq_slice + bq_slice)
The whole score path (W_in@Wq/Wk weight combine, Q/K projections, QK^T)
runs in fp8e4m3 DoubleRow perf mode with contraction chunks paired into
the two DoubleRow slots: 4x MACs/cycle over bf16 for combine+projections
and 2x for scores (score lhsT slots = (K, K) via a stride-0 broadcast,
rhs slots = (Q, 0)).  fp8 score noise washes out in the softmax; the
value path (V, attn@V, Wo) stays bf16 — fp8 there measured 2.4e-2
relative error, over the 2e-2 gate.  V is projected straight into the
natural [seq, head, d_k] layout (no PE transposes) with its bias folded
in via a rank-1 ones matmul.  Softmax is exp(s/8) without max-subtraction;
the denominator comes free from a ones column appended to V.  Fully-masked
score blocks are skipped at trace time, diagonal blocks get an on-chip
triangular mask.  Projection work for chunk c+1 is interleaved into the
attention steps of chunk c (attention is Act/exp-heavy, projections are
PE-heavy), and attention itself is software-pipelined so PV of step i-1
overlaps the exp of step i.

Output stage is sequence-parallel instead of tensor-parallel: after
attention chunk c, an 8-way AllToAll (bf16, 256 KB) redistributes the
attention outputs so every core holds all 1024 features for 64 q rows of
each batch, then applies the full Wo locally — there is no reduction
collective at all.  The per-chunk collectives overlap with the next
chunk's attention; only the last chunk's exchange is exposed.
"""

import sys

sys.path.append("/opt/trn_rl_repo")

import math
from contextlib import ExitStack

import numpy as np

import concourse.bass as bass
import concourse.bacc as bacc
import concourse.mybir as mybir
import concourse.tile as tile
from concourse import bass_utils
from concourse.bass_interp import get_hw_module

# problem dims
B, S, DM, DE, H, DK = 2, 2048, 1024, 1024, 16, 64
N_CORES = 8
P = 128                      # partitions
QC = 512                     # q chunk (psum bank width in fp32)
KB = 128                     # k block (scores^T partition block)
GW = 2                       # kb blocks per score-psum tile (2 banks)
TAIL_WARM = 58               # PE keep-warm matmuls bridging the last a2a wait

F32 = mybir.dt.float32
BF16 = mybir.dt.bfloat16
FP8 = mybir.dt.float8e4

FULL, TRI, GEN, SKIP = 0, 1, 2, 3


def make_plan(mask_np, s=S, qc=QC, kb=KB):
    """Classify scores^T blocks [kb x qc] from the (B, S, S) 0/1 mask.

    Returns (blocks, n_gen_tiles, gen_tiles_per_batch):
      blocks[iqc] = list of (ikb, mode, arg)
    """
    nqc, nkb = s // qc, s // kb
    m = np.asarray(mask_np) != 0          # [B, S(q), S(k)] True = attend
    tril = np.tril(np.ones((s, s), bool))
    causal = all(np.array_equal(m[b], tril) for b in range(m.shape[0]))
    blocks = []
    if causal:
        for iqc in range(nqc):
            row = []
            for ikb in range(nkb):
                if (ikb + 1) * kb <= iqc * qc:
                    row.append((ikb, FULL, 0))
                elif ikb * kb < (iqc + 1) * qc:
                    row.append((ikb, TRI, (ikb * kb - iqc * qc) // kb))
                # else fully masked -> skip
            blocks.append(row)
        return blocks, 0, None

    # general path: per-block classification, unioned across batches
    nb = m.shape[0]
    # every query row must attend to >= 1 key (else softmax semantics differ)
    assert m.any(axis=-1).all(), "fully-masked query rows unsupported"
    gen_tiles = [[] for _ in range(nb)]
    for iqc in range(nqc):
        row = []
        for ikb in range(nkb):
            sub = m[:, iqc * qc:(iqc + 1) * qc, ikb * kb:(ikb + 1) * kb]
            if sub.all():
                row.append((ikb, FULL, 0))
            elif not sub.any():
                continue
            else:
                idx = len(gen_tiles[0])
                for b in range(nb):
                    gen_tiles[b].append(sub[b].T.astype(np.int32))  # [kb, qc]
                row.append((ikb, GEN, idx))
        blocks.append(row)
    n_gen = len(gen_tiles[0])
    gt = [np.stack(g) if n_gen else np.zeros((1, kb, qc), np.int32)
          for g in gen_tiles]
    return blocks, n_gen, gt


def build_mha(blocks, n_gen, *, s=S, dm=DM, de=DE, dh=None, mm="fp8",
              collective=True, chunked_cc=True, cc_reps=1):
    """Trace the per-core MHA program.  dh = per-core emb slice (256)."""
    if dh is None:
        dh = DE // 4
    nqc, nkb, ndm, nde = s // QC, s // KB, dm // P, de // P
    ndh = dh // P            # feature chunks per core (2)
    hloc = dh // DK          # heads per core (4)
    nsub = QC // 64          # a2a sub-blocks per chunk (8)
    out_rows = nqc * P       # output rows per core (4 chunks x 2 x 64)

    qk8 = mm in ("fp8", "fp8qk")
    pv8 = (mm == "fp8")
    qkt = FP8 if qk8 else BF16
    pvt = FP8 if pv8 else BF16   # dtype of probs, V, and mask tiles
    st = BF16

    # can attention chunk c start right after projection chunk c?
    causal_dep = all(
        max([c] + [ikb * KB // QC for (ikb, _, _) in blocks[c]]) <= c
        for c in range(nqc))

    nc = bacc.Bacc("TRN2", target_bir_lowering=False, debug=False,
                   num_devices=N_CORES)

    # ---- kernel I/O (per core) ----
    qT = nc.dram_tensor("qT", [dm, s], qkt, kind="ExternalInput")
    kT = nc.dram_tensor("kT", [dm, s], qkt, kind="ExternalInput")
    vT = nc.dram_tensor("vT", [dm, s], BF16, kind="ExternalInput")
    w_inT = nc.dram_tensor("w_inT", [de, dm], BF16, kind="ExternalInput")
    w_inT8 = nc.dram_tensor("w_inT8", [de, dm], qkt, kind="ExternalInput")
    wq = nc.dram_tensor("wq", [de, dh], BF16, kind="ExternalInput")
    wk = nc.dram_tensor("wk", [de, dh], BF16, kind="ExternalInput")
    wq8 = nc.dram_tensor("wq8", [de, dh], qkt, kind="ExternalInput")
    wk8 = nc.dram_tensor("wk8", [de, dh], qkt, kind="ExternalInput")
    wv = nc.dram_tensor("wv", [de, dh], BF16, kind="ExternalInput")
    wo = nc.dram_tensor("wo", [de, dm], BF16, kind="ExternalInput")
    b_in = nc.dram_tensor("b_in", [de], BF16, kind="ExternalInput")
    bq = nc.dram_tensor("bq", [dh], F32, kind="ExternalInput")
    bk = nc.dram_tensor("bk", [dh], F32, kind="ExternalInput")
    bv = nc.dram_tensor("bv", [dh], F32, kind="ExternalInput")
    bo = nc.dram_tensor("bo", [dm], F32, kind="ExternalInput")
    m_tiles = nc.dram_tensor("m_tiles", [max(n_gen, 1), KB, QC], mybir.dt.int32,
                             kind="ExternalInput")
    y_out = nc.dram_tensor("y_out", [out_rows, dm], F32, kind="ExternalOutput")

    # a2a staging: chunked mode [chunk][8 dest blocks][256 feats][64 q];
    # single mode [8 dest blocks][chunk][256 feats][64 q] (one collective)
    if chunked_cc:
        a2a_in = nc.dram_tensor("a2a_in", [nqc, nsub, dh, 64], BF16)
        a2a_out = nc.dram_tensor("a2a_out", [nqc, nsub, dh, 64], BF16)
    else:
        a2a_in = nc.dram_tensor("a2a_in", [nsub, nqc, dh, 64], BF16)
        a2a_out = nc.dram_tensor("a2a_out", [nsub, nqc, dh, 64], BF16)

    with tile.TileContext(nc) as tc, ExitStack() as ex:
        persist = ex.enter_context(tc.tile_pool(name="persist", bufs=1))
        work = ex.enter_context(tc.tile_pool(name="work", bufs=4))
        ps_w = ex.enter_context(tc.tile_pool(name="ps_w", bufs=2, space="PSUM"))
        ps_s = ex.enter_context(tc.tile_pool(name="ps_s", bufs=2, space="PSUM"))
        ps_o = ex.enter_context(tc.tile_pool(name="ps_o", bufs=2, space="PSUM"))
        qbufs = 2 if causal_dep else nqc
        xpool = ex.enter_context(tc.tile_pool(name="xpool", bufs=4))
        qpool = ex.enter_context(tc.tile_pool(name="qpool", bufs=qbufs))
        ppool = ex.enter_context(tc.tile_pool(name="ppool", bufs=8))
        cpool = ex.enter_context(tc.tile_pool(name="cpool", bufs=2))
        ypool = ex.enter_context(tc.tile_pool(name="ypool", bufs=2))
        wpool = ex.enter_context(tc.tile_pool(name="wpool", bufs=1))

        # ---- constants ----
        # tri[k, q] = 1.0 where k <= q (keep), else 0
        tri = persist.tile([P, P], pvt, tag="tri", name="tri")
        tri_b = persist.tile([P, P], st, tag="tri_b", name="tri_b")
        nc.gpsimd.memset(tri_b[:], 0.0)
        nc.gpsimd.affine_select(out=tri_b[:], in_=tri_b[:],
                                compare_op=mybir.AluOpType.is_gt,
                                fill=1.0, base=0,
                                pattern=[[-1, P]], channel_multiplier=1)
        if pvt == st:
            tri = tri_b
        else:
            nc.vector.tensor_copy(tri[:], tri_b[:])
        ones1 = persist.tile([1, P], st, tag="ones1", name="ones1")
        nc.gpsimd.memset(ones1[:], 1.0)
        # preload the Exp table while DMAs stream in
        actwarm = persist.tile([1, 1], F32, tag="actwarm", name="actwarm")
        nc.scalar.activation(actwarm[:], ones1[0:1, 0:1],
                             mybir.ActivationFunctionType.Exp)
        scr = persist.tile([1, QC], st, tag="scr", name="scr")
        nc.vector.memset(scr[:], 1.0)
        HEAD_WARM = 11

        def pe_keepwarm(n):
            # dummy matmuls bridge a PE idle window so the clock does not
            # drop out of max p-state before the next real matmul burst
            pwu = ps_o.tile([1, QC], F32, tag="ps_av", name="pwu")
            for i in range(n):
                nc.tensor.matmul(pwu[:], scr[0:1, 0:1], scr[:],
                                 start=(i == 0), stop=(i == n - 1))

        gen_sb = None
        if n_gen:
            gen_sb = persist.tile([P, n_gen, QC], pvt, tag="gen", name="gen")
            gi = persist.tile([P, n_gen, QC], mybir.dt.int32, tag="gen_i",
                              name="gen_i")
            nc.sync.dma_start(gi[:], m_tiles[:].rearrange("n p q -> p n q"))
            for i in range(n_gen):
                if pvt == st:
                    nc.vector.tensor_copy(gen_sb[:, i, :], gi[:, i, :])
                else:
                    gb = work.tile([P, QC], st, tag="gen_b", name="gen_b")
                    nc.vector.tensor_copy(gb[:], gi[:, i, :])
                    nc.vector.tensor_copy(gen_sb[:, i, :], gb[:])

        DR0 = mybir.MatmulPerfMode.DoubleRow

        # ---- persistent activation storage (memsets run at t=0) ----
        kT_sb = [persist.tile([P, s], qkt, tag=f"kT{t}", name=f"kT{t}")
                 for t in range(ndh)]
        # V in natural layout, heads side by side, with a ones column:
        # v_all[kb_row, ikb, h, 0:DK] = v_h[key, :], v_all[.., DK] = 1
        # fp8 DoubleRow ldweights needs 4-byte-aligned slot strides: pad
        # each head's [d_k | ones] slot to VW columns (tail zeroed)
        VW = DK + 4 if pv8 else DK + 1
        v_all = persist.tile([P, nkb, hloc, VW], pvt, tag="v_all",
                             name="v_all")
        nc.gpsimd.memset(v_all[:, :, :, DK], 1.0)
        if VW > DK + 1:
            nc.gpsimd.memset(v_all[:, :, :, DK + 1:VW], 0.0)

        qf_tiles = {}

        def make_qf(iqc, memset=False):
            qf = [qpool.tile([P, 2, QC], qkt, tag=f"qf{t}", name=f"qf{t}_{iqc}")
                  for t in range(ndh)]
            qf_tiles[iqc] = qf
            if memset and qk8:
                # pool ring: zero slots persist across later buffer reuse
                for t in range(ndh):
                    nc.vector.memset(qf[t][:, 1, :], 0.0)
            return qf

        for c in range(qbufs):
            make_qf(c, memset=True)

        # ---- load weights (bf16 from host; batched DMAs) ----
        # spread DMAs across both HWDGE queues (SP + Activation)
        _dmaq = [0]

        def dmaq():
            _dmaq[0] ^= 1
            return nc.sync if _dmaq[0] else nc.scalar

        # wq first, then w_inT in quarters: the first combine matmuls only
        # need w_in chunk u=0 + wq, so PE can start ~2.7us in
        w_sb = {}
        w8_sb = {}
        if qk8:
            # fp8 copies drive the DoubleRow q/k combine (4x MACs/cycle)
            wb_q8 = wpool.tile([P, nde, dh], qkt, tag="wq8", name="wq8_b")
            nc.sync.dma_start(out=wb_q8[:],
                              in_=wq8[:].rearrange("(u p) d -> p u d", p=P))
            w8_sb["q"] = wb_q8
            w_in8_b = wpool.tile([P, nde, dm], qkt, tag="w_in8", name="w_in8_b")
            hd8 = nde // 2
            for i in range(2):
                dmaq().dma_start(
                    out=w_in8_b[:, i * hd8:(i + 1) * hd8, :],
                    in_=w_inT8[i * hd8 * P:(i + 1) * hd8 * P, :]
                        .rearrange("(u p) m -> p u m", p=P))
            wb_k8 = wpool.tile([P, nde, dh], qkt, tag="wk8", name="wk8_b")
            dmaq().dma_start(out=wb_k8[:],
                             in_=wk8[:].rearrange("(u p) d -> p u d", p=P))
            w8_sb["k"] = wb_k8
        w_inT_b = wpool.tile([P, nde, dm], st, tag="w_inT", name="w_inT_b")
        w_inT_sb = [w_inT_b[:, u, :] for u in range(nde)]
        b_inT = wpool.tile([P, nde], st, tag="b_inT", name="b_inT")
        bo_bcast = persist.tile([P, dm], F32, tag="bo_b", name="bo_bcast")
        for name in ("q", "k", "v"):
            wb = wpool.tile([P, nde, dh], st, tag=f"w{name}", name=f"w{name}_b")
            w_sb[name] = [wb[:, u, :] for u in range(nde)]
            w_sb[name + "_t"] = wb

        def load_w_small(name):
            # bf16 head-projection weights (bias combine) + b_in
            def f():
                dmaq().dma_start(
                    out=w_sb[name + "_t"][:],
                    in_={"q": wq, "k": wk, "v": wv}[name]
                        .rearrange("(u p) d -> p u d", p=P))
                if name == "q":
                    nc.scalar.dma_start(
                        out=b_inT[:], in_=b_in[:].rearrange("(t p) -> p t", p=P))
            return f

        def load_w_inT_bf16():
            hd4 = nde // 4
            for i in range(4):
                dmaq().dma_start(
                    out=w_inT_b[:, i * hd4:(i + 1) * hd4, :],
                    in_=w_inT[i * hd4 * P:(i + 1) * hd4 * P, :]
                        .rearrange("(u p) m -> p u m", p=P))
            nc.scalar.dma_start(out=bo_bcast[:],
                                in_=bo[:].unsqueeze(0).broadcast_to([P, dm]))
        wo_sb = persist.tile([P, nde, dm], st, tag="wo", name="wo_b")

        def load_wo():
            # deferred: wo is not needed until the first output projection
            hdo = nde // 2
            nc.sync.dma_start(out=wo_sb[:, 0:hdo, :],
                              in_=wo[0:hdo * P, :].rearrange("(u p) m -> p u m", p=P))
            nc.scalar.dma_start(out=wo_sb[:, hdo:nde, :],
                                in_=wo[hdo * P:, :].rearrange("(u p) m -> p u m", p=P))

        # ---- combine weights: Wc_x = W_in @ Wx (+ bias fold) ----
        # q/k: fp8 DoubleRow over paired de-chunks -> paired-layout wc8
        # (wc8[name][t//2][:, t%2, :] = Wc rows of dm-chunk t); v: bf16
        wc = {}
        wc8 = {}
        bc = {}

        def combine_qk8(name):
            wc8[name] = [persist.tile([P, 2, dh], qkt, tag=f"wc8{name}{t}",
                                      name=f"wc8{name}{t}")
                         for t in range(ndm // 2)]
            for tp in range(ndm // 2):
                ps = ps_w.tile([P, 2 * dh], F32, tag="ps_w", name="ps_w")
                for half in range(2):
                    t = 2 * tp + half
                    for i in range(nde // 2):
                        nc.tensor.matmul(
                            ps[:, half * dh:(half + 1) * dh],
                            w_in8_b[:, 2 * i:2 * i + 2, t * P:(t + 1) * P],
                            w8_sb[name][:, 2 * i:2 * i + 2, :],
                            perf_mode=DR0,
                            start=(i == 0), stop=(i == nde // 2 - 1))
                nc.vector.tensor_copy(
                    wc8[name][tp][:],
                    ps[:].rearrange("p (two d) -> p two d", two=2))

        def combine_bf16_closures(name):
            wc[name] = [persist.tile([P, dh], st, tag=f"wc{name}{t}",
                                     name=f"wc{name}{t}") for t in range(ndm)]

            def piece(ts_):
                def f():
                    for t in ts_:
                        ps = ps_w.tile([P, dh], F32, tag="ps_w", name="ps_w")
                        for u in range(nde):
                            nc.tensor.matmul(
                                ps[:], w_inT_sb[u][:, t * P:(t + 1) * P],
                                w_sb[name][u][:],
                                start=(u == 0), stop=(u == nde - 1))
                        nc.vector.tensor_copy(wc[name][t][:], ps[:])
                return f
            return [piece(ts_) for ts_ in
                    ([0, 1], [2, 3], [4, 5], [6, 7])]

        def bias_qk(name):
            bvec = {"q": bq, "k": bk}[name]
            bxT = wpool.tile([P, ndh], F32, tag=f"bxT{name}", name=f"bxT{name}")
            nc.sync.dma_start(out=bxT[:], in_=bvec[:].rearrange("(t p) -> p t", p=P))
            bc[name] = persist.tile([P, ndh], F32, tag=f"bc{name}", name=f"bc{name}")
            for t in range(ndh):
                ps = ps_w.tile([P, 1], F32, tag="ps_w", name="ps_w")
                for u in range(nde):
                    nc.tensor.matmul(
                        ps[:], w_sb[name][u][:, t * P:(t + 1) * P],
                        b_inT[:, u:u + 1],
                        start=(u == 0), stop=(u == nde - 1))
                nc.vector.tensor_add(bc[name][:, t:t + 1], ps[:], bxT[:, t:t + 1])

        bcv_row = persist.tile([1, dh], st, tag="bcv", name="bcv_row")

        def bias_v():
            bv_row = wpool.tile([1, dh], F32, tag="bv_row", name="bv_row")
            nc.sync.dma_start(out=bv_row[:], in_=bv[:].unsqueeze(0))
            ps = ps_w.tile([1, dh], F32, tag="ps_w", name="ps_w")
            for u in range(nde):
                nc.tensor.matmul(ps[:], b_inT[:, u:u + 1], w_sb["v"][u][:],
                                 start=(u == 0), stop=(u == nde - 1))
            nc.vector.tensor_add(bcv_row[:], ps[:], bv_row[:])

        def proj_closures(iqc):
            """Per-chunk projection emission, split into PE-sized closures."""
            clos = []
            qf = qf_tiles.get(iqc) or make_qf(iqc, memset=iqc < qbufs)
            xbs = {}

            def load(name, xdram):
                def f():
                    dt_ = qkt if (qk8 and name in ("q", "k")) else st
                    tag = "xb8" if (qk8 and name in ("q", "k")) else "xb"
                    xb = xpool.tile([P, ndm, QC], dt_, tag=tag,
                                    name=f"xb_{name}{iqc}")
                    xbs[name] = xb
                    dmaq().dma_start(
                        out=xb[:],
                        in_=xdram[:, iqc * QC:(iqc + 1) * QC]
                            .rearrange("(u p) s -> p u s", p=P))
                return f

            def qk_part(name, t):
                def f():
                    xb = xbs[name]
                    ps = ps_w.tile([P, QC], F32, tag="ps_w", name="ps_w")
                    if qk8:
                        for i in range(ndm // 2):
                            nc.tensor.matmul(
                                ps[:],
                                wc8[name][i][:, :, t * P:(t + 1) * P],
                                xb[:, 2 * i:2 * i + 2, :],
                                perf_mode=DR0,
                                start=(i == 0), stop=(i == ndm // 2 - 1))
                    else:
                        for u in range(ndm):
                            nc.tensor.matmul(
                                ps[:], wc[name][u][:, t * P:(t + 1) * P],
                                xb[:, u, :], start=(u == 0),
                                stop=(u == ndm - 1))
                    if name == "k":
                        nc.vector.tensor_scalar_add(
                            kT_sb[t][:, iqc * QC:(iqc + 1) * QC], ps[:],
                            bc["k"][:, t:t + 1])
                    else:
                        nc.vector.tensor_scalar_add(
                            qf[t][:, 0, :], ps[:], bc["q"][:, t:t + 1])
                return f

            def v_part(j):
                def f():
                    xb = xbs["v"]
                    ikb = iqc * (QC // P) + j
                    ps = ps_w.tile([P, dh], F32, tag="ps_w", name="ps_w")
                    for u in range(ndm):
                        nc.tensor.matmul(
                            ps[:], xb[:, u, j * P:(j + 1) * P], wc["v"][u][:],
                            start=(u == 0), stop=False)
                    nc.tensor.matmul(ps[:], ones1[:], bcv_row[:],
                                     start=False, stop=True)
                    nc.vector.tensor_copy(
                        v_all[:, ikb, :, 0:DK],
                        ps[:].rearrange("p (h d) -> p h d", h=hloc))
                return f

            clos.append(load("q", qT))
            for t in range(ndh):
                clos.append(qk_part("q", t))
            clos.append(load("k", kT))
            for t in range(ndh):
                clos.append(qk_part("k", t))
            clos.append(load("v", vT))
            for j in range(QC // P):
                clos.append(v_part(j))
            return clos

        # ---- attention ----
        inv_sqrt = 1.0 / math.sqrt(DK)
        DR = mybir.MatmulPerfMode.DoubleRow
        cT_tiles = {}

        def attention_chunk(iqc, fillers=(), mid=None):
            """QK+exp of step i overlaps PV of step i-1; `fillers` (next
            chunk's projection closures) are spread over the early steps;
            `mid` (the previous chunk's yin load) fires ~70% through."""
            qf = qf_tiles[iqc]
            cT = cpool.tile([P, ndh, QC], st, tag="cT", name=f"cT{iqc}")
            cT_tiles[iqc] = cT
            blist = blocks[iqc]
            steps = []
            for h in range(hloc):
                grps = [blist[g0:g0 + GW] for g0 in range(0, len(blist), GW)]
                for g in range(len(grps)):
                    steps.append((h, grps[g], g == 0, g == len(grps) - 1))
            po = {}
            pending = []
            fillers = list(fillers)
            n_steps = len(steps)
            fill_at = {}
            if fillers:
                # spread fillers uniformly across the steps
                for fi in range(len(fillers)):
                    at = (fi * n_steps) // len(fillers)
                    fill_at.setdefault(min(at, n_steps - 1), []).append(
                        fillers[fi])
            mid_at = (6 * n_steps) // 10

            def emit_qk_exp(h, grp):
                t, off = h // 2, (h % 2) * DK
                pss = ps_s.tile([P, GW * QC], F32, tag="ps_scores",
                                name="ps_scores")
                for j, (ikb, mode, arg) in enumerate(grp):
                    kv = kT_sb[t][off:off + DK, ikb * KB:(ikb + 1) * KB]
                    if qk8:
                        nc.tensor.matmul(
                            pss[:, j * QC:(j + 1) * QC],
                            kv.unsqueeze(1).broadcast_to([DK, 2, KB]),
                            qf[t][off:off + DK, :, :],
                            perf_mode=DR, start=True, stop=True)
                    else:
                        nc.tensor.matmul(pss[:, j * QC:(j + 1) * QC],
                                         kv, qf[t][off:off + DK, 0, :])
                pt = ppool.tile([P, GW * QC], pvt, tag="p", name="p")
                nw = len(grp) * QC
                nc.scalar.activation(pt[:, 0:nw], pss[:, 0:nw],
                                     mybir.ActivationFunctionType.Exp,
                                     scale=inv_sqrt)
                for j, (ikb, mode, arg) in enumerate(grp):
                    pj = pt[:, j * QC:(j + 1) * QC]
                    if mode == TRI:
                        r = arg
                        if r > 0:
                            nc.gpsimd.memset(pj[:, 0:r * P], 0.0)
                        nc.vector.tensor_mul(
                            pj[:, r * P:(r + 1) * P],
                            pj[:, r * P:(r + 1) * P], tri[:])
                    elif mode == GEN:
                        nc.vector.tensor_mul(pj[:], pj[:], gen_sb[:, arg, :])
                return pt

            def emit_pv(h, grp, pt, first, last):
                if first:
                    po[h] = ps_o.tile([VW, QC], F32, tag="ps_av",
                                      name="ps_av")
                ikbs = [ikb for (ikb, _, _) in grp]
                if pv8 and len(grp) == 2 and ikbs[1] == ikbs[0] + 1:
                    nc.tensor.matmul(
                        po[h][:], v_all[:, ikbs[0]:ikbs[0] + 2, h, :],
                        pt[:].rearrange("p (two q) -> p two q", two=2),
                        perf_mode=DR,
                        start=first, stop=last)
                else:
                    for j, (ikb, mode, arg) in enumerate(grp):
                        nc.tensor.matmul(
                            po[h][:, 0:QC], v_all[:, ikb, h, :],
                            pt[:, j * QC:(j + 1) * QC],
                            start=(first and j == 0),
                            stop=(last and j == len(grp) - 1))
                if last:
                    rec1 = work.tile([1, QC], F32, tag="rec1", name="rec1")
                    nc.vector.reciprocal(rec1[:], po[h][DK:DK + 1, :])
                    recb = work.tile([DK, QC], F32, tag="recb", name="recb")
                    nc.gpsimd.partition_broadcast(recb[:], rec1[:])
                    nc.vector.tensor_mul(
                        cT[(h % 2) * DK:(h % 2) * DK + DK, h // 2, :],
                        po[h][0:DK, :], recb[:])
                    del po[h]

            for i, (h, grp, first, last) in enumerate(steps):
                pt = emit_qk_exp(h, grp)
                pending.append((h, grp, pt, first, last))
                if len(pending) > 3:
                    emit_pv(*pending.pop(0))
                for f in fill_at.get(i, ()):
                    f()
                if mid is not None and i == mid_at:
                    mid()
                    mid = None
            for p_ in pending:
                emit_pv(*p_)
            if mid is not None:
                mid()

        # ---- a2a + output projection ----
        def a2a_chunk(c):
            # cT [128, ndh, QC] -> a2a_in[c or :, c] [8, 256, 64]
            cT = cT_tiles[c]
            dst = a2a_in[c] if chunked_cc else a2a_in[:, c]
            for fh in range(ndh):
                (nc.sync if fh % 2 == 0 else nc.scalar).dma_start(
                    out=dst[:, fh * P:(fh + 1) * P, :]
                        .rearrange("r p j -> p r j"),
                    in_=cT[:, fh, :].rearrange("p (r j) -> p r j", r=nsub))
            if not chunked_cc:
                if c == nqc - 1:
                    if collective:
                        for _ in range(cc_reps):
                            nc.gpsimd.collective_compute(
                                "AllToAll", mybir.AluOpType.bypass,
                                replica_groups=[list(range(N_CORES))],
                                ins=[a2a_in[:].opt()], outs=[a2a_out[:].opt()])
                    else:
                        nc.sync.dma_start(out=a2a_out[:], in_=a2a_in[:])
                return
            if collective:
                for _ in range(cc_reps):
                    nc.gpsimd.collective_compute(
                        "AllToAll", mybir.AluOpType.bypass,
                        replica_groups=[list(range(N_CORES))],
                        ins=[a2a_in[c].opt()], outs=[a2a_out[c].opt()])
            else:
                nc.sync.dma_start(out=a2a_out[c], in_=a2a_in[c])

        yin_tiles = {}

        def yin_load(c):
            yin = ypool.tile([P, nde, P], st, tag="yin", name=f"yin{c}")
            yin_tiles[c] = yin
            half = nsub // 2
            if chunked_cc:
                src = a2a_out[c]
                nc.sync.dma_start(
                    out=yin[:, :, 0:64],
                    in_=src[0:half].rearrange("s (fh p) j -> p (s fh) j", p=P))
                nc.scalar.dma_start(
                    out=yin[:, :, 64:128],
                    in_=src[half:nsub].rearrange("s (fh p) j -> p (s fh) j",
                                                 p=P))
            else:
                yv = yin[:].rearrange("p (s fh) j -> p s fh j", fh=ndh)
                for b0, sl in ((0, slice(0, 64)), (half, slice(64, 128))):
                    for fh in range(ndh):
                        nc.sync.dma_start(
                            out=yv[:, :, fh, sl],
                            in_=a2a_out[b0:b0 + half, c, fh * P:(fh + 1) * P, :]
                                .rearrange("s p j -> p s j"))

        def yproj_mm(c, split_store=False):
            yin = yin_tiles[c]
            ys = ypool.tile([P, dm], F32, tag="ys", name=f"ys{c}")
            for mb in range(dm // QC):
                ps = ps_w.tile([P, QC], F32, tag="ps_w", name="ps_w")
                for u in range(nde):
                    nc.tensor.matmul(
                        ps[:], yin[:, u, :],
                        wo_sb[:, u, mb * QC:(mb + 1) * QC],
                        start=(u == 0), stop=(u == nde - 1))
                nc.vector.tensor_add(ys[:, mb * QC:(mb + 1) * QC], ps[:],
                                     bo_bcast[:, mb * QC:(mb + 1) * QC])
                if split_store:
                    # let the first half's store overlap the second half
                    (nc.sync if mb % 2 == 0 else nc.scalar).dma_start(
                        out=y_out[c * P:(c + 1) * P, mb * QC:(mb + 1) * QC],
                        in_=ys[:, mb * QC:(mb + 1) * QC])
            if not split_store:
                nc.sync.dma_start(out=y_out[c * P:(c + 1) * P, :], in_=ys[:])

        # ---- schedule ----
        # head: score-path (fp8) combine + q/k projections first so the
        # first attention exp fires ~7us in; the bf16 v-combine and chunk-0
        # V projection become PE filler inside attention chunk 0.
        # yproj(c) is deferred into later chunks' attention as PE filler:
        # the final chunks have the most exp work and no projections left.
        if causal_dep:
            pc0 = proj_closures(0)
            pc0[0]()                       # load q chunk 0
            pc0[3]()                       # load k chunk 0
            if qk8:
                load_w_small("q")()
                load_w_small("k")()
                combine_qk8("q")
                bias_qk("q")
                pc0[1](); pc0[2]()         # project q chunk 0
                combine_qk8("k")
                bias_qk("k")
                pc0[4](); pc0[5]()         # project k chunk 0
            else:
                load_w_small("q")()
                load_w_small("k")()
                load_w_inT_bf16()
                for f in combine_bf16_closures("q"):
                    f()
                bias_qk("q")
                pc0[1](); pc0[2]()
                for f in combine_bf16_closures("k"):
                    f()
                bias_qk("k")
                pc0[4](); pc0[5]()
            pc0[6]()                       # load v chunk 0
            load_w_small("v")()
            if qk8:
                load_w_inT_bf16()
            head_fill = combine_bf16_closures("v") + [bias_v]
            load_wo()
            for c in range(nqc):
                fillers = list(head_fill)
                head_fill = []
                if c == 0:
                    fillers += pc0[7:]     # chunk-0 V projection
                if c + 1 < nqc:
                    fillers += list(proj_closures(c + 1))
                if chunked_cc and c == nqc - 1:
                    for cc in range(nqc - 2):
                        fillers.append((lambda c2: lambda: yproj_mm(c2))(cc))
                mid = ((lambda cc: lambda: yin_load(cc))(c - 1)
                       if c > 0 and chunked_cc else None)
                attention_chunk(c, fillers=fillers, mid=mid)
                a2a_chunk(c)
            if chunked_cc:
                yproj_mm(nqc - 2)
                pe_keepwarm(TAIL_WARM)
            else:
                for c in range(nqc - 1):
                    yin_load(c)
                    yproj_mm(c)
        else:
            # general masks: all projections first, then attention
            load_w_small("q")()
            load_w_small("k")()
            load_w_small("v")()
            load_w_inT_bf16()
            if qk8:
                combine_qk8("q")
                combine_qk8("k")
            else:
                for f in combine_bf16_closures("q") + combine_bf16_closures("k"):
                    f()
            bias_qk("q")
            bias_qk("k")
            for f in combine_bf16_closures("v"):
                f()
            bias_v()
            for c in range(nqc):
                for f in proj_closures(c):
                    f()
            load_wo()
            for c in range(nqc):
                mid = (lambda cc: lambda: yin_load(cc))(c - 1) if c > 0 else None
                attention_chunk(c, mid=mid)
                a2a_chunk(c)
                if c > 0:
                    yproj_mm(c - 1)
        yin_load(nqc - 1)
        yproj_mm(nqc - 1, split_store=True)

    nc.compile()
    return nc


# ------------------------------------------------------------------
_CACHE = {}


def _get_compiled(plan_key, blocks, n_gen, mm):
    if plan_key not in _CACHE:
        nc = build_mha(blocks, n_gen, mm=mm)
        nc.m = get_hw_module(nc.m)
        _CACHE[plan_key] = nc
    return _CACHE[plan_key]


def make_in_maps(q, k, v, mask, W_in, b_in, Wq, bq, Wk, bk, Wv, bv, Wo, bo,
                 blocks=None, n_gen=None, gen_tiles=None):
    if blocks is None:
        blocks, n_gen, gen_tiles = make_plan(mask)
    bf16 = mybir.dt.np(BF16)
    fp8 = mybir.dt.np(FP8)
    dh = DE // 4
    tb = lambda a: np.ascontiguousarray(np.asarray(a).T).astype(bf16)
    cb = lambda a: np.ascontiguousarray(np.asarray(a)).astype(bf16)
    t8 = lambda a: np.ascontiguousarray(np.asarray(a).T).astype(fp8)
    c8 = lambda a: np.ascontiguousarray(np.asarray(a)).astype(fp8)
    in_maps = []
    for c in range(N_CORES):
        b, g = c // 4, c % 4
        sl = slice(g * dh, (g + 1) * dh)
        mt = (gen_tiles[b] if n_gen else
              np.zeros((1, KB, QC), np.int32))
        qk8 = MM_MODE in ("fp8", "fp8qk")
        tq = t8 if qk8 else tb
        in_maps.append({
            "qT": tq(q[b]), "kT": tq(k[b]), "vT": tb(v[b]),
            "w_inT": tb(W_in), "w_inT8": t8(W_in),
            "wq": cb(Wq[:, sl]),
            "wk": cb(Wk[:, sl]),
            "wq8": c8(Wq[:, sl]),
            "wk8": c8(Wk[:, sl]),
            "wv": cb(Wv[:, sl]),
            "wo": cb(Wo),
            "b_in": np.asarray(b_in).astype(bf16),
            "bq": np.ascontiguousarray(np.asarray(bq)[sl]),
            "bk": np.ascontiguousarray(np.asarray(bk)[sl]),
            "bv": np.ascontiguousarray(np.asarray(bv)[sl]),
            "bo": np.asarray(bo),
            "m_tiles": mt,
        })
    return in_maps, blocks, n_gen


def assemble(results):
    out = np.empty((B, S, DM), np.float32)
    for core in range(N_CORES):
        y = results[core]["y_out"]            # [nqc*128, DM]
        for c in range(S // QC):
            for b in range(B):
                out[b, c * QC + core * 64:c * QC + (core + 1) * 64, :] = \
                    y[c * P + b * 64:c * P + (b + 1) * 64, :]
    return out


MM_MODE = "fp8qk"


def kernel(**inputs):
    mask = inputs["mask"]
    blocks, n_gen, gen_tiles = make_plan(np.asarray(mask))
    plan_key = (str(blocks), n_gen, MM_MODE)
    nc = _get_compiled(plan_key, blocks, n_gen, MM_MODE)
    in_maps, _, _ = make_in_maps(
        inputs["q"], inputs["k"], inputs["v"], mask,
        inputs["W_in"], inputs["b_in"], inputs["Wq"], inputs["bq"],
        inputs["Wk"], inputs["bk"], inputs["Wv"], inputs["bv"],
        inputs["Wo"], inputs["bo"],
        blocks=blocks, n_gen=n_gen, gen_tiles=gen_tiles)
    res = bass_utils.run_bass_kernel_spmd(nc, in_maps,
                                          core_ids=list(range(N_CORES)))
    return assemble(res.results)


# revision 8
# speedup vs baseline: 1.0534x; 1.0143x over previous
"""Trainium2 Bass kernel for nn_MultiHeadAttention_72069551227273 (v2).

Reference computation (B=2, S=2048, D_MODEL=D_EMB=1024, H=16, d_k=64):
    q_p = q @ W_in + b_in                    (shared input projection)
    qh  = heads(q_p @ Wq + bq)               (per-head projections)
    s   = qh @ kh^T / sqrt(d_k), causal-masked softmax
    out = (attn @ vh, concat heads) @ Wo + bo

Sharding: 8 cores = 2 (batch) x 4 (head groups of 4 heads / 256 emb cols).
Per core the input and head projections are fused on device:
    Q = q @ (W_in @ Wq_slice) + (b_in @ Wq_slice + bq_slice)
The whole score path (W_in@Wq/Wk weight combine, Q/K projections, QK^T)
runs in fp8e4m3 DoubleRow perf mode with contraction chunks paired into
the two DoubleRow slots: 4x MACs/cycle over bf16 for combine+projections
and 2x for scores (score lhsT slots = (K, K) via a stride-0 broadcast,
rhs slots = (Q, 0)).  fp8 score noise washes out in the softmax; the
value path (V, attn@V, Wo) stays bf16 — fp8 there measured 2.4e-2
relative error, over the 2e-2 gate.  V is projected straight into the
natural [seq, head, d_k] layout (no PE transposes) with its bias folded
in via a rank-1 ones matmul.  Softmax is exp(s/8) without max-subtraction;
the denominator comes free from a ones column appended to V.  Fully-masked
score blocks are skipped at trace time, diagonal blocks get an on-chip
triangular mask.  Projection work for chunk c+1 is interleaved into the
attention steps of chunk c (attention is Act/exp-heavy, projections are
PE-heavy), and attention itself is software-pipelined so PV of step i-1
overlaps the exp of step i.

Output stage is sequence-parallel instead of tensor-parallel: after
attention chunk c, an 8-way AllToAll (bf16, 256 KB) redistributes the
attention outputs so every core holds all 1024 features for 64 q rows of
each batch, then applies the full Wo locally — there is no reduction
collective at all.  The per-chunk collectives overlap with the next
chunk's attention; only the last chunk's exchange is exposed.
"""

import sys

sys.path.append("/opt/trn_rl_repo")

import math
from contextlib import ExitStack

import numpy as np

import concourse.bass as bass
import concourse.bacc as bacc
import concourse.mybir as mybir
import concourse.tile as tile
from concourse import bass_utils
from concourse.bass_interp import get_hw_module

# problem dims
B, S, DM, DE, H, DK = 2, 2048, 1024, 1024, 16, 64
N_CORES = 8
P = 128                      # partitions
QC = 512                     # q chunk (psum bank width in fp32)
KB = 128                     # k block (scores^T partition block)
GW = 2                       # kb blocks per score-psum tile (2 banks)
TAIL_WARM = 58               # PE keep-warm matmuls bridging the last a2a wait

F32 = mybir.dt.float32
BF16 = mybir.dt.bfloat16
FP8 = mybir.dt.float8e4

FULL, TRI, GEN, SKIP = 0, 1, 2, 3


def make_plan(mask_np, s=S, qc=QC, kb=KB):
    """Classify scores^T blocks [kb x qc] from the (B, S, S) 0/1 mask.

    Returns (blocks, n_gen_tiles, gen_tiles_per_batch):
      blocks[iqc] = list of (ikb, mode, arg)
    """
    nqc, nkb = s // qc, s // kb
    m = np.asarray(mask_np) != 0          # [B, S(q), S(k)] True = attend
    tril = np.tril(np.ones((s, s), bool))
    causal = all(np.array_equal(m[b], tril) for b in range(m.shape[0]))
    blocks = []
    if causal:
        for iqc in range(nqc):
            row = []
            for ikb in range(nkb):
                if (ikb + 1) * kb <= iqc * qc:
                    row.append((ikb, FULL, 0))
                elif ikb * kb < (iqc + 1) * qc:
                    row.append((ikb, TRI, (ikb * kb - iqc * qc) // kb))
                # else fully masked -> skip
            blocks.append(row)
        return blocks, 0, None

    # general path: per-block classification, unioned across batches
    nb = m.shape[0]
    # every query row must attend to >= 1 key (else softmax semantics differ)
    assert m.any(axis=-1).all(), "fully-masked query rows unsupported"
    gen_tiles = [[] for _ in range(nb)]
    for iqc in range(nqc):
        row = []
        for ikb in range(nkb):
            sub = m[:, iqc * qc:(iqc + 1) * qc, ikb * kb:(ikb + 1) * kb]
            if sub.all():
                row.append((ikb, FULL, 0))
            elif not sub.any():
                continue
            else:
                idx = len(gen_tiles[0])
                for b in range(nb):
                    gen_tiles[b].append(sub[b].T.astype(np.int32))  # [kb, qc]
                row.append((ikb, GEN, idx))
        blocks.append(row)
    n_gen = len(gen_tiles[0])
    gt = [np.stack(g) if n_gen else np.zeros((1, kb, qc), np.int32)
          for g in gen_tiles]
    return blocks, n_gen, gt


def build_mha(blocks, n_gen, *, s=S, dm=DM, de=DE, dh=None, mm="fp8",
              collective=True, chunked_cc=True, cc_reps=1):
    """Trace the per-core MHA program.  dh = per-core emb slice (256)."""
    if dh is None:
        dh = DE // 4
    nqc, nkb, ndm, nde = s // QC, s // KB, dm // P, de // P
    ndh = dh // P            # feature chunks per core (2)
    hloc = dh // DK          # heads per core (4)
    nsub = QC // 64          # a2a sub-blocks per chunk (8)
    out_rows = nqc * P       # output rows per core (4 chunks x 2 x 64)

    qk8 = mm in ("fp8", "fp8qk")
    pv8 = (mm == "fp8")
    qkt = FP8 if qk8 else BF16
    pvt = FP8 if pv8 else BF16   # dtype of probs, V, and mask tiles
    st = BF16

    # can attention chunk c start right after projection chunk c?
    causal_dep = all(
        max([c] + [ikb * KB // QC for (ikb, _, _) in blocks[c]]) <= c
        for c in range(nqc))

    nc = bacc.Bacc("TRN2", target_bir_lowering=False, debug=False,
                   num_devices=N_CORES)

    # ---- kernel I/O (per core) ----
    qT = nc.dram_tensor("qT", [dm, s], qkt, kind="ExternalInput")
    kT = nc.dram_tensor("kT", [dm, s], qkt, kind="ExternalInput")
    vT = nc.dram_tensor("vT", [dm, s], BF16, kind="ExternalInput")
    w_inT = nc.dram_tensor("w_inT", [de, dm], BF16, kind="ExternalInput")
    wq = nc.dram_tensor("wq", [de, dh], BF16, kind="ExternalInput")
    wk = nc.dram_tensor("wk", [de, dh], BF16, kind="ExternalInput")
    w8pack = nc.dram_tensor("w8pack", [de, 2 * dh + dm], qkt,
                            kind="ExternalInput")
    wv = nc.dram_tensor("wv", [de, dh], BF16, kind="ExternalInput")
    wo = nc.dram_tensor("wo", [de, dm], BF16, kind="ExternalInput")
    b_in = nc.dram_tensor("b_in", [de], BF16, kind="ExternalInput")
    bq = nc.dram_tensor("bq", [dh], F32, kind="ExternalInput")
    bk = nc.dram_tensor("bk", [dh], F32, kind="ExternalInput")
    bv = nc.dram_tensor("bv", [dh], F32, kind="ExternalInput")
    bo = nc.dram_tensor("bo", [dm], F32, kind="ExternalInput")
    m_tiles = nc.dram_tensor("m_tiles", [max(n_gen, 1), KB, QC], mybir.dt.int32,
                             kind="ExternalInput")
    y_out = nc.dram_tensor("y_out", [out_rows, dm], F32, kind="ExternalOutput")

    # a2a staging: chunked mode [chunk][8 dest blocks][256 feats][64 q];
    # single mode [8 dest blocks][chunk][256 feats][64 q] (one collective)
    if chunked_cc:
        a2a_in = nc.dram_tensor("a2a_in", [nqc, nsub, dh, 64], BF16)
        a2a_out = nc.dram_tensor("a2a_out", [nqc, nsub, dh, 64], BF16)
    else:
        a2a_in = nc.dram_tensor("a2a_in", [nsub, nqc, dh, 64], BF16)
        a2a_out = nc.dram_tensor("a2a_out", [nsub, nqc, dh, 64], BF16)

    with tile.TileContext(nc) as tc, ExitStack() as ex:
        persist = ex.enter_context(tc.tile_pool(name="persist", bufs=1))
        work = ex.enter_context(tc.tile_pool(name="work", bufs=4))
        ps_w = ex.enter_context(tc.tile_pool(name="ps_w", bufs=2, space="PSUM"))
        ps_s = ex.enter_context(tc.tile_pool(name="ps_s", bufs=2, space="PSUM"))
        ps_o = ex.enter_context(tc.tile_pool(name="ps_o", bufs=2, space="PSUM"))
        qbufs = 2 if causal_dep else nqc
        xpool = ex.enter_context(tc.tile_pool(name="xpool", bufs=4))
        qpool = ex.enter_context(tc.tile_pool(name="qpool", bufs=qbufs))
        ppool = ex.enter_context(tc.tile_pool(name="ppool", bufs=8))
        cpool = ex.enter_context(tc.tile_pool(name="cpool", bufs=2))
        ypool = ex.enter_context(tc.tile_pool(name="ypool", bufs=2))
        wpool = ex.enter_context(tc.tile_pool(name="wpool", bufs=1))

        # ---- constants ----
        # tri[k, q] = 1.0 where k <= q (keep), else 0
        tri = persist.tile([P, P], pvt, tag="tri", name="tri")
        tri_b = persist.tile([P, P], st, tag="tri_b", name="tri_b")
        nc.gpsimd.memset(tri_b[:], 0.0)
        nc.gpsimd.affine_select(out=tri_b[:], in_=tri_b[:],
                                compare_op=mybir.AluOpType.is_gt,
                                fill=1.0, base=0,
                                pattern=[[-1, P]], channel_multiplier=1)
        if pvt == st:
            tri = tri_b
        else:
            nc.vector.tensor_copy(tri[:], tri_b[:])
        ones1 = persist.tile([1, P], st, tag="ones1", name="ones1")
        nc.gpsimd.memset(ones1[:], 1.0)
        # preload the Exp table while DMAs stream in
        actwarm = persist.tile([1, 1], F32, tag="actwarm", name="actwarm")
        nc.scalar.activation(actwarm[:], ones1[0:1, 0:1],
                             mybir.ActivationFunctionType.Exp)
        scr = persist.tile([1, QC], st, tag="scr", name="scr")
        nc.vector.memset(scr[:], 1.0)
        HEAD_WARM = 11

        def pe_keepwarm(n):
            # dummy matmuls bridge a PE idle window so the clock does not
            # drop out of max p-state before the next real matmul burst
            pwu = ps_o.tile([1, QC], F32, tag="ps_av", name="pwu")
            for i in range(n):
                nc.tensor.matmul(pwu[:], scr[0:1, 0:1], scr[:],
                                 start=(i == 0), stop=(i == n - 1))

        gen_sb = None
        if n_gen:
            gen_sb = persist.tile([P, n_gen, QC], pvt, tag="gen", name="gen")
            gi = persist.tile([P, n_gen, QC], mybir.dt.int32, tag="gen_i",
                              name="gen_i")
            nc.sync.dma_start(gi[:], m_tiles[:].rearrange("n p q -> p n q"))
            for i in range(n_gen):
                if pvt == st:
                    nc.vector.tensor_copy(gen_sb[:, i, :], gi[:, i, :])
                else:
                    gb = work.tile([P, QC], st, tag="gen_b", name="gen_b")
                    nc.vector.tensor_copy(gb[:], gi[:, i, :])
                    nc.vector.tensor_copy(gen_sb[:, i, :], gb[:])

        DR0 = mybir.MatmulPerfMode.DoubleRow

        # ---- persistent activation storage (memsets run at t=0) ----
        kT_sb = [persist.tile([P, s], qkt, tag=f"kT{t}", name=f"kT{t}")
                 for t in range(ndh)]
        # V in natural layout, heads side by side, with a ones column:
        # v_all[kb_row, ikb, h, 0:DK] = v_h[key, :], v_all[.., DK] = 1
        # fp8 DoubleRow ldweights needs 4-byte-aligned slot strides: pad
        # each head's [d_k | ones] slot to VW columns (tail zeroed)
        VW = DK + 4 if pv8 else DK + 1
        v_all = persist.tile([P, nkb, hloc, VW], pvt, tag="v_all",
                             name="v_all")
        nc.gpsimd.memset(v_all[:, :, :, DK], 1.0)
        if VW > DK + 1:
            nc.gpsimd.memset(v_all[:, :, :, DK + 1:VW], 0.0)

        qf_tiles = {}

        def make_qf(iqc, memset=False):
            qf = [qpool.tile([P, 2, QC], qkt, tag=f"qf{t}", name=f"qf{t}_{iqc}")
                  for t in range(ndh)]
            qf_tiles[iqc] = qf
            if memset and qk8:
                # pool ring: zero slots persist across later buffer reuse
                for t in range(ndh):
                    nc.vector.memset(qf[t][:, 1, :], 0.0)
            return qf

        for c in range(qbufs):
            make_qf(c, memset=True)

        # ---- load weights (bf16 from host; batched DMAs) ----
        # spread DMAs across both HWDGE queues (SP + Activation)
        _dmaq = [0]

        def dmaq():
            _dmaq[0] ^= 1
            return nc.sync if _dmaq[0] else nc.scalar

        # wq first, then w_inT in quarters: the first combine matmuls only
        # need w_in chunk u=0 + wq, so PE can start ~2.7us in
        w_sb = {}
        w8_sb = {}
        if qk8:
            # one packed DMA delivers all score-path fp8 weights
            # ([Wq8 | Wk8 | W_inT8] along the column axis)
            wide = 2 * dh + dm
            w8all = wpool.tile([P, nde, wide], qkt, tag="w8all", name="w8all")
            hw8 = nde // 4
            for i in range(4):
                dmaq().dma_start(
                    out=w8all[:, i * hw8:(i + 1) * hw8, :],
                    in_=w8pack[i * hw8 * P:(i + 1) * hw8 * P, :]
                        .rearrange("(u p) d -> p u d", p=P))
            w8_sb["q"] = w8all[:, :, 0:dh]
            w8_sb["k"] = w8all[:, :, dh:2 * dh]
            w_in8_b = w8all[:, :, 2 * dh:wide]
        w_inT_b = wpool.tile([P, nde, dm], st, tag="w_inT", name="w_inT_b")
        w_inT_sb = [w_inT_b[:, u, :] for u in range(nde)]
        b_inT = wpool.tile([P, nde], st, tag="b_inT", name="b_inT")
        bo_bcast = persist.tile([P, dm], F32, tag="bo_b", name="bo_bcast")
        for name in ("q", "k", "v"):
            wb = wpool.tile([P, nde, dh], st, tag=f"w{name}", name=f"w{name}_b")
            w_sb[name] = [wb[:, u, :] for u in range(nde)]
            w_sb[name + "_t"] = wb

        def load_w_small(name):
            # bf16 head-projection weights (bias combine) + b_in
            def f():
                dmaq().dma_start(
                    out=w_sb[name + "_t"][:],
                    in_={"q": wq, "k": wk, "v": wv}[name]
                        .rearrange("(u p) d -> p u d", p=P))
                if name == "q":
                    nc.scalar.dma_start(
                        out=b_inT[:], in_=b_in[:].rearrange("(t p) -> p t", p=P))
            return f

        def load_w_inT_bf16():
            hd4 = nde // 4
            for i in range(4):
                dmaq().dma_start(
                    out=w_inT_b[:, i * hd4:(i + 1) * hd4, :],
                    in_=w_inT[i * hd4 * P:(i + 1) * hd4 * P, :]
                        .rearrange("(u p) m -> p u m", p=P))
            nc.scalar.dma_start(out=bo_bcast[:],
                                in_=bo[:].unsqueeze(0).broadcast_to([P, dm]))
        wo_sb = persist.tile([P, nde, dm], st, tag="wo", name="wo_b")

        def load_wo():
            # deferred: wo is not needed until the first output projection
            hdo = nde // 2
            nc.sync.dma_start(out=wo_sb[:, 0:hdo, :],
                              in_=wo[0:hdo * P, :].rearrange("(u p) m -> p u m", p=P))
            nc.scalar.dma_start(out=wo_sb[:, hdo:nde, :],
                                in_=wo[hdo * P:, :].rearrange("(u p) m -> p u m", p=P))

        # ---- combine weights: Wc_x = W_in @ Wx (+ bias fold) ----
        # q/k: fp8 DoubleRow over paired de-chunks -> paired-layout wc8
        # (wc8[name][t//2][:, t%2, :] = Wc rows of dm-chunk t); v: bf16
        wc = {}
        wc8 = {}
        bc = {}

        def combine_qk8(name):
            wc8[name] = [persist.tile([P, 2, dh], qkt, tag=f"wc8{name}{t}",
                                      name=f"wc8{name}{t}")
                         for t in range(ndm // 2)]
            for tp in range(ndm // 2):
                ps = ps_w.tile([P, 2 * dh], F32, tag="ps_w", name="ps_w")
                for half in range(2):
                    t = 2 * tp + half
                    for i in range(nde // 2):
                        nc.tensor.matmul(
                            ps[:, half * dh:(half + 1) * dh],
                            w_in8_b[:, 2 * i:2 * i + 2, t * P:(t + 1) * P],
                            w8_sb[name][:, 2 * i:2 * i + 2, :],
                            perf_mode=DR0,
                            start=(i == 0), stop=(i == nde // 2 - 1))
                nc.vector.tensor_copy(
                    wc8[name][tp][:],
                    ps[:].rearrange("p (two d) -> p two d", two=2))

        def combine_bf16_closures(name):
            wc[name] = [persist.tile([P, dh], st, tag=f"wc{name}{t}",
                                     name=f"wc{name}{t}") for t in range(ndm)]

            def piece(ts_):
                def f():
                    for t in ts_:
                        ps = ps_w.tile([P, dh], F32, tag="ps_w", name="ps_w")
                        for u in range(nde):
                            nc.tensor.matmul(
                                ps[:], w_inT_sb[u][:, t * P:(t + 1) * P],
                                w_sb[name][u][:],
                                start=(u == 0), stop=(u == nde - 1))
                        nc.vector.tensor_copy(wc[name][t][:], ps[:])
                return f
            return [piece(ts_) for ts_ in
                    ([0, 1], [2, 3], [4, 5], [6, 7])]

        def bias_qk(name):
            bvec = {"q": bq, "k": bk}[name]
            bxT = wpool.tile([P, ndh], F32, tag=f"bxT{name}", name=f"bxT{name}")
            nc.sync.dma_start(out=bxT[:], in_=bvec[:].rearrange("(t p) -> p t", p=P))
            bc[name] = persist.tile([P, ndh], F32, tag=f"bc{name}", name=f"bc{name}")
            for t in range(ndh):
                ps = ps_w.tile([P, 1], F32, tag="ps_w", name="ps_w")
                for u in range(nde):
                    nc.tensor.matmul(
                        ps[:], w_sb[name][u][:, t * P:(t + 1) * P],
                        b_inT[:, u:u + 1],
                        start=(u == 0), stop=(u == nde - 1))
                nc.vector.tensor_add(bc[name][:, t:t + 1], ps[:], bxT[:, t:t + 1])

        bcv_row = persist.tile([1, dh], st, tag="bcv", name="bcv_row")

        def bias_v():
            bv_row = wpool.tile([1, dh], F32, tag="bv_row", name="bv_row")
            nc.sync.dma_start(out=bv_row[:], in_=bv[:].unsqueeze(0))
            ps = ps_w.tile([1, dh], F32, tag="ps_w", name="ps_w")
            for u in range(nde):
                nc.tensor.matmul(ps[:], b_inT[:, u:u + 1], w_sb["v"][u][:],
                                 start=(u == 0), stop=(u == nde - 1))
            nc.vector.tensor_add(bcv_row[:], ps[:], bv_row[:])

        def proj_closures(iqc):
            """Per-chunk projection emission, split into PE-sized closures."""
            clos = []
            qf = qf_tiles.get(iqc) or make_qf(iqc, memset=iqc < qbufs)
            xbs = {}

            def load(name, xdram):
                def f():
                    dt_ = qkt if (qk8 and name in ("q", "k")) else st
                    tag = "xb8" if (qk8 and name in ("q", "k")) else "xb"
                    xb = xpool.tile([P, ndm, QC], dt_, tag=tag,
                                    name=f"xb_{name}{iqc}")
                    xbs[name] = xb
                    dmaq().dma_start(
                        out=xb[:],
                        in_=xdram[:, iqc * QC:(iqc + 1) * QC]
                            .rearrange("(u p) s -> p u s", p=P))
                return f

            def qk_part(name, t):
                def f():
                    xb = xbs[name]
                    ps = ps_w.tile([P, QC], F32, tag="ps_w", name="ps_w")
                    if qk8:
                        for i in range(ndm // 2):
                            nc.tensor.matmul(
                                ps[:],
                                wc8[name][i][:, :, t * P:(t + 1) * P],
                                xb[:, 2 * i:2 * i + 2, :],
                                perf_mode=DR0,
                                start=(i == 0), stop=(i == ndm // 2 - 1))
                    else:
                        for u in range(ndm):
                            nc.tensor.matmul(
                                ps[:], wc[name][u][:, t * P:(t + 1) * P],
                                xb[:, u, :], start=(u == 0),
                                stop=(u == ndm - 1))
                    if name == "k":
                        nc.vector.tensor_scalar_add(
                            kT_sb[t][:, iqc * QC:(iqc + 1) * QC], ps[:],
                            bc["k"][:, t:t + 1])
                    else:
                        nc.vector.tensor_scalar_add(
                            qf[t][:, 0, :], ps[:], bc["q"][:, t:t + 1])
                return f

            def v_part(j):
                def f():
                    xb = xbs["v"]
                    ikb = iqc * (QC // P) + j
                    ps = ps_w.tile([P, dh], F32, tag="ps_w", name="ps_w")
                    for u in range(ndm):
                        nc.tensor.matmul(
                            ps[:], xb[:, u, j * P:(j + 1) * P], wc["v"][u][:],
                            start=(u == 0), stop=False)
                    nc.tensor.matmul(ps[:], ones1[:], bcv_row[:],
                                     start=False, stop=True)
                    nc.vector.tensor_copy(
                        v_all[:, ikb, :, 0:DK],
                        ps[:].rearrange("p (h d) -> p h d", h=hloc))
                return f

            clos.append(load("q", qT))
            for t in range(ndh):
                clos.append(qk_part("q", t))
            clos.append(load("k", kT))
            for t in range(ndh):
                clos.append(qk_part("k", t))
            clos.append(load("v", vT))
            for j in range(QC // P):
                clos.append(v_part(j))
            return clos

        # ---- attention ----
        inv_sqrt = 1.0 / math.sqrt(DK)
        DR = mybir.MatmulPerfMode.DoubleRow
        cT_tiles = {}

        def attention_chunk(iqc, fillers=(), mid=None):
            """QK+exp of step i overlaps PV of step i-1; `fillers` (next
            chunk's projection closures) are spread over the early steps;
            `mid` (the previous chunk's yin load) fires ~70% through."""
            qf = qf_tiles[iqc]
            cT = cpool.tile([P, ndh, QC], st, tag="cT", name=f"cT{iqc}")
            cT_tiles[iqc] = cT
            blist = blocks[iqc]
            steps = []
            for h in range(hloc):
                grps = [blist[g0:g0 + GW] for g0 in range(0, len(blist), GW)]
                for g in range(len(grps)):
                    steps.append((h, grps[g], g == 0, g == len(grps) - 1))
            po = {}
            pending = []
            fillers = list(fillers)
            n_steps = len(steps)
            fill_at = {}
            if fillers:
                # spread fillers uniformly across the steps
                for fi in range(len(fillers)):
                    at = (fi * n_steps) // len(fillers)
                    fill_at.setdefault(min(at, n_steps - 1), []).append(
                        fillers[fi])
            mid_at = (6 * n_steps) // 10

            def emit_qk_exp(h, grp):
                t, off = h // 2, (h % 2) * DK
                pss = ps_s.tile([P, GW * QC], F32, tag="ps_scores",
                                name="ps_scores")
                for j, (ikb, mode, arg) in enumerate(grp):
                    kv = kT_sb[t][off:off + DK, ikb * KB:(ikb + 1) * KB]
                    if qk8:
                        nc.tensor.matmul(
                            pss[:, j * QC:(j + 1) * QC],
                            kv.unsqueeze(1).broadcast_to([DK, 2, KB]),
                            qf[t][off:off + DK, :, :],
                            perf_mode=DR, start=True, stop=True)
                    else:
                        nc.tensor.matmul(pss[:, j * QC:(j + 1) * QC],
                                         kv, qf[t][off:off + DK, 0, :])
                pt = ppool.tile([P, GW * QC], pvt, tag="p", name="p")
                nw = len(grp) * QC
                nc.scalar.activation(pt[:, 0:nw], pss[:, 0:nw],
                                     mybir.ActivationFunctionType.Exp,
                                     scale=inv_sqrt)
                for j, (ikb, mode, arg) in enumerate(grp):
                    pj = pt[:, j * QC:(j + 1) * QC]
                    if mode == TRI:
                        r = arg
                        if r > 0:
                            nc.gpsimd.memset(pj[:, 0:r * P], 0.0)
                        nc.vector.tensor_mul(
                            pj[:, r * P:(r + 1) * P],
                            pj[:, r * P:(r + 1) * P], tri[:])
                    elif mode == GEN:
                        nc.vector.tensor_mul(pj[:], pj[:], gen_sb[:, arg, :])
                return pt

            def emit_pv(h, grp, pt, first, last):
                if first:
                    po[h] = ps_o.tile([VW, QC], F32, tag="ps_av",
                                      name="ps_av")
                ikbs = [ikb for (ikb, _, _) in grp]
                if pv8 and len(grp) == 2 and ikbs[1] == ikbs[0] + 1:
                    nc.tensor.matmul(
                        po[h][:], v_all[:, ikbs[0]:ikbs[0] + 2, h, :],
                        pt[:].rearrange("p (two q) -> p two q", two=2),
                        perf_mode=DR,
                        start=first, stop=last)
                else:
                    for j, (ikb, mode, arg) in enumerate(grp):
                        nc.tensor.matmul(
                            po[h][:, 0:QC], v_all[:, ikb, h, :],
                            pt[:, j * QC:(j + 1) * QC],
                            start=(first and j == 0),
                            stop=(last and j == len(grp) - 1))
                if last:
                    rec1 = work.tile([1, QC], F32, tag="rec1", name="rec1")
                    nc.vector.reciprocal(rec1[:], po[h][DK:DK + 1, :])
                    recb = work.tile([DK, QC], F32, tag="recb", name="recb")
                    nc.gpsimd.partition_broadcast(recb[:], rec1[:])
                    nc.vector.tensor_mul(
                        cT[(h % 2) * DK:(h % 2) * DK + DK, h // 2, :],
                        po[h][0:DK, :], recb[:])
                    del po[h]

            for i, (h, grp, first, last) in enumerate(steps):
                pt = emit_qk_exp(h, grp)
                pending.append((h, grp, pt, first, last))
                if len(pending) > 3:
                    emit_pv(*pending.pop(0))
                for f in fill_at.get(i, ()):
                    f()
                if mid is not None and i == mid_at:
                    mid()
                    mid = None
            for p_ in pending:
                emit_pv(*p_)
            if mid is not None:
                mid()

        # ---- a2a + output projection ----
        def a2a_chunk(c):
            # cT [128, ndh, QC] -> a2a_in[c or :, c] [8, 256, 64]
            cT = cT_tiles[c]
            dst = a2a_in[c] if chunked_cc else a2a_in[:, c]
            for fh in range(ndh):
                (nc.sync if fh % 2 == 0 else nc.scalar).dma_start(
                    out=dst[:, fh * P:(fh + 1) * P, :]
                        .rearrange("r p j -> p r j"),
                    in_=cT[:, fh, :].rearrange("p (r j) -> p r j", r=nsub))
            if not chunked_cc:
                if c == nqc - 1:
                    if collective:
                        for _ in range(cc_reps):
                            nc.gpsimd.collective_compute(
                                "AllToAll", mybir.AluOpType.bypass,
                                replica_groups=[list(range(N_CORES))],
                                ins=[a2a_in[:].opt()], outs=[a2a_out[:].opt()])
                    else:
                        nc.sync.dma_start(out=a2a_out[:], in_=a2a_in[:])
                return
            if collective:
                for _ in range(cc_reps):
                    nc.gpsimd.collective_compute(
                        "AllToAll", mybir.AluOpType.bypass,
                        replica_groups=[list(range(N_CORES))],
                        ins=[a2a_in[c].opt()], outs=[a2a_out[c].opt()])
            else:
                nc.sync.dma_start(out=a2a_out[c], in_=a2a_in[c])

        yin_tiles = {}

        def yin_load(c):
            yin = ypool.tile([P, nde, P], st, tag="yin", name=f"yin{c}")
            yin_tiles[c] = yin
            half = nsub // 2
            if chunked_cc:
                src = a2a_out[c]
                nc.sync.dma_start(
                    out=yin[:, :, 0:64],
                    in_=src[0:half].rearrange("s (fh p) j -> p (s fh) j", p=P))
                nc.scalar.dma_start(
                    out=yin[:, :, 64:128],
                    in_=src[half:nsub].rearrange("s (fh p) j -> p (s fh) j",
                                                 p=P))
            else:
                yv = yin[:].rearrange("p (s fh) j -> p s fh j", fh=ndh)
                for b0, sl in ((0, slice(0, 64)), (half, slice(64, 128))):
                    for fh in range(ndh):
                        nc.sync.dma_start(
                            out=yv[:, :, fh, sl],
                            in_=a2a_out[b0:b0 + half, c, fh * P:(fh + 1) * P, :]
                                .rearrange("s p j -> p s j"))

        def yproj_mm(c, split_store=False):
            yin = yin_tiles[c]
            ys = ypool.tile([P, dm], F32, tag="ys", name=f"ys{c}")
            for mb in range(dm // QC):
                ps = ps_w.tile([P, QC], F32, tag="ps_w", name="ps_w")
                for u in range(nde):
                    nc.tensor.matmul(
                        ps[:], yin[:, u, :],
                        wo_sb[:, u, mb * QC:(mb + 1) * QC],
                        start=(u == 0), stop=(u == nde - 1))
                nc.vector.tensor_add(ys[:, mb * QC:(mb + 1) * QC], ps[:],
                                     bo_bcast[:, mb * QC:(mb + 1) * QC])
                if split_store:
                    # let the first half's store overlap the second half
                    (nc.sync if mb % 2 == 0 else nc.scalar).dma_start(
                        out=y_out[c * P:(c + 1) * P, mb * QC:(mb + 1) * QC],
                        in_=ys[:, mb * QC:(mb + 1) * QC])
            if not split_store:
                nc.sync.dma_start(out=y_out[c * P:(c + 1) * P, :], in_=ys[:])

        # ---- schedule ----
        # head: score-path (fp8) combine + q/k projections first so the
        # first attention exp fires ~7us in; the bf16 v-combine and chunk-0
        # V projection become PE filler inside attention chunk 0.
        # yproj(c) is deferred into later chunks' attention as PE filler:
        # the final chunks have the most exp work and no projections left.
        if causal_dep:
            pc0 = proj_closures(0)
            pc0[0]()                       # load q chunk 0
            pc0[3]()                       # load k chunk 0
            if qk8:
                load_w_small("q")()
                load_w_small("k")()
                combine_qk8("q")
                bias_qk("q")
                pc0[1](); pc0[2]()         # project q chunk 0
                combine_qk8("k")
                bias_qk("k")
                pc0[4](); pc0[5]()         # project k chunk 0
            else:
                load_w_small("q")()
                load_w_small("k")()
                load_w_inT_bf16()
                for f in combine_bf16_closures("q"):
                    f()
                bias_qk("q")
                pc0[1](); pc0[2]()
                for f in combine_bf16_closures("k"):
                    f()
                bias_qk("k")
                pc0[4](); pc0[5]()
            pc0[6]()                       # load v chunk 0
            load_w_small("v")()
            if qk8:
                load_w_inT_bf16()
            head_fill = combine_bf16_closures("v") + [bias_v]
            load_wo()
            for c in range(nqc):
                fillers = list(head_fill)
                head_fill = []
                if c == 0:
                    fillers += pc0[7:]     # chunk-0 V projection
                if c + 1 < nqc:
                    fillers += list(proj_closures(c + 1))
                if chunked_cc and c == nqc - 1:
                    for cc in range(nqc - 2):
                        fillers.append((lambda c2: lambda: yproj_mm(c2))(cc))
                mid = ((lambda cc: lambda: yin_load(cc))(c - 1)
                       if c > 0 and chunked_cc else None)
                attention_chunk(c, fillers=fillers, mid=mid)
                a2a_chunk(c)
            if chunked_cc:
                yproj_mm(nqc - 2)
                pe_keepwarm(TAIL_WARM)
            else:
                for c in range(nqc - 1):
                    yin_load(c)
                    yproj_mm(c)
        else:
            # general masks: all projections first, then attention
            load_w_small("q")()
            load_w_small("k")()
            load_w_small("v")()
            load_w_inT_bf16()
            if qk8:
                combine_qk8("q")
                combine_qk8("k")
            else:
                for f in combine_bf16_closures("q") + combine_bf16_closures("k"):
                    f()
            bias_qk("q")
            bias_qk("k")
            for f in combine_bf16_closures("v"):
                f()
            bias_v()
            for c in range(nqc):
                for f in proj_closures(c):
                    f()
            load_wo()
            for c in range(nqc):
                mid = (lambda cc: lambda: yin_load(cc))(c - 1) if c > 0 else None
                attention_chunk(c, mid=mid)
                a2a_chunk(c)
                if c > 0:
                    yproj_mm(c - 1)
        yin_load(nqc - 1)
        yproj_mm(nqc - 1, split_store=True)

    nc.compile()
    return nc


# ------------------------------------------------------------------
_CACHE = {}


def _get_compiled(plan_key, blocks, n_gen, mm):
    if plan_key not in _CACHE:
        nc = build_mha(blocks, n_gen, mm=mm)
        nc.m = get_hw_module(nc.m)
        _CACHE[plan_key] = nc
    return _CACHE[plan_key]


def make_in_maps(q, k, v, mask, W_in, b_in, Wq, bq, Wk, bk, Wv, bv, Wo, bo,
                 blocks=None, n_gen=None, gen_tiles=None):
    if blocks is None:
        blocks, n_gen, gen_tiles = make_plan(mask)
    bf16 = mybir.dt.np(BF16)
    fp8 = mybir.dt.np(FP8)
    dh = DE // 4
    tb = lambda a: np.ascontiguousarray(np.asarray(a).T).astype(bf16)
    cb = lambda a: np.ascontiguousarray(np.asarray(a)).astype(bf16)
    t8 = lambda a: np.ascontiguousarray(np.asarray(a).T).astype(fp8)
    c8 = lambda a: np.ascontiguousarray(np.asarray(a)).astype(fp8)
    in_maps = []
    for c in range(N_CORES):
        b, g = c // 4, c % 4
        sl = slice(g * dh, (g + 1) * dh)
        mt = (gen_tiles[b] if n_gen else
              np.zeros((1, KB, QC), np.int32))
        qk8 = MM_MODE in ("fp8", "fp8qk")
        tq = t8 if qk8 else tb
        w8p = np.concatenate(
            [np.asarray(Wq)[:, sl], np.asarray(Wk)[:, sl],
             np.ascontiguousarray(np.asarray(W_in).T)], axis=1)
        in_maps.append({
            "qT": tq(q[b]), "kT": tq(k[b]), "vT": tb(v[b]),
            "w_inT": tb(W_in),
            "w8pack": c8(w8p),
            "wq": cb(Wq[:, sl]),
            "wk": cb(Wk[:, sl]),
            "wv": cb(Wv[:, sl]),
            "wo": cb(Wo),
            "b_in": np.asarray(b_in).astype(bf16),
            "bq": np.ascontiguousarray(np.asarray(bq)[sl]),
            "bk": np.ascontiguousarray(np.asarray(bk)[sl]),
            "bv": np.ascontiguousarray(np.asarray(bv)[sl]),
            "bo": np.asarray(bo),
            "m_tiles": mt,
        })
    return in_maps, blocks, n_gen


def assemble(results):
    out = np.empty((B, S, DM), np.float32)
    for core in range(N_CORES):
        y = results[core]["y_out"]            # [nqc*128, DM]
        for c in range(S // QC):
            for b in range(B):
                out[b, c * QC + core * 64:c * QC + (core + 1) * 64, :] = \
                    y[c * P + b * 64:c * P + (b + 1) * 64, :]
    return out


MM_MODE = "fp8qk"


def kernel(**inputs):
    mask = inputs["mask"]
    blocks, n_gen, gen_tiles = make_plan(np.asarray(mask))
    plan_key = (str(blocks), n_gen, MM_MODE)
    nc = _get_compiled(plan_key, blocks, n_gen, MM_MODE)
    in_maps, _, _ = make_in_maps(
        inputs["q"], inputs["k"], inputs["v"], mask,
        inputs["W_in"], inputs["b_in"], inputs["Wq"], inputs["bq"],
        inputs["Wk"], inputs["bk"], inputs["Wv"], inputs["bv"],
        inputs["Wo"], inputs["bo"],
        blocks=blocks, n_gen=n_gen, gen_tiles=gen_tiles)
    res = bass_utils.run_bass_kernel_spmd(nc, in_maps,
                                          core_ids=list(range(N_CORES)))
    return assemble(res.results)


# revision 9
# speedup vs baseline: 1.0580x; 1.0044x over previous
"""Trainium2 Bass kernel for nn_MultiHeadAttention_72069551227273 (v2).

Reference computation (B=2, S=2048, D_MODEL=D_EMB=1024, H=16, d_k=64):
    q_p = q @ W_in + b_in                    (shared input projection)
    qh  = heads(q_p @ Wq + bq)               (per-head projections)
    s   = qh @ kh^T / sqrt(d_k), causal-masked softmax
    out = (attn @ vh, concat heads) @ Wo + bo

Sharding: 8 cores = 2 (batch) x 4 (head groups of 4 heads / 256 emb cols).
Per core the input and head projections are fused on device:
    Q = q @ (W_in @ Wq_slice) + (b_in @ Wq_slice + bq_slice)
The whole score path (W_in@Wq/Wk weight combine, Q/K projections, QK^T)
runs in fp8e4m3 DoubleRow perf mode with contraction chunks paired into
the two DoubleRow slots: 4x MACs/cycle over bf16 for combine+projections
and 2x for scores (score lhsT slots = (K, K) via a stride-0 broadcast,
rhs slots = (Q, 0)).  fp8 score noise washes out in the softmax; the
value path (V, attn@V, Wo) stays bf16 — fp8 there measured 2.4e-2
relative error, over the 2e-2 gate.  V is projected straight into the
natural [seq, head, d_k] layout (no PE transposes) with its bias folded
in via a rank-1 ones matmul.  Softmax is exp(s/8) without max-subtraction;
the denominator comes free from a ones column appended to V.  Fully-masked
score blocks are skipped at trace time, diagonal blocks get an on-chip
triangular mask.  Projection work for chunk c+1 is interleaved into the
attention steps of chunk c (attention is Act/exp-heavy, projections are
PE-heavy), and attention itself is software-pipelined so PV of step i-1
overlaps the exp of step i.

Output stage is sequence-parallel instead of tensor-parallel: after
attention chunk c, an 8-way AllToAll (bf16, 256 KB) redistributes the
attention outputs so every core holds all 1024 features for 64 q rows of
each batch, then applies the full Wo locally — there is no reduction
collective at all.  The per-chunk collectives overlap with the next
chunk's attention; only the last chunk's exchange is exposed.
"""

import sys

sys.path.append("/opt/trn_rl_repo")

import math
from contextlib import ExitStack

import numpy as np

import concourse.bass as bass
import concourse.bacc as bacc
import concourse.mybir as mybir
import concourse.tile as tile
from concourse import bass_utils
from concourse.bass_interp import get_hw_module

# problem dims
B, S, DM, DE, H, DK = 2, 2048, 1024, 1024, 16, 64
N_CORES = 8
P = 128                      # partitions
QC = 512                     # q chunk (psum bank width in fp32)
KB = 128                     # k block (scores^T partition block)
GW = 2                       # kb blocks per score-psum tile (2 banks)
TAIL_WARM = 58               # PE keep-warm matmuls bridging the last a2a wait

F32 = mybir.dt.float32
BF16 = mybir.dt.bfloat16
FP8 = mybir.dt.float8e4

FULL, TRI, GEN, SKIP = 0, 1, 2, 3


def make_plan(mask_np, s=S, qc=QC, kb=KB):
    """Classify scores^T blocks [kb x qc] from the (B, S, S) 0/1 mask.

    Returns (blocks, n_gen_tiles, gen_tiles_per_batch):
      blocks[iqc] = list of (ikb, mode, arg)
    """
    nqc, nkb = s // qc, s // kb
    m = np.asarray(mask_np) != 0          # [B, S(q), S(k)] True = attend
    tril = np.tril(np.ones((s, s), bool))
    causal = all(np.array_equal(m[b], tril) for b in range(m.shape[0]))
    blocks = []
    if causal:
        for iqc in range(nqc):
            row = []
            for ikb in range(nkb):
                if (ikb + 1) * kb <= iqc * qc:
                    row.append((ikb, FULL, 0))
                elif ikb * kb < (iqc + 1) * qc:
                    row.append((ikb, TRI, (ikb * kb - iqc * qc) // kb))
                # else fully masked -> skip
            blocks.append(row)
        return blocks, 0, None

    # general path: per-block classification, unioned across batches
    nb = m.shape[0]
    # every query row must attend to >= 1 key (else softmax semantics differ)
    assert m.any(axis=-1).all(), "fully-masked query rows unsupported"
    gen_tiles = [[] for _ in range(nb)]
    for iqc in range(nqc):
        row = []
        for ikb in range(nkb):
            sub = m[:, iqc * qc:(iqc + 1) * qc, ikb * kb:(ikb + 1) * kb]
            if sub.all():
                row.append((ikb, FULL, 0))
            elif not sub.any():
                continue
            else:
                idx = len(gen_tiles[0])
                for b in range(nb):
                    gen_tiles[b].append(sub[b].T.astype(np.int32))  # [kb, qc]
                row.append((ikb, GEN, idx))
        blocks.append(row)
    n_gen = len(gen_tiles[0])
    gt = [np.stack(g) if n_gen else np.zeros((1, kb, qc), np.int32)
          for g in gen_tiles]
    return blocks, n_gen, gt


def build_mha(blocks, n_gen, *, s=S, dm=DM, de=DE, dh=None, mm="fp8",
              collective=True, chunked_cc=True, cc_reps=1):
    """Trace the per-core MHA program.  dh = per-core emb slice (256)."""
    if dh is None:
        dh = DE // 4
    nqc, nkb, ndm, nde = s // QC, s // KB, dm // P, de // P
    ndh = dh // P            # feature chunks per core (2)
    hloc = dh // DK          # heads per core (4)
    nsub = QC // 64          # a2a sub-blocks per chunk (8)
    out_rows = nqc * P       # output rows per core (4 chunks x 2 x 64)

    qk8 = mm in ("fp8", "fp8qk")
    pv8 = (mm == "fp8")
    qkt = FP8 if qk8 else BF16
    pvt = FP8 if pv8 else BF16   # dtype of probs, V, and mask tiles
    st = BF16

    # can attention chunk c start right after projection chunk c?
    causal_dep = all(
        max([c] + [ikb * KB // QC for (ikb, _, _) in blocks[c]]) <= c
        for c in range(nqc))

    nc = bacc.Bacc("TRN2", target_bir_lowering=False, debug=False,
                   num_devices=N_CORES)

    # ---- kernel I/O (per core) ----
    qT = nc.dram_tensor("qT", [dm, s], qkt, kind="ExternalInput")
    kT = nc.dram_tensor("kT", [dm, s], qkt, kind="ExternalInput")
    vT = nc.dram_tensor("vT", [dm, s], BF16, kind="ExternalInput")
    w_inT = nc.dram_tensor("w_inT", [de, dm], BF16, kind="ExternalInput")
    wq = nc.dram_tensor("wq", [de, dh], BF16, kind="ExternalInput")
    wk = nc.dram_tensor("wk", [de, dh], BF16, kind="ExternalInput")
    w8pack = nc.dram_tensor("w8pack", [de, 2 * dh + dm], qkt,
                            kind="ExternalInput")
    wv = nc.dram_tensor("wv", [de, dh], BF16, kind="ExternalInput")
    wo = nc.dram_tensor("wo", [de, dm], BF16, kind="ExternalInput")
    b_in = nc.dram_tensor("b_in", [de], BF16, kind="ExternalInput")
    bq = nc.dram_tensor("bq", [dh], F32, kind="ExternalInput")
    bk = nc.dram_tensor("bk", [dh], F32, kind="ExternalInput")
    bv = nc.dram_tensor("bv", [dh], F32, kind="ExternalInput")
    bo = nc.dram_tensor("bo", [dm], F32, kind="ExternalInput")
    m_tiles = nc.dram_tensor("m_tiles", [max(n_gen, 1), KB, QC], mybir.dt.int32,
                             kind="ExternalInput")
    y_out = nc.dram_tensor("y_out", [out_rows, dm], F32, kind="ExternalOutput")

    # a2a staging: chunked mode [chunk][8 dest blocks][256 feats][64 q];
    # single mode [8 dest blocks][chunk][256 feats][64 q] (one collective)
    if chunked_cc:
        a2a_in = nc.dram_tensor("a2a_in", [nqc, nsub, dh, 64], BF16)
        a2a_out = nc.dram_tensor("a2a_out", [nqc, nsub, dh, 64], BF16)
    else:
        a2a_in = nc.dram_tensor("a2a_in", [nsub, nqc, dh, 64], BF16)
        a2a_out = nc.dram_tensor("a2a_out", [nsub, nqc, dh, 64], BF16)

    with tile.TileContext(nc) as tc, ExitStack() as ex:
        persist = ex.enter_context(tc.tile_pool(name="persist", bufs=1))
        work = ex.enter_context(tc.tile_pool(name="work", bufs=4))
        ps_w = ex.enter_context(tc.tile_pool(name="ps_w", bufs=2, space="PSUM"))
        ps_s = ex.enter_context(tc.tile_pool(name="ps_s", bufs=2, space="PSUM"))
        ps_o = ex.enter_context(tc.tile_pool(name="ps_o", bufs=2, space="PSUM"))
        qbufs = 2 if causal_dep else nqc
        xpool = ex.enter_context(tc.tile_pool(name="xpool", bufs=4))
        qpool = ex.enter_context(tc.tile_pool(name="qpool", bufs=qbufs))
        ppool = ex.enter_context(tc.tile_pool(name="ppool", bufs=8))
        cpool = ex.enter_context(tc.tile_pool(name="cpool", bufs=2))
        ypool = ex.enter_context(tc.tile_pool(name="ypool", bufs=2))
        wpool = ex.enter_context(tc.tile_pool(name="wpool", bufs=1))

        # ---- constants ----
        # tri[k, q] = 1.0 where k <= q (keep), else 0
        tri = persist.tile([P, P], pvt, tag="tri", name="tri")
        tri_b = persist.tile([P, P], st, tag="tri_b", name="tri_b")
        nc.gpsimd.memset(tri_b[:], 0.0)
        nc.gpsimd.affine_select(out=tri_b[:], in_=tri_b[:],
                                compare_op=mybir.AluOpType.is_gt,
                                fill=1.0, base=0,
                                pattern=[[-1, P]], channel_multiplier=1)
        if pvt == st:
            tri = tri_b
        else:
            nc.vector.tensor_copy(tri[:], tri_b[:])
        ones1 = persist.tile([1, P], st, tag="ones1", name="ones1")
        nc.gpsimd.memset(ones1[:], 1.0)
        # preload the Exp table while DMAs stream in
        actwarm = persist.tile([1, 1], F32, tag="actwarm", name="actwarm")
        nc.scalar.activation(actwarm[:], ones1[0:1, 0:1],
                             mybir.ActivationFunctionType.Exp)
        scr = persist.tile([1, QC], st, tag="scr", name="scr")
        nc.vector.memset(scr[:], 1.0)
        HEAD_WARM = 11

        def pe_keepwarm(n):
            # dummy matmuls bridge a PE idle window so the clock does not
            # drop out of max p-state before the next real matmul burst
            pwu = ps_o.tile([1, QC], F32, tag="ps_av", name="pwu")
            for i in range(n):
                nc.tensor.matmul(pwu[:], scr[0:1, 0:1], scr[:],
                                 start=(i == 0), stop=(i == n - 1))

        gen_sb = None
        if n_gen:
            gen_sb = persist.tile([P, n_gen, QC], pvt, tag="gen", name="gen")
            gi = persist.tile([P, n_gen, QC], mybir.dt.int32, tag="gen_i",
                              name="gen_i")
            nc.sync.dma_start(gi[:], m_tiles[:].rearrange("n p q -> p n q"))
            for i in range(n_gen):
                if pvt == st:
                    nc.vector.tensor_copy(gen_sb[:, i, :], gi[:, i, :])
                else:
                    gb = work.tile([P, QC], st, tag="gen_b", name="gen_b")
                    nc.vector.tensor_copy(gb[:], gi[:, i, :])
                    nc.vector.tensor_copy(gen_sb[:, i, :], gb[:])

        DR0 = mybir.MatmulPerfMode.DoubleRow

        # ---- persistent activation storage (memsets run at t=0) ----
        kT_sb = [persist.tile([P, s], qkt, tag=f"kT{t}", name=f"kT{t}")
                 for t in range(ndh)]
        # V in natural layout, heads side by side, with a ones column:
        # v_all[kb_row, ikb, h, 0:DK] = v_h[key, :], v_all[.., DK] = 1
        # fp8 DoubleRow ldweights needs 4-byte-aligned slot strides: pad
        # each head's [d_k | ones] slot to VW columns (tail zeroed)
        VW = DK + 4 if pv8 else DK + 1
        v_all = persist.tile([P, nkb, hloc, VW], pvt, tag="v_all",
                             name="v_all")
        nc.gpsimd.memset(v_all[:, :, :, DK], 1.0)
        if VW > DK + 1:
            nc.gpsimd.memset(v_all[:, :, :, DK + 1:VW], 0.0)

        qf_tiles = {}

        def make_qf(iqc, memset=False):
            qf = [qpool.tile([P, 2, QC], qkt, tag=f"qf{t}", name=f"qf{t}_{iqc}")
                  for t in range(ndh)]
            qf_tiles[iqc] = qf
            if memset and qk8:
                # pool ring: zero slots persist across later buffer reuse
                for t in range(ndh):
                    nc.vector.memset(qf[t][:, 1, :], 0.0)
            return qf

        for c in range(qbufs):
            make_qf(c, memset=True)

        # ---- load weights (bf16 from host; batched DMAs) ----
        # spread DMAs across both HWDGE queues (SP + Activation)
        _dmaq = [0]

        def dmaq():
            _dmaq[0] ^= 1
            return nc.sync if _dmaq[0] else nc.scalar

        # wq first, then w_inT in quarters: the first combine matmuls only
        # need w_in chunk u=0 + wq, so PE can start ~2.7us in
        w_sb = {}
        w8_sb = {}
        if qk8:
            # one packed DMA delivers all score-path fp8 weights
            # ([Wq8 | Wk8 | W_inT8] along the column axis)
            wide = 2 * dh + dm
            w8all = wpool.tile([P, nde, wide], qkt, tag="w8all", name="w8all")
            hw8 = nde // 4
            for i in range(4):
                dmaq().dma_start(
                    out=w8all[:, i * hw8:(i + 1) * hw8, :],
                    in_=w8pack[i * hw8 * P:(i + 1) * hw8 * P, :]
                        .rearrange("(u p) d -> p u d", p=P))
            w8_sb["q"] = w8all[:, :, 0:dh]
            w8_sb["k"] = w8all[:, :, dh:2 * dh]
            w_in8_b = w8all[:, :, 2 * dh:wide]
        w_inT_b = wpool.tile([P, nde, dm], st, tag="w_inT", name="w_inT_b")
        w_inT_sb = [w_inT_b[:, u, :] for u in range(nde)]
        b_inT = wpool.tile([P, nde], st, tag="b_inT", name="b_inT")
        bo_bcast = persist.tile([P, dm], F32, tag="bo_b", name="bo_bcast")
        for name in ("q", "k", "v"):
            wb = wpool.tile([P, nde, dh], st, tag=f"w{name}", name=f"w{name}_b")
            w_sb[name] = [wb[:, u, :] for u in range(nde)]
            w_sb[name + "_t"] = wb

        def load_w_small(name):
            # bf16 head-projection weights (bias combine) + b_in
            def f():
                dmaq().dma_start(
                    out=w_sb[name + "_t"][:],
                    in_={"q": wq, "k": wk, "v": wv}[name]
                        .rearrange("(u p) d -> p u d", p=P))
                if name == "q":
                    nc.scalar.dma_start(
                        out=b_inT[:], in_=b_in[:].rearrange("(t p) -> p t", p=P))
            return f

        def load_w_inT_bf16():
            hd4 = nde // 4
            for i in range(4):
                dmaq().dma_start(
                    out=w_inT_b[:, i * hd4:(i + 1) * hd4, :],
                    in_=w_inT[i * hd4 * P:(i + 1) * hd4 * P, :]
                        .rearrange("(u p) m -> p u m", p=P))
            nc.scalar.dma_start(out=bo_bcast[:],
                                in_=bo[:].unsqueeze(0).broadcast_to([P, dm]))
        wo_sb = persist.tile([P, nde, dm], st, tag="wo", name="wo_b")

        def load_wo():
            # deferred: wo is not needed until the first output projection
            hdo = nde // 2
            nc.sync.dma_start(out=wo_sb[:, 0:hdo, :],
                              in_=wo[0:hdo * P, :].rearrange("(u p) m -> p u m", p=P))
            nc.scalar.dma_start(out=wo_sb[:, hdo:nde, :],
                                in_=wo[hdo * P:, :].rearrange("(u p) m -> p u m", p=P))

        # ---- combine weights: Wc_x = W_in @ Wx (+ bias fold) ----
        # q/k: fp8 DoubleRow over paired de-chunks -> paired-layout wc8
        # (wc8[name][t//2][:, t%2, :] = Wc rows of dm-chunk t); v: bf16
        wc = {}
        wc8 = {}
        bc = {}

        def combine_qk8(name):
            wc8[name] = [persist.tile([P, 2, dh], qkt, tag=f"wc8{name}{t}",
                                      name=f"wc8{name}{t}")
                         for t in range(ndm // 2)]
            for tp in range(ndm // 2):
                ps = ps_w.tile([P, 2 * dh], F32, tag="ps_w", name="ps_w")
                for half in range(2):
                    t = 2 * tp + half
                    for i in range(nde // 2):
                        nc.tensor.matmul(
                            ps[:, half * dh:(half + 1) * dh],
                            w_in8_b[:, 2 * i:2 * i + 2, t * P:(t + 1) * P],
                            w8_sb[name][:, 2 * i:2 * i + 2, :],
                            perf_mode=DR0,
                            start=(i == 0), stop=(i == nde // 2 - 1))
                nc.vector.tensor_copy(
                    wc8[name][tp][:],
                    ps[:].rearrange("p (two d) -> p two d", two=2))

        def combine_bf16_closures(name):
            wc[name] = [persist.tile([P, dh], st, tag=f"wc{name}{t}",
                                     name=f"wc{name}{t}") for t in range(ndm)]

            def piece(ts_):
                def f():
                    for t in ts_:
                        ps = ps_w.tile([P, dh], F32, tag="ps_w", name="ps_w")
                        for u in range(nde):
                            nc.tensor.matmul(
                                ps[:], w_inT_sb[u][:, t * P:(t + 1) * P],
                                w_sb[name][u][:],
                                start=(u == 0), stop=(u == nde - 1))
                        nc.vector.tensor_copy(wc[name][t][:], ps[:])
                return f
            return [piece(ts_) for ts_ in
                    ([0, 1], [2, 3], [4, 5], [6, 7])]

        def bias_qk(name):
            bvec = {"q": bq, "k": bk}[name]
            bxT = wpool.tile([P, ndh], F32, tag=f"bxT{name}", name=f"bxT{name}")
            nc.sync.dma_start(out=bxT[:], in_=bvec[:].rearrange("(t p) -> p t", p=P))
            bc[name] = persist.tile([P, ndh], F32, tag=f"bc{name}", name=f"bc{name}")
            for t in range(ndh):
                ps = ps_w.tile([P, 1], F32, tag="ps_w", name="ps_w")
                for u in range(nde):
                    nc.tensor.matmul(
                        ps[:], w_sb[name][u][:, t * P:(t + 1) * P],
                        b_inT[:, u:u + 1],
                        start=(u == 0), stop=(u == nde - 1))
                nc.vector.tensor_add(bc[name][:, t:t + 1], ps[:], bxT[:, t:t + 1])

        bcv_row = persist.tile([1, dh], st, tag="bcv", name="bcv_row")

        def bias_v():
            bv_row = wpool.tile([1, dh], F32, tag="bv_row", name="bv_row")
            nc.sync.dma_start(out=bv_row[:], in_=bv[:].unsqueeze(0))
            ps = ps_w.tile([1, dh], F32, tag="ps_w", name="ps_w")
            for u in range(nde):
                nc.tensor.matmul(ps[:], b_inT[:, u:u + 1], w_sb["v"][u][:],
                                 start=(u == 0), stop=(u == nde - 1))
            nc.vector.tensor_add(bcv_row[:], ps[:], bv_row[:])

        def proj_closures(iqc):
            """Per-chunk projection emission, split into PE-sized closures."""
            clos = []
            qf = qf_tiles.get(iqc) or make_qf(iqc, memset=iqc < qbufs)
            xbs = {}

            def load(name, xdram):
                def f():
                    dt_ = qkt if (qk8 and name in ("q", "k")) else st
                    tag = "xb8" if (qk8 and name in ("q", "k")) else "xb"
                    xb = xpool.tile([P, ndm, QC], dt_, tag=tag,
                                    name=f"xb_{name}{iqc}")
                    xbs[name] = xb
                    dmaq().dma_start(
                        out=xb[:],
                        in_=xdram[:, iqc * QC:(iqc + 1) * QC]
                            .rearrange("(u p) s -> p u s", p=P))
                return f

            def qk_part(name, t):
                def f():
                    xb = xbs[name]
                    ps = ps_w.tile([P, QC], F32, tag="ps_w", name="ps_w")
                    if qk8:
                        for i in range(ndm // 2):
                            nc.tensor.matmul(
                                ps[:],
                                wc8[name][i][:, :, t * P:(t + 1) * P],
                                xb[:, 2 * i:2 * i + 2, :],
                                perf_mode=DR0,
                                start=(i == 0), stop=(i == ndm // 2 - 1))
                    else:
                        for u in range(ndm):
                            nc.tensor.matmul(
                                ps[:], wc[name][u][:, t * P:(t + 1) * P],
                                xb[:, u, :], start=(u == 0),
                                stop=(u == ndm - 1))
                    if name == "k":
                        nc.vector.tensor_scalar_add(
                            kT_sb[t][:, iqc * QC:(iqc + 1) * QC], ps[:],
                            bc["k"][:, t:t + 1])
                    else:
                        nc.vector.tensor_scalar_add(
                            qf[t][:, 0, :], ps[:], bc["q"][:, t:t + 1])
                return f

            def v_part(j):
                def f():
                    xb = xbs["v"]
                    ikb = iqc * (QC // P) + j
                    ps = ps_w.tile([P, dh], F32, tag="ps_w", name="ps_w")
                    for u in range(ndm):
                        nc.tensor.matmul(
                            ps[:], xb[:, u, j * P:(j + 1) * P], wc["v"][u][:],
                            start=(u == 0), stop=False)
                    nc.tensor.matmul(ps[:], ones1[:], bcv_row[:],
                                     start=False, stop=True)
                    nc.vector.tensor_copy(
                        v_all[:, ikb, :, 0:DK],
                        ps[:].rearrange("p (h d) -> p h d", h=hloc))
                return f

            clos.append(load("q", qT))
            for t in range(ndh):
                clos.append(qk_part("q", t))
            clos.append(load("k", kT))
            for t in range(ndh):
                clos.append(qk_part("k", t))
            clos.append(load("v", vT))
            for j in range(QC // P):
                clos.append(v_part(j))
            return clos

        # ---- attention ----
        inv_sqrt = 1.0 / math.sqrt(DK)
        DR = mybir.MatmulPerfMode.DoubleRow
        cT_tiles = {}

        def attention_chunk(iqc, fillers=(), mid=None):
            """QK+exp of step i overlaps PV of step i-1; `fillers` (next
            chunk's projection closures) are spread over the early steps;
            `mid` (the previous chunk's yin load) fires ~70% through."""
            qf = qf_tiles[iqc]
            cT = cpool.tile([P, ndh, QC], st, tag="cT", name=f"cT{iqc}")
            cT_tiles[iqc] = cT
            blist = blocks[iqc]
            steps = []
            for h in range(hloc):
                grps = [blist[g0:g0 + GW] for g0 in range(0, len(blist), GW)]
                for g in range(len(grps)):
                    steps.append((h, grps[g], g == 0, g == len(grps) - 1))
            po = {}
            pending = []
            fillers = list(fillers)
            n_steps = len(steps)
            fill_at = {}
            if fillers:
                # spread fillers uniformly across the steps
                for fi in range(len(fillers)):
                    at = (fi * n_steps) // len(fillers)
                    fill_at.setdefault(min(at, n_steps - 1), []).append(
                        fillers[fi])
            mid_at = (6 * n_steps) // 10

            def emit_qk_exp(h, grp):
                # TRI blocks with offset r have their first r*P q-columns
                # fully masked: skip them in both the QK matmul and the exp
                t, off = h // 2, (h % 2) * DK
                q0s = [arg * P if mode == TRI else 0
                       for (ikb, mode, arg) in grp]
                pss = ps_s.tile([P, GW * QC], F32, tag="ps_scores",
                                name="ps_scores")
                for j, (ikb, mode, arg) in enumerate(grp):
                    q0 = q0s[j]
                    kv = kT_sb[t][off:off + DK, ikb * KB:(ikb + 1) * KB]
                    if qk8:
                        nc.tensor.matmul(
                            pss[:, j * QC + q0:(j + 1) * QC],
                            kv.unsqueeze(1).broadcast_to([DK, 2, KB]),
                            qf[t][off:off + DK, :, q0:QC],
                            perf_mode=DR, start=True, stop=True)
                    else:
                        nc.tensor.matmul(pss[:, j * QC + q0:(j + 1) * QC],
                                         kv, qf[t][off:off + DK, 0, q0:QC])
                pt = ppool.tile([P, GW * QC], pvt, tag="p", name="p")
                # one group-wide exp: skipped QK leads hold stale (finite)
                # psum values; the TRI memset below overwrites their pt cols
                nw = len(grp) * QC
                nc.scalar.activation(pt[:, 0:nw], pss[:, 0:nw],
                                     mybir.ActivationFunctionType.Exp,
                                     scale=inv_sqrt)
                for j, (ikb, mode, arg) in enumerate(grp):
                    pj = pt[:, j * QC:(j + 1) * QC]
                    if mode == TRI:
                        r = arg
                        if r > 0:
                            nc.gpsimd.memset(pj[:, 0:r * P], 0.0)
                        nc.vector.tensor_mul(
                            pj[:, r * P:(r + 1) * P],
                            pj[:, r * P:(r + 1) * P], tri[:])
                    elif mode == GEN:
                        nc.vector.tensor_mul(pj[:], pj[:], gen_sb[:, arg, :])
                return pt

            def emit_pv(h, grp, pt, first, last):
                if first:
                    po[h] = ps_o.tile([VW, QC], F32, tag="ps_av",
                                      name="ps_av")
                ikbs = [ikb for (ikb, _, _) in grp]
                if pv8 and len(grp) == 2 and ikbs[1] == ikbs[0] + 1:
                    nc.tensor.matmul(
                        po[h][:], v_all[:, ikbs[0]:ikbs[0] + 2, h, :],
                        pt[:].rearrange("p (two q) -> p two q", two=2),
                        perf_mode=DR,
                        start=first, stop=last)
                else:
                    for j, (ikb, mode, arg) in enumerate(grp):
                        nc.tensor.matmul(
                            po[h][:, 0:QC], v_all[:, ikb, h, :],
                            pt[:, j * QC:(j + 1) * QC],
                            start=(first and j == 0),
                            stop=(last and j == len(grp) - 1))
                if last:
                    rec1 = work.tile([1, QC], F32, tag="rec1", name="rec1")
                    nc.vector.reciprocal(rec1[:], po[h][DK:DK + 1, :])
                    recb = work.tile([DK, QC], F32, tag="recb", name="recb")
                    nc.gpsimd.partition_broadcast(recb[:], rec1[:])
                    nc.vector.tensor_mul(
                        cT[(h % 2) * DK:(h % 2) * DK + DK, h // 2, :],
                        po[h][0:DK, :], recb[:])
                    del po[h]

            for i, (h, grp, first, last) in enumerate(steps):
                pt = emit_qk_exp(h, grp)
                pending.append((h, grp, pt, first, last))
                if len(pending) > 3:
                    emit_pv(*pending.pop(0))
                for f in fill_at.get(i, ()):
                    f()
                if mid is not None and i == mid_at:
                    mid()
                    mid = None
            for p_ in pending:
                emit_pv(*p_)
            if mid is not None:
                mid()

        # ---- a2a + output projection ----
        def a2a_chunk(c):
            # cT [128, ndh, QC] -> a2a_in[c or :, c] [8, 256, 64]
            cT = cT_tiles[c]
            dst = a2a_in[c] if chunked_cc else a2a_in[:, c]
            for fh in range(ndh):
                (nc.sync if fh % 2 == 0 else nc.scalar).dma_start(
                    out=dst[:, fh * P:(fh + 1) * P, :]
                        .rearrange("r p j -> p r j"),
                    in_=cT[:, fh, :].rearrange("p (r j) -> p r j", r=nsub))
            if not chunked_cc:
                if c == nqc - 1:
                    if collective:
                        for _ in range(cc_reps):
                            nc.gpsimd.collective_compute(
                                "AllToAll", mybir.AluOpType.bypass,
                                replica_groups=[list(range(N_CORES))],
                                ins=[a2a_in[:].opt()], outs=[a2a_out[:].opt()])
                    else:
                        nc.sync.dma_start(out=a2a_out[:], in_=a2a_in[:])
                return
            if collective:
                for _ in range(cc_reps):
                    nc.gpsimd.collective_compute(
                        "AllToAll", mybir.AluOpType.bypass,
                        replica_groups=[list(range(N_CORES))],
                        ins=[a2a_in[c].opt()], outs=[a2a_out[c].opt()])
            else:
                nc.sync.dma_start(out=a2a_out[c], in_=a2a_in[c])

        yin_tiles = {}

        def yin_load(c):
            yin = ypool.tile([P, nde, P], st, tag="yin", name=f"yin{c}")
            yin_tiles[c] = yin
            half = nsub // 2
            if chunked_cc:
                src = a2a_out[c]
                nc.sync.dma_start(
                    out=yin[:, :, 0:64],
                    in_=src[0:half].rearrange("s (fh p) j -> p (s fh) j", p=P))
                nc.scalar.dma_start(
                    out=yin[:, :, 64:128],
                    in_=src[half:nsub].rearrange("s (fh p) j -> p (s fh) j",
                                                 p=P))
            else:
                yv = yin[:].rearrange("p (s fh) j -> p s fh j", fh=ndh)
                for b0, sl in ((0, slice(0, 64)), (half, slice(64, 128))):
                    for fh in range(ndh):
                        nc.sync.dma_start(
                            out=yv[:, :, fh, sl],
                            in_=a2a_out[b0:b0 + half, c, fh * P:(fh + 1) * P, :]
                                .rearrange("s p j -> p s j"))

        def yproj_mm(c, split_store=False):
            yin = yin_tiles[c]
            ys = ypool.tile([P, dm], F32, tag="ys", name=f"ys{c}")
            for mb in range(dm // QC):
                ps = ps_w.tile([P, QC], F32, tag="ps_w", name="ps_w")
                for u in range(nde):
                    nc.tensor.matmul(
                        ps[:], yin[:, u, :],
                        wo_sb[:, u, mb * QC:(mb + 1) * QC],
                        start=(u == 0), stop=(u == nde - 1))
                nc.vector.tensor_add(ys[:, mb * QC:(mb + 1) * QC], ps[:],
                                     bo_bcast[:, mb * QC:(mb + 1) * QC])
                if split_store:
                    # let the first half's store overlap the second half
                    (nc.sync if mb % 2 == 0 else nc.scalar).dma_start(
                        out=y_out[c * P:(c + 1) * P, mb * QC:(mb + 1) * QC],
                        in_=ys[:, mb * QC:(mb + 1) * QC])
            if not split_store:
                nc.sync.dma_start(out=y_out[c * P:(c + 1) * P, :], in_=ys[:])

        # ---- schedule ----
        # head: score-path (fp8) combine + q/k projections first so the
        # first attention exp fires ~7us in; the bf16 v-combine and chunk-0
        # V projection become PE filler inside attention chunk 0.
        # yproj(c) is deferred into later chunks' attention as PE filler:
        # the final chunks have the most exp work and no projections left.
        if causal_dep:
            pc0 = proj_closures(0)
            pc0[0]()                       # load q chunk 0
            pc0[3]()                       # load k chunk 0
            if qk8:
                load_w_small("q")()
                load_w_small("k")()
                combine_qk8("q")
                bias_qk("q")
                pc0[1](); pc0[2]()         # project q chunk 0
                combine_qk8("k")
                bias_qk("k")
                pc0[4](); pc0[5]()         # project k chunk 0
            else:
                load_w_small("q")()
                load_w_small("k")()
                load_w_inT_bf16()
                for f in combine_bf16_closures("q"):
                    f()
                bias_qk("q")
                pc0[1](); pc0[2]()
                for f in combine_bf16_closures("k"):
                    f()
                bias_qk("k")
                pc0[4](); pc0[5]()
            pc0[6]()                       # load v chunk 0
            load_w_small("v")()
            if qk8:
                load_w_inT_bf16()
            head_fill = combine_bf16_closures("v") + [bias_v]
            load_wo()
            for c in range(nqc):
                fillers = list(head_fill)
                head_fill = []
                if c == 0:
                    fillers += pc0[7:]     # chunk-0 V projection
                if c + 1 < nqc:
                    fillers += list(proj_closures(c + 1))
                if chunked_cc and c == nqc - 1:
                    for cc in range(nqc - 2):
                        fillers.append((lambda c2: lambda: yproj_mm(c2))(cc))
                mid = ((lambda cc: lambda: yin_load(cc))(c - 1)
                       if c > 0 and chunked_cc else None)
                attention_chunk(c, fillers=fillers, mid=mid)
                a2a_chunk(c)
            if chunked_cc:
                yproj_mm(nqc - 2)
                pe_keepwarm(TAIL_WARM)
            else:
                for c in range(nqc - 1):
                    yin_load(c)
                    yproj_mm(c)
        else:
            # general masks: all projections first, then attention
            load_w_small("q")()
            load_w_small("k")()
            load_w_small("v")()
            load_w_inT_bf16()
            if qk8:
                combine_qk8("q")
                combine_qk8("k")
            else:
                for f in combine_bf16_closures("q") + combine_bf16_closures("k"):
                    f()
            bias_qk("q")
            bias_qk("k")
            for f in combine_bf16_closures("v"):
                f()
            bias_v()
            for c in range(nqc):
                for f in proj_closures(c):
                    f()
            load_wo()
            for c in range(nqc):
                mid = (lambda cc: lambda: yin_load(cc))(c - 1) if c > 0 else None
                attention_chunk(c, mid=mid)
                a2a_chunk(c)
                if c > 0:
                    yproj_mm(c - 1)
        yin_load(nqc - 1)
        yproj_mm(nqc - 1, split_store=True)

    nc.compile()
    return nc


# ------------------------------------------------------------------
_CACHE = {}


def _get_compiled(plan_key, blocks, n_gen, mm):
    if plan_key not in _CACHE:
        nc = build_mha(blocks, n_gen, mm=mm)
        nc.m = get_hw_module(nc.m)
        _CACHE[plan_key] = nc
    return _CACHE[plan_key]


def make_in_maps(q, k, v, mask, W_in, b_in, Wq, bq, Wk, bk, Wv, bv, Wo, bo,
                 blocks=None, n_gen=None, gen_tiles=None):
    if blocks is None:
        blocks, n_gen, gen_tiles = make_plan(mask)
    bf16 = mybir.dt.np(BF16)
    fp8 = mybir.dt.np(FP8)
    dh = DE // 4
    tb = lambda a: np.ascontiguousarray(np.asarray(a).T).astype(bf16)
    cb = lambda a: np.ascontiguousarray(np.asarray(a)).astype(bf16)
    t8 = lambda a: np.ascontiguousarray(np.asarray(a).T).astype(fp8)
    c8 = lambda a: np.ascontiguousarray(np.asarray(a)).astype(fp8)
    in_maps = []
    for c in range(N_CORES):
        b, g = c // 4, c % 4
        sl = slice(g * dh, (g + 1) * dh)
        mt = (gen_tiles[b] if n_gen else
              np.zeros((1, KB, QC), np.int32))
        qk8 = MM_MODE in ("fp8", "fp8qk")
        tq = t8 if qk8 else tb
        w8p = np.concatenate(
            [np.asarray(Wq)[:, sl], np.asarray(Wk)[:, sl],
             np.ascontiguousarray(np.asarray(W_in).T)], axis=1)
        in_maps.append({
            "qT": tq(q[b]), "kT": tq(k[b]), "vT": tb(v[b]),
            "w_inT": tb(W_in),
            "w8pack": c8(w8p),
            "wq": cb(Wq[:, sl]),
            "wk": cb(Wk[:, sl]),
            "wv": cb(Wv[:, sl]),
            "wo": cb(Wo),
            "b_in": np.asarray(b_in).astype(bf16),
            "bq": np.ascontiguousarray(np.asarray(bq)[sl]),
            "bk": np.ascontiguousarray(np.asarray(bk)[sl]),
            "bv": np.ascontiguousarray(np.asarray(bv)[sl]),
            "bo": np.asarray(bo),
            "m_tiles": mt,
        })
    return in_maps, blocks, n_gen


def assemble(results):
    out = np.empty((B, S, DM), np.float32)
    for core in range(N_CORES):
        y = results[core]["y_out"]            # [nqc*128, DM]
        for c in range(S // QC):
            for b in range(B):
                out[b, c * QC + core * 64:c * QC + (core + 1) * 64, :] = \
                    y[c * P + b * 64:c * P + (b + 1) * 64, :]
    return out


MM_MODE = "fp8qk"


def kernel(**inputs):
    mask = inputs["mask"]
    blocks, n_gen, gen_tiles = make_plan(np.asarray(mask))
    plan_key = (str(blocks), n_gen, MM_MODE)
    nc = _get_compiled(plan_key, blocks, n_gen, MM_MODE)
    in_maps, _, _ = make_in_maps(
        inputs["q"], inputs["k"], inputs["v"], mask,
        inputs["W_in"], inputs["b_in"], inputs["Wq"], inputs["bq"],
        inputs["Wk"], inputs["bk"], inputs["Wv"], inputs["bv"],
        inputs["Wo"], inputs["bo"],
        blocks=blocks, n_gen=n_gen, gen_tiles=gen_tiles)
    res = bass_utils.run_bass_kernel_spmd(nc, in_maps,
                                          core_ids=list(range(N_CORES)))
    return assemble(res.results)


# revision 10
# speedup vs baseline: 1.0692x; 1.0106x over previous
"""Trainium2 Bass kernel for nn_MultiHeadAttention_72069551227273 (v2).

Reference computation (B=2, S=2048, D_MODEL=D_EMB=1024, H=16, d_k=64):
    q_p = q @ W_in + b_in                    (shared input projection)
    qh  = heads(q_p @ Wq + bq)               (per-head projections)
    s   = qh @ kh^T / sqrt(d_k), causal-masked softmax
    out = (attn @ vh, concat heads) @ Wo + bo

Sharding: 8 cores = 2 (batch) x 4 (head groups of 4 heads / 256 emb cols).
Per core the input and head projections are fused on device:
    Q = q @ (W_in @ Wq_slice) + (b_in @ Wq_slice + bq_slice)
The whole score path (W_in@Wq/Wk weight combine, Q/K projections, QK^T)
runs in fp8e4m3 DoubleRow perf mode with contraction chunks paired into
the two DoubleRow slots: 4x MACs/cycle over bf16 for combine+projections
and 2x for scores (score lhsT slots = (K, K) via a stride-0 broadcast,
rhs slots = (Q, 0)).  fp8 score noise washes out in the softmax; the
value path (V, attn@V, Wo) stays bf16 — fp8 there measured 2.4e-2
relative error, over the 2e-2 gate.  V is projected straight into the
natural [seq, head, d_k] layout (no PE transposes) with its bias folded
in via a rank-1 ones matmul.  Softmax is exp(s/8) without max-subtraction;
the denominator comes free from a ones column appended to V.  Fully-masked
score blocks are skipped at trace time, diagonal blocks get an on-chip
triangular mask.  Projection work for chunk c+1 is interleaved into the
attention steps of chunk c (attention is Act/exp-heavy, projections are
PE-heavy), and attention itself is software-pipelined so PV of step i-1
overlaps the exp of step i.

Output stage is sequence-parallel instead of tensor-parallel: after
attention chunk c, an 8-way AllToAll (bf16, 256 KB) redistributes the
attention outputs so every core holds all 1024 features for 64 q rows of
each batch, then applies the full Wo locally — there is no reduction
collective at all.  The per-chunk collectives overlap with the next
chunk's attention; only the last chunk's exchange is exposed.
"""

import sys

sys.path.append("/opt/trn_rl_repo")

import math
from contextlib import ExitStack

import numpy as np

import concourse.bass as bass
import concourse.bacc as bacc
import concourse.mybir as mybir
import concourse.tile as tile
from concourse import bass_utils
from concourse.bass_interp import get_hw_module

# problem dims
B, S, DM, DE, H, DK = 2, 2048, 1024, 1024, 16, 64
N_CORES = 8
P = 128                      # partitions
QC = 512                     # q chunk (psum bank width in fp32)
KB = 128                     # k block (scores^T partition block)
GW = 2                       # kb blocks per score-psum tile (2 banks)
TAIL_WARM = 58               # PE keep-warm matmuls bridging the last a2a wait

F32 = mybir.dt.float32
BF16 = mybir.dt.bfloat16
FP8 = mybir.dt.float8e4

FULL, TRI, GEN, SKIP = 0, 1, 2, 3


def make_plan(mask_np, s=S, qc=QC, kb=KB):
    """Classify scores^T blocks [kb x qc] from the (B, S, S) 0/1 mask.

    Returns (blocks, n_gen_tiles, gen_tiles_per_batch):
      blocks[iqc] = list of (ikb, mode, arg)
    """
    nqc, nkb = s // qc, s // kb
    m = np.asarray(mask_np) != 0          # [B, S(q), S(k)] True = attend
    tril = np.tril(np.ones((s, s), bool))
    causal = all(np.array_equal(m[b], tril) for b in range(m.shape[0]))
    blocks = []
    if causal:
        for iqc in range(nqc):
            row = []
            for ikb in range(nkb):
                if (ikb + 1) * kb <= iqc * qc:
                    row.append((ikb, FULL, 0))
                elif ikb * kb < (iqc + 1) * qc:
                    row.append((ikb, TRI, (ikb * kb - iqc * qc) // kb))
                # else fully masked -> skip
            blocks.append(row)
        return blocks, 0, None

    # general path: per-block classification, unioned across batches
    nb = m.shape[0]
    # every query row must attend to >= 1 key (else softmax semantics differ)
    assert m.any(axis=-1).all(), "fully-masked query rows unsupported"
    gen_tiles = [[] for _ in range(nb)]
    for iqc in range(nqc):
        row = []
        for ikb in range(nkb):
            sub = m[:, iqc * qc:(iqc + 1) * qc, ikb * kb:(ikb + 1) * kb]
            if sub.all():
                row.append((ikb, FULL, 0))
            elif not sub.any():
                continue
            else:
                idx = len(gen_tiles[0])
                for b in range(nb):
                    gen_tiles[b].append(sub[b].T.astype(np.int32))  # [kb, qc]
                row.append((ikb, GEN, idx))
        blocks.append(row)
    n_gen = len(gen_tiles[0])
    gt = [np.stack(g) if n_gen else np.zeros((1, kb, qc), np.int32)
          for g in gen_tiles]
    return blocks, n_gen, gt


def build_mha(blocks, n_gen, *, s=S, dm=DM, de=DE, dh=None, mm="fp8",
              collective=True, chunked_cc=True, cc_reps=1):
    """Trace the per-core MHA program.  dh = per-core emb slice (256)."""
    if dh is None:
        dh = DE // 4
    nqc, nkb, ndm, nde = s // QC, s // KB, dm // P, de // P
    ndh = dh // P            # feature chunks per core (2)
    hloc = dh // DK          # heads per core (4)
    nsub = QC // 64          # a2a sub-blocks per chunk (8)
    out_rows = nqc * P       # output rows per core (4 chunks x 2 x 64)

    qk8 = mm in ("fp8", "fp8qk")
    pv8 = (mm == "fp8")
    qkt = FP8 if qk8 else BF16
    pvt = FP8 if pv8 else BF16   # dtype of probs, V, and mask tiles
    st = BF16

    # can attention chunk c start right after projection chunk c?
    causal_dep = all(
        max([c] + [ikb * KB // QC for (ikb, _, _) in blocks[c]]) <= c
        for c in range(nqc))

    nc = bacc.Bacc("TRN2", target_bir_lowering=False, debug=False,
                   num_devices=N_CORES)

    # ---- kernel I/O (per core) ----
    qT = nc.dram_tensor("qT", [dm, s], qkt, kind="ExternalInput")
    kT = nc.dram_tensor("kT", [dm, s], qkt, kind="ExternalInput")
    vT = nc.dram_tensor("vT", [dm, s], BF16, kind="ExternalInput")
    w_inT = nc.dram_tensor("w_inT", [de, dm], BF16, kind="ExternalInput")
    wq = nc.dram_tensor("wq", [de, dh], BF16, kind="ExternalInput")
    wk = nc.dram_tensor("wk", [de, dh], BF16, kind="ExternalInput")
    w8pack = nc.dram_tensor("w8pack", [de, 2 * dh + dm], qkt,
                            kind="ExternalInput")
    wv = nc.dram_tensor("wv", [de, dh], BF16, kind="ExternalInput")
    wo = nc.dram_tensor("wo", [de, dm], BF16, kind="ExternalInput")
    b_in = nc.dram_tensor("b_in", [de], BF16, kind="ExternalInput")
    bq = nc.dram_tensor("bq", [dh], F32, kind="ExternalInput")
    bk = nc.dram_tensor("bk", [dh], F32, kind="ExternalInput")
    bv = nc.dram_tensor("bv", [dh], F32, kind="ExternalInput")
    bo = nc.dram_tensor("bo", [dm], F32, kind="ExternalInput")
    m_tiles = nc.dram_tensor("m_tiles", [max(n_gen, 1), KB, QC], mybir.dt.int32,
                             kind="ExternalInput")
    y_out = nc.dram_tensor("y_out", [out_rows, dm], F32, kind="ExternalOutput")

    # a2a staging: chunked mode [chunk][8 dest blocks][256 feats][64 q];
    # single mode [8 dest blocks][chunk][256 feats][64 q] (one collective)
    if chunked_cc:
        a2a_in = nc.dram_tensor("a2a_in", [nqc, nsub, dh, 64], BF16)
        a2a_out = nc.dram_tensor("a2a_out", [nqc, nsub, dh, 64], BF16)
    else:
        a2a_in = nc.dram_tensor("a2a_in", [nsub, nqc, dh, 64], BF16)
        a2a_out = nc.dram_tensor("a2a_out", [nsub, nqc, dh, 64], BF16)

    with tile.TileContext(nc) as tc, ExitStack() as ex:
        persist = ex.enter_context(tc.tile_pool(name="persist", bufs=1))
        work = ex.enter_context(tc.tile_pool(name="work", bufs=4))
        ps_w = ex.enter_context(tc.tile_pool(name="ps_w", bufs=2, space="PSUM"))
        ps_s = ex.enter_context(tc.tile_pool(name="ps_s", bufs=2, space="PSUM"))
        ps_o = ex.enter_context(tc.tile_pool(name="ps_o", bufs=2, space="PSUM"))
        qbufs = 2 if causal_dep else nqc
        xpool = ex.enter_context(tc.tile_pool(name="xpool", bufs=4))
        qpool = ex.enter_context(tc.tile_pool(name="qpool", bufs=qbufs))
        ppool = ex.enter_context(tc.tile_pool(name="ppool", bufs=8))
        cpool = ex.enter_context(tc.tile_pool(name="cpool", bufs=2))
        ypool = ex.enter_context(tc.tile_pool(name="ypool", bufs=2))
        wpool = ex.enter_context(tc.tile_pool(name="wpool", bufs=1))

        # ---- constants ----
        # tri[k, q] = 1.0 where k <= q (keep), else 0
        tri = persist.tile([P, P], pvt, tag="tri", name="tri")
        tri_b = persist.tile([P, P], st, tag="tri_b", name="tri_b")
        nc.gpsimd.memset(tri_b[:], 0.0)
        nc.gpsimd.affine_select(out=tri_b[:], in_=tri_b[:],
                                compare_op=mybir.AluOpType.is_gt,
                                fill=1.0, base=0,
                                pattern=[[-1, P]], channel_multiplier=1)
        if pvt == st:
            tri = tri_b
        else:
            nc.vector.tensor_copy(tri[:], tri_b[:])
        ones1 = persist.tile([1, P], st, tag="ones1", name="ones1")
        nc.gpsimd.memset(ones1[:], 1.0)
        # preload the Exp table while DMAs stream in
        actwarm = persist.tile([1, 1], F32, tag="actwarm", name="actwarm")
        nc.scalar.activation(actwarm[:], ones1[0:1, 0:1],
                             mybir.ActivationFunctionType.Exp)
        scr = persist.tile([1, QC], st, tag="scr", name="scr")
        nc.vector.memset(scr[:], 1.0)
        HEAD_WARM = 11

        def pe_keepwarm(n):
            # dummy matmuls bridge a PE idle window so the clock does not
            # drop out of max p-state before the next real matmul burst
            pwu = ps_o.tile([1, QC], F32, tag="ps_av", name="pwu")
            for i in range(n):
                nc.tensor.matmul(pwu[:], scr[0:1, 0:1], scr[:],
                                 start=(i == 0), stop=(i == n - 1))

        gen_sb = None
        if n_gen:
            gen_sb = persist.tile([P, n_gen, QC], pvt, tag="gen", name="gen")
            gi = persist.tile([P, n_gen, QC], mybir.dt.int32, tag="gen_i",
                              name="gen_i")
            nc.sync.dma_start(gi[:], m_tiles[:].rearrange("n p q -> p n q"))
            for i in range(n_gen):
                if pvt == st:
                    nc.vector.tensor_copy(gen_sb[:, i, :], gi[:, i, :])
                else:
                    gb = work.tile([P, QC], st, tag="gen_b", name="gen_b")
                    nc.vector.tensor_copy(gb[:], gi[:, i, :])
                    nc.vector.tensor_copy(gen_sb[:, i, :], gb[:])

        DR0 = mybir.MatmulPerfMode.DoubleRow

        # ---- persistent activation storage (memsets run at t=0) ----
        kT_sb = [persist.tile([P, s], qkt, tag=f"kT{t}", name=f"kT{t}")
                 for t in range(ndh)]
        # V in natural layout, heads side by side, with a ones column:
        # v_all[kb_row, ikb, h, 0:DK] = v_h[key, :], v_all[.., DK] = 1
        # fp8 DoubleRow ldweights needs 4-byte-aligned slot strides: pad
        # each head's [d_k | ones] slot to VW columns (tail zeroed)
        VW = DK + 4 if pv8 else DK + 1
        v_all = persist.tile([P, nkb, hloc, VW], pvt, tag="v_all",
                             name="v_all")
        nc.gpsimd.memset(v_all[:, :, :, DK], 1.0)
        if VW > DK + 1:
            nc.gpsimd.memset(v_all[:, :, :, DK + 1:VW], 0.0)

        qf_tiles = {}

        def make_qf(iqc, memset=False):
            qf = [qpool.tile([P, 2, QC], qkt, tag=f"qf{t}", name=f"qf{t}_{iqc}")
                  for t in range(ndh)]
            qf_tiles[iqc] = qf
            if memset and qk8:
                # pool ring: zero slots persist across later buffer reuse
                for t in range(ndh):
                    nc.vector.memset(qf[t][:, 1, :], 0.0)
            return qf

        for c in range(qbufs):
            make_qf(c, memset=True)

        # ---- load weights (bf16 from host; batched DMAs) ----
        # spread DMAs across both HWDGE queues (SP + Activation)
        _dmaq = [0]

        def dmaq():
            _dmaq[0] ^= 1
            return nc.sync if _dmaq[0] else nc.scalar

        # wq first, then w_inT in quarters: the first combine matmuls only
        # need w_in chunk u=0 + wq, so PE can start ~2.7us in
        w_sb = {}
        w8_sb = {}
        if qk8:
            # one packed DMA delivers all score-path fp8 weights
            # ([Wq8 | Wk8 | W_inT8] along the column axis)
            wide = 2 * dh + dm
            w8all = wpool.tile([P, nde, wide], qkt, tag="w8all", name="w8all")
            hw8 = nde // 4
            for i in range(4):
                dmaq().dma_start(
                    out=w8all[:, i * hw8:(i + 1) * hw8, :],
                    in_=w8pack[i * hw8 * P:(i + 1) * hw8 * P, :]
                        .rearrange("(u p) d -> p u d", p=P))
            w8_sb["q"] = w8all[:, :, 0:dh]
            w8_sb["k"] = w8all[:, :, dh:2 * dh]
            w_in8_b = w8all[:, :, 2 * dh:wide]
        w_inT_b = wpool.tile([P, nde, dm], st, tag="w_inT", name="w_inT_b")
        w_inT_sb = [w_inT_b[:, u, :] for u in range(nde)]
        b_inT = wpool.tile([P, nde], st, tag="b_inT", name="b_inT")
        bo_bcast = persist.tile([P, dm], F32, tag="bo_b", name="bo_bcast")
        for name in ("q", "k", "v"):
            wb = wpool.tile([P, nde, dh], st, tag=f"w{name}", name=f"w{name}_b")
            w_sb[name] = [wb[:, u, :] for u in range(nde)]
            w_sb[name + "_t"] = wb

        def load_w_small(name):
            # bf16 head-projection weights (bias combine) + b_in
            def f():
                dmaq().dma_start(
                    out=w_sb[name + "_t"][:],
                    in_={"q": wq, "k": wk, "v": wv}[name]
                        .rearrange("(u p) d -> p u d", p=P))
                if name == "q":
                    nc.scalar.dma_start(
                        out=b_inT[:], in_=b_in[:].rearrange("(t p) -> p t", p=P))
            return f

        def load_w_inT_bf16():
            hd4 = nde // 4
            for i in range(4):
                dmaq().dma_start(
                    out=w_inT_b[:, i * hd4:(i + 1) * hd4, :],
                    in_=w_inT[i * hd4 * P:(i + 1) * hd4 * P, :]
                        .rearrange("(u p) m -> p u m", p=P))
            nc.scalar.dma_start(out=bo_bcast[:],
                                in_=bo[:].unsqueeze(0).broadcast_to([P, dm]))
        wo_sb = persist.tile([P, nde, dm], st, tag="wo", name="wo_b")

        def load_wo():
            # deferred: wo is not needed until the first output projection
            hdo = nde // 2
            nc.sync.dma_start(out=wo_sb[:, 0:hdo, :],
                              in_=wo[0:hdo * P, :].rearrange("(u p) m -> p u m", p=P))
            nc.scalar.dma_start(out=wo_sb[:, hdo:nde, :],
                                in_=wo[hdo * P:, :].rearrange("(u p) m -> p u m", p=P))

        # ---- combine weights: Wc_x = W_in @ Wx (+ bias fold) ----
        # q/k: fp8 DoubleRow over paired de-chunks -> paired-layout wc8
        # (wc8[name][t//2][:, t%2, :] = Wc rows of dm-chunk t); v: bf16
        wc = {}
        wc8 = {}
        bc = {}

        def combine_qk8(name):
            wc8[name] = [persist.tile([P, 2, dh], qkt, tag=f"wc8{name}{t}",
                                      name=f"wc8{name}{t}")
                         for t in range(ndm // 2)]
            for tp in range(ndm // 2):
                ps = ps_w.tile([P, 2 * dh], F32, tag="ps_w", name="ps_w")
                for half in range(2):
                    t = 2 * tp + half
                    for i in range(nde // 2):
                        nc.tensor.matmul(
                            ps[:, half * dh:(half + 1) * dh],
                            w_in8_b[:, 2 * i:2 * i + 2, t * P:(t + 1) * P],
                            w8_sb[name][:, 2 * i:2 * i + 2, :],
                            perf_mode=DR0,
                            start=(i == 0), stop=(i == nde // 2 - 1))
                nc.vector.tensor_copy(
                    wc8[name][tp][:],
                    ps[:].rearrange("p (two d) -> p two d", two=2))

        def combine_bf16_closures(name):
            wc[name] = [persist.tile([P, dh], st, tag=f"wc{name}{t}",
                                     name=f"wc{name}{t}") for t in range(ndm)]

            def piece(ts_):
                def f():
                    for t in ts_:
                        ps = ps_w.tile([P, dh], F32, tag="ps_w", name="ps_w")
                        for u in range(nde):
                            nc.tensor.matmul(
                                ps[:], w_inT_sb[u][:, t * P:(t + 1) * P],
                                w_sb[name][u][:],
                                start=(u == 0), stop=(u == nde - 1))
                        nc.vector.tensor_copy(wc[name][t][:], ps[:])
                return f
            return [piece(ts_) for ts_ in
                    ([0, 1], [2, 3], [4, 5], [6, 7])]

        def bias_qk(name):
            bvec = {"q": bq, "k": bk}[name]
            bxT = wpool.tile([P, ndh], F32, tag=f"bxT{name}", name=f"bxT{name}")
            nc.sync.dma_start(out=bxT[:], in_=bvec[:].rearrange("(t p) -> p t", p=P))
            bc[name] = persist.tile([P, ndh], F32, tag=f"bc{name}", name=f"bc{name}")
            for t in range(ndh):
                ps = ps_w.tile([P, 1], F32, tag="ps_w", name="ps_w")
                for u in range(nde):
                    nc.tensor.matmul(
                        ps[:], w_sb[name][u][:, t * P:(t + 1) * P],
                        b_inT[:, u:u + 1],
                        start=(u == 0), stop=(u == nde - 1))
                nc.vector.tensor_add(bc[name][:, t:t + 1], ps[:], bxT[:, t:t + 1])

        bcv_row = persist.tile([1, dh], st, tag="bcv", name="bcv_row")

        def bias_v():
            bv_row = wpool.tile([1, dh], F32, tag="bv_row", name="bv_row")
            nc.sync.dma_start(out=bv_row[:], in_=bv[:].unsqueeze(0))
            ps = ps_w.tile([1, dh], F32, tag="ps_w", name="ps_w")
            for u in range(nde):
                nc.tensor.matmul(ps[:], b_inT[:, u:u + 1], w_sb["v"][u][:],
                                 start=(u == 0), stop=(u == nde - 1))
            nc.vector.tensor_add(bcv_row[:], ps[:], bv_row[:])

        def proj_closures(iqc):
            """Per-chunk projection emission, split into PE-sized closures."""
            clos = []
            qf = qf_tiles.get(iqc) or make_qf(iqc, memset=iqc < qbufs)
            xbs = {}

            def load(name, xdram):
                def f():
                    dt_ = qkt if (qk8 and name in ("q", "k")) else st
                    tag = "xb8" if (qk8 and name in ("q", "k")) else "xb"
                    xb = xpool.tile([P, ndm, QC], dt_, tag=tag,
                                    name=f"xb_{name}{iqc}")
                    xbs[name] = xb
                    dmaq().dma_start(
                        out=xb[:],
                        in_=xdram[:, iqc * QC:(iqc + 1) * QC]
                            .rearrange("(u p) s -> p u s", p=P))
                return f

            def qk_part(name, t):
                def f():
                    xb = xbs[name]
                    ps = ps_w.tile([P, QC], F32, tag="ps_w", name="ps_w")
                    if qk8:
                        for i in range(ndm // 2):
                            nc.tensor.matmul(
                                ps[:],
                                wc8[name][i][:, :, t * P:(t + 1) * P],
                                xb[:, 2 * i:2 * i + 2, :],
                                perf_mode=DR0,
                                start=(i == 0), stop=(i == ndm // 2 - 1))
                    else:
                        for u in range(ndm):
                            nc.tensor.matmul(
                                ps[:], wc[name][u][:, t * P:(t + 1) * P],
                                xb[:, u, :], start=(u == 0),
                                stop=(u == ndm - 1))
                    if name == "k":
                        nc.vector.tensor_scalar_add(
                            kT_sb[t][:, iqc * QC:(iqc + 1) * QC], ps[:],
                            bc["k"][:, t:t + 1])
                    else:
                        nc.vector.tensor_scalar_add(
                            qf[t][:, 0, :], ps[:], bc["q"][:, t:t + 1])
                return f

            def v_part(j):
                def f():
                    xb = xbs["v"]
                    ikb = iqc * (QC // P) + j
                    ps = ps_w.tile([P, dh], F32, tag="ps_w", name="ps_w")
                    for u in range(ndm):
                        nc.tensor.matmul(
                            ps[:], xb[:, u, j * P:(j + 1) * P], wc["v"][u][:],
                            start=(u == 0), stop=False)
                    nc.tensor.matmul(ps[:], ones1[:], bcv_row[:],
                                     start=False, stop=True)
                    nc.vector.tensor_copy(
                        v_all[:, ikb, :, 0:DK],
                        ps[:].rearrange("p (h d) -> p h d", h=hloc))
                return f

            clos.append(load("q", qT))
            for t in range(ndh):
                clos.append(qk_part("q", t))
            clos.append(load("k", kT))
            for t in range(ndh):
                clos.append(qk_part("k", t))
            clos.append(load("v", vT))
            for j in range(QC // P):
                clos.append(v_part(j))
            return clos

        # ---- attention ----
        inv_sqrt = 1.0 / math.sqrt(DK)
        DR = mybir.MatmulPerfMode.DoubleRow
        cT_tiles = {}

        def attention_chunk(iqc, fillers=(), mid=None):
            """QK+exp of step i overlaps PV of step i-1; `fillers` (next
            chunk's projection closures) are spread over the early steps;
            `mid` (the previous chunk's yin load) fires ~70% through."""
            qf = qf_tiles[iqc]
            cT = cpool.tile([P, ndh, QC], st, tag="cT", name=f"cT{iqc}")
            cT_tiles[iqc] = cT
            blist = blocks[iqc]
            steps = []
            for h in range(hloc):
                grps = [blist[g0:g0 + GW] for g0 in range(0, len(blist), GW)]
                for g in range(len(grps)):
                    steps.append((h, grps[g], g == 0, g == len(grps) - 1))
            po = {}
            pending = []
            fillers = list(fillers)
            n_steps = len(steps)
            fill_at = {}
            if fillers:
                # spread fillers uniformly across the steps
                for fi in range(len(fillers)):
                    at = (fi * n_steps) // len(fillers)
                    fill_at.setdefault(min(at, n_steps - 1), []).append(
                        fillers[fi])
            mid_at = (6 * n_steps) // 10

            def emit_qk_exp(h, grp):
                # TRI blocks with offset r have their first r*P q-columns
                # fully masked: skip them in both the QK matmul and the exp
                t, off = h // 2, (h % 2) * DK
                q0s = [arg * P if mode == TRI else 0
                       for (ikb, mode, arg) in grp]
                pss = ps_s.tile([P, GW * QC], F32, tag="ps_scores",
                                name="ps_scores")
                for j, (ikb, mode, arg) in enumerate(grp):
                    q0 = q0s[j]
                    kv = kT_sb[t][off:off + DK, ikb * KB:(ikb + 1) * KB]
                    if qk8:
                        nc.tensor.matmul(
                            pss[:, j * QC + q0:(j + 1) * QC],
                            kv.unsqueeze(1).broadcast_to([DK, 2, KB]),
                            qf[t][off:off + DK, :, q0:QC],
                            perf_mode=DR, start=True, stop=True)
                    else:
                        nc.tensor.matmul(pss[:, j * QC + q0:(j + 1) * QC],
                                         kv, qf[t][off:off + DK, 0, q0:QC])
                pt = ppool.tile([P, GW * QC], pvt, tag="p", name="p")
                # one group-wide exp: skipped QK leads hold stale (finite)
                # psum values; the TRI memset below overwrites their pt cols
                nw = len(grp) * QC
                nc.scalar.activation(pt[:, 0:nw], pss[:, 0:nw],
                                     mybir.ActivationFunctionType.Exp,
                                     scale=inv_sqrt)
                for j, (ikb, mode, arg) in enumerate(grp):
                    pj = pt[:, j * QC:(j + 1) * QC]
                    if mode == TRI:
                        r = arg
                        if r > 0:
                            nc.gpsimd.memset(pj[:, 0:r * P], 0.0)
                        nc.vector.tensor_mul(
                            pj[:, r * P:(r + 1) * P],
                            pj[:, r * P:(r + 1) * P], tri[:])
                    elif mode == GEN:
                        nc.vector.tensor_mul(pj[:], pj[:], gen_sb[:, arg, :])
                return pt

            def emit_pv(h, grp, pt, first, last):
                if first:
                    po[h] = ps_o.tile([VW, QC], F32, tag="ps_av",
                                      name="ps_av")
                ikbs = [ikb for (ikb, _, _) in grp]
                if pv8 and len(grp) == 2 and ikbs[1] == ikbs[0] + 1:
                    nc.tensor.matmul(
                        po[h][:], v_all[:, ikbs[0]:ikbs[0] + 2, h, :],
                        pt[:].rearrange("p (two q) -> p two q", two=2),
                        perf_mode=DR,
                        start=first, stop=last)
                else:
                    for j, (ikb, mode, arg) in enumerate(grp):
                        # TRI leads are all-zero in pt: skipping them adds
                        # nothing to columns already fed by FULL blocks
                        q0 = arg * P if mode == TRI else 0
                        if first and j == 0:
                            q0 = 0   # the start matmul must zero the bank
                        nc.tensor.matmul(
                            po[h][:, q0:QC], v_all[:, ikb, h, :],
                            pt[:, j * QC + q0:(j + 1) * QC],
                            start=(first and j == 0),
                            stop=(last and j == len(grp) - 1))
                if last:
                    rec1 = work.tile([1, QC], F32, tag="rec1", name="rec1")
                    nc.vector.reciprocal(rec1[:], po[h][DK:DK + 1, :])
                    recb = work.tile([DK, QC], F32, tag="recb", name="recb")
                    nc.gpsimd.partition_broadcast(recb[:], rec1[:])
                    nc.vector.tensor_mul(
                        cT[(h % 2) * DK:(h % 2) * DK + DK, h // 2, :],
                        po[h][0:DK, :], recb[:])
                    del po[h]

            for i, (h, grp, first, last) in enumerate(steps):
                pt = emit_qk_exp(h, grp)
                pending.append((h, grp, pt, first, last))
                if len(pending) > 3:
                    emit_pv(*pending.pop(0))
                for f in fill_at.get(i, ()):
                    f()
                if mid is not None and i == mid_at:
                    mid()
                    mid = None
            for p_ in pending:
                emit_pv(*p_)
            if mid is not None:
                mid()

        # ---- a2a + output projection ----
        def a2a_chunk(c):
            # cT [128, ndh, QC] -> a2a_in[c or :, c] [8, 256, 64]
            cT = cT_tiles[c]
            dst = a2a_in[c] if chunked_cc else a2a_in[:, c]
            for fh in range(ndh):
                (nc.sync if fh % 2 == 0 else nc.scalar).dma_start(
                    out=dst[:, fh * P:(fh + 1) * P, :]
                        .rearrange("r p j -> p r j"),
                    in_=cT[:, fh, :].rearrange("p (r j) -> p r j", r=nsub))
            if not chunked_cc:
                if c == nqc - 1:
                    if collective:
                        for _ in range(cc_reps):
                            nc.gpsimd.collective_compute(
                                "AllToAll", mybir.AluOpType.bypass,
                                replica_groups=[list(range(N_CORES))],
                                ins=[a2a_in[:].opt()], outs=[a2a_out[:].opt()])
                    else:
                        nc.sync.dma_start(out=a2a_out[:], in_=a2a_in[:])
                return
            if collective:
                for _ in range(cc_reps):
                    nc.gpsimd.collective_compute(
                        "AllToAll", mybir.AluOpType.bypass,
                        replica_groups=[list(range(N_CORES))],
                        ins=[a2a_in[c].opt()], outs=[a2a_out[c].opt()])
            else:
                nc.sync.dma_start(out=a2a_out[c], in_=a2a_in[c])

        yin_tiles = {}

        def yin_load(c):
            yin = ypool.tile([P, nde, P], st, tag="yin", name=f"yin{c}")
            yin_tiles[c] = yin
            half = nsub // 2
            if chunked_cc:
                src = a2a_out[c]
                nc.sync.dma_start(
                    out=yin[:, :, 0:64],
                    in_=src[0:half].rearrange("s (fh p) j -> p (s fh) j", p=P))
                nc.scalar.dma_start(
                    out=yin[:, :, 64:128],
                    in_=src[half:nsub].rearrange("s (fh p) j -> p (s fh) j",
                                                 p=P))
            else:
                yv = yin[:].rearrange("p (s fh) j -> p s fh j", fh=ndh)
                for b0, sl in ((0, slice(0, 64)), (half, slice(64, 128))):
                    for fh in range(ndh):
                        nc.sync.dma_start(
                            out=yv[:, :, fh, sl],
                            in_=a2a_out[b0:b0 + half, c, fh * P:(fh + 1) * P, :]
                                .rearrange("s p j -> p s j"))

        def yproj_mm(c, split_store=False):
            yin = yin_tiles[c]
            ys = ypool.tile([P, dm], F32, tag="ys", name=f"ys{c}")
            for mb in range(dm // QC):
                ps = ps_w.tile([P, QC], F32, tag="ps_w", name="ps_w")
                for u in range(nde):
                    nc.tensor.matmul(
                        ps[:], yin[:, u, :],
                        wo_sb[:, u, mb * QC:(mb + 1) * QC],
                        start=(u == 0), stop=(u == nde - 1))
                nc.vector.tensor_add(ys[:, mb * QC:(mb + 1) * QC], ps[:],
                                     bo_bcast[:, mb * QC:(mb + 1) * QC])
                if split_store:
                    # let the first half's store overlap the second half
                    (nc.sync if mb % 2 == 0 else nc.scalar).dma_start(
                        out=y_out[c * P:(c + 1) * P, mb * QC:(mb + 1) * QC],
                        in_=ys[:, mb * QC:(mb + 1) * QC])
            if not split_store:
                nc.sync.dma_start(out=y_out[c * P:(c + 1) * P, :], in_=ys[:])

        # ---- schedule ----
        # head: score-path (fp8) combine + q/k projections first so the
        # first attention exp fires ~7us in; the bf16 v-combine and chunk-0
        # V projection become PE filler inside attention chunk 0.
        # yproj(c) is deferred into later chunks' attention as PE filler:
        # the final chunks have the most exp work and no projections left.
        if causal_dep:
            pc0 = proj_closures(0)
            pc0[0]()                       # load q chunk 0
            pc0[3]()                       # load k chunk 0
            if qk8:
                load_w_small("q")()
                load_w_small("k")()
                combine_qk8("q")
                bias_qk("q")
                pc0[1](); pc0[2]()         # project q chunk 0
                combine_qk8("k")
                bias_qk("k")
                pc0[4](); pc0[5]()         # project k chunk 0
            else:
                load_w_small("q")()
                load_w_small("k")()
                load_w_inT_bf16()
                for f in combine_bf16_closures("q"):
                    f()
                bias_qk("q")
                pc0[1](); pc0[2]()
                for f in combine_bf16_closures("k"):
                    f()
                bias_qk("k")
                pc0[4](); pc0[5]()
            pc0[6]()                       # load v chunk 0
            load_w_small("v")()
            if qk8:
                load_w_inT_bf16()
            head_fill = combine_bf16_closures("v") + [bias_v]
            load_wo()
            for c in range(nqc):
                fillers = list(head_fill)
                head_fill = []
                if c == 0:
                    fillers += pc0[7:]     # chunk-0 V projection
                if c + 1 < nqc:
                    fillers += list(proj_closures(c + 1))
                if chunked_cc and c == nqc - 1:
                    for cc in range(nqc - 2):
                        fillers.append((lambda c2: lambda: yproj_mm(c2))(cc))
                mid = ((lambda cc: lambda: yin_load(cc))(c - 1)
                       if c > 0 and chunked_cc else None)
                attention_chunk(c, fillers=fillers, mid=mid)
                a2a_chunk(c)
            if chunked_cc:
                yproj_mm(nqc - 2)
                pe_keepwarm(TAIL_WARM)
            else:
                for c in range(nqc - 1):
                    yin_load(c)
                    yproj_mm(c)
        else:
            # general masks: all projections first, then attention
            load_w_small("q")()
            load_w_small("k")()
            load_w_small("v")()
            load_w_inT_bf16()
            if qk8:
                combine_qk8("q")
                combine_qk8("k")
            else:
                for f in combine_bf16_closures("q") + combine_bf16_closures("k"):
                    f()
            bias_qk("q")
            bias_qk("k")
            for f in combine_bf16_closures("v"):
                f()
            bias_v()
            for c in range(nqc):
                for f in proj_closures(c):
                    f()
            load_wo()
            for c in range(nqc):
                mid = (lambda cc: lambda: yin_load(cc))(c - 1) if c > 0 else None
                attention_chunk(c, mid=mid)
                a2a_chunk(c)
                if c > 0:
                    yproj_mm(c - 1)
        yin_load(nqc - 1)
        yproj_mm(nqc - 1, split_store=True)

    nc.compile()
    return nc


# ------------------------------------------------------------------
_CACHE = {}


def _get_compiled(plan_key, blocks, n_gen, mm):
    if plan_key not in _CACHE:
        nc = build_mha(blocks, n_gen, mm=mm)
        nc.m = get_hw_module(nc.m)
        _CACHE[plan_key] = nc
    return _CACHE[plan_key]


def make_in_maps(q, k, v, mask, W_in, b_in, Wq, bq, Wk, bk, Wv, bv, Wo, bo,
                 blocks=None, n_gen=None, gen_tiles=None):
    if blocks is None:
        blocks, n_gen, gen_tiles = make_plan(mask)
    bf16 = mybir.dt.np(BF16)
    fp8 = mybir.dt.np(FP8)
    dh = DE // 4
    tb = lambda a: np.ascontiguousarray(np.asarray(a).T).astype(bf16)
    cb = lambda a: np.ascontiguousarray(np.asarray(a)).astype(bf16)
    t8 = lambda a: np.ascontiguousarray(np.asarray(a).T).astype(fp8)
    c8 = lambda a: np.ascontiguousarray(np.asarray(a)).astype(fp8)
    in_maps = []
    for c in range(N_CORES):
        b, g = c // 4, c % 4
        sl = slice(g * dh, (g + 1) * dh)
        mt = (gen_tiles[b] if n_gen else
              np.zeros((1, KB, QC), np.int32))
        qk8 = MM_MODE in ("fp8", "fp8qk")
        tq = t8 if qk8 else tb
        w8p = np.concatenate(
            [np.asarray(Wq)[:, sl], np.asarray(Wk)[:, sl],
             np.ascontiguousarray(np.asarray(W_in).T)], axis=1)
        in_maps.append({
            "qT": tq(q[b]), "kT": tq(k[b]), "vT": tb(v[b]),
            "w_inT": tb(W_in),
            "w8pack": c8(w8p),
            "wq": cb(Wq[:, sl]),
            "wk": cb(Wk[:, sl]),
            "wv": cb(Wv[:, sl]),
            "wo": cb(Wo),
            "b_in": np.asarray(b_in).astype(bf16),
            "bq": np.ascontiguousarray(np.asarray(bq)[sl]),
            "bk": np.ascontiguousarray(np.asarray(bk)[sl]),
            "bv": np.ascontiguousarray(np.asarray(bv)[sl]),
            "bo": np.asarray(bo),
            "m_tiles": mt,
        })
    return in_maps, blocks, n_gen


def assemble(results):
    out = np.empty((B, S, DM), np.float32)
    for core in range(N_CORES):
        y = results[core]["y_out"]            # [nqc*128, DM]
        for c in range(S // QC):
            for b in range(B):
                out[b, c * QC + core * 64:c * QC + (core + 1) * 64, :] = \
                    y[c * P + b * 64:c * P + (b + 1) * 64, :]
    return out


MM_MODE = "fp8qk"


def kernel(**inputs):
    mask = inputs["mask"]
    blocks, n_gen, gen_tiles = make_plan(np.asarray(mask))
    plan_key = (str(blocks), n_gen, MM_MODE)
    nc = _get_compiled(plan_key, blocks, n_gen, MM_MODE)
    in_maps, _, _ = make_in_maps(
        inputs["q"], inputs["k"], inputs["v"], mask,
        inputs["W_in"], inputs["b_in"], inputs["Wq"], inputs["bq"],
        inputs["Wk"], inputs["bk"], inputs["Wv"], inputs["bv"],
        inputs["Wo"], inputs["bo"],
        blocks=blocks, n_gen=n_gen, gen_tiles=gen_tiles)
    res = bass_utils.run_bass_kernel_spmd(nc, in_maps,
                                          core_ids=list(range(N_CORES)))
    return assemble(res.results)
